# revision 24
# baseline (speedup 1.0000x reference)
"""Trainium2 Bass kernel for nn_FCGF_point_att3_sft_7000 (8 NeuronCores).

Model: pointwise attention MLP (32->16->8->1, BN+relu, BN stats over the full
512000-point batch), per-segment softmax over 2000 points, attention-weighted
pooling to [256, 64000], FC head 64000->1024->256 (BN+relu, stats over the
256-segment batch), final L2 row-normalize.

Sharding: points-within-segment. Core c owns points p in [250c, 250(c+1)) of
every segment. Stage A is data-parallel over points with AllGather'd BN stats;
fc1 is contraction-sharded (each core owns 8000 of the 64000 inputs and the
matching fw1 rows) with the output in [segs, feats] orientation so a
ReduceScatter over segments hands each core 32 complete segments; the softmax
denominators ride the same collective as an extra column. The whole tail
(BN1, fc2, BN2, L2-normalize) then runs locally per core on its 32 segments,
with two tiny AllGathers for the cross-segment BN statistics; each core emits
its own [32, 256] slab of the output.

Stage-A layout: "quartered" A-orientation. x.T is [128, 16000] with the
channels of free-quarter a on partitions [32a, 32a+32). Matmuls use
tile_position=(32a, 32a) so outputs land on partitions 32a+ch and every
eviction / BN / softmax op runs 128 partitions wide. Weight tiles are
zero-padded to M=32 so all PSUM rows are defined.

Training-mode BN is shift-invariant => conv/linear biases (b1,b2,b3,fb1,fb2)
drop out exactly; they are accepted and ignored.
"""

import sys

sys.path.insert(0, "/opt/trn_rl_repo")

import numpy as np

import concourse.bass as bass
import concourse.tile as tile
from concourse import mybir
from concourse.masks import make_identity

B = 256
P = 2000
C = 32
NCORES = 8
PL = P // NCORES           # 250
PH = PL // 2               # 125
NPTS = B * PL              # 64000 points per core
QF = NPTS // 4             # 16000 per quarter
NCH = 500                  # stage-A free chunk
NCHUNK = QF // NCH         # 32
SEGC = B // NCORES         # 32 segments per core after the ReduceScatter
RSW = 1028                 # rs payload width: 1024 feats + z + 3 pad
EPS_BN = 1e-5
F32 = mybir.dt.float32
BF16 = mybir.dt.float16  # fp16: same speed as bf16, 8x lower rounding noise
F8 = mybir.dt.float8e3   # e3m4: fc1 weight stream at half the HBM bytes
FW_SCALE = 64.0          # fw1*64 fits e3m4 range; undone in the z-normalize
RG = [list(range(NCORES))]
AF = mybir.ActivationFunctionType

_cache = {}


# ------------------------------------------------------------------ walrus fix
def _install_walrus_patch():
    """This container's walrus accepts only ONE semaphore wait per instruction.
    Spread Tile's end-of-kernel drain waits across single-wait nops, and split
    any instruction carrying >1 waits onto same-engine carrier nops."""
    if _cache.get("patched"):
        return
    from concourse.vector_clock import ScopedClock, VectorClock

    counter = [0]

    def split_waits(nc):
        for bb in nc.main_func.blocks:
            out = []
            changed = False
            for ins in bb.instructions:
                si = ins.sync_info
                waits = list(si.on_wait) if si and si.on_wait else []
                if len(waits) > 1:
                    changed = True
                    for w in waits[:-1]:
                        counter[0] += 1
                        out.append(mybir.InstNoOp(
                            name=f"I-wsplit-{counter[0]}",
                            engine=ins.engine, ins=[], outs=[],
                            sync_info=mybir.SyncInfo(on_wait=[w], on_update=[]),
                            bass_nofuse=True))
                    si.on_wait = waits[-1:]
                out.append(ins)
            if changed:
                try:
                    bb.instructions = out
                except Exception:
                    bb.instructions.clear()
                    for x in out:
                        bb.instructions.append(x)

    def _patched(self, tick_clock, wait_clock):
        nc = self.nc
        gc = tick_clock.global_clock
        n = len(gc)
        for i in range(n):
            if gc[i] > 0:
                vec = [0] * n
                vec[i] = gc[i]
                nop = nc.sync.nop(nofuse=True, hint=f"drain_wait_p{i}")
                wait_clock.add_sem_waits(
                    nop.ins, ScopedClock({None: VectorClock(vec)}))
        nc.sync.drain()
        nc.all_engine_barrier()
        assert self.sems is not None
        popped = nc._tile_sem_poison_stack.pop()
        assert popped is self._sem_poison
        nc.clear_and_free_semaphores(list(self.sems.allocated().values()))
        nc.all_engine_barrier()
        split_waits(nc)

    tile.TileContext._drain_and_barrier = _patched
    _cache["patched"] = True


# ------------------------------------------------------------------ bass build
def _build():
    _install_walrus_patch()
    nc = bass.Bass()

    def ein(name, shape, dt):
        return nc.dram_tensor(name, shape, dt, kind="ExternalInput")

    d = {}
    d["xA4"] = ein("xA4", [128, QF], BF16)
    d["xB"] = ein("xB", [PH, C * 2 * B], BF16)
    d["w1D"] = ein("w1D", [128, 128], BF16)
    d["w2D"] = ein("w2D", [128, 128], BF16)
    d["w3D"] = ein("w3D", [128, 128], BF16)
    for n in ("g1q", "be1q", "g2q", "be2q"):
        d[n] = ein(n, [128, 1], F32)
    d["g3s"] = ein("g3s", [1, 1], F32)
    d["be3s"] = ein("be3s", [1, 1], F32)
    d["f1"] = ein("f1", [128, 16], F32)
    d["ft1"] = ein("ft1", [16, 128], F32)
    d["f2"] = ein("f2", [128, 8], F32)
    d["ft2"] = ein("ft2", [8, 128], F32)
    d["f8_16"] = ein("f8_16", [128, 16], F32)
    d["f8_8"] = ein("f8_8", [64, 8], F32)
    d["fw1t"] = ein("fw1t", [PH, C * 2, 1024], F8)
    d["fw2s"] = ein("fw2s", [128, 8 * 256], BF16)
    d["fg1t"] = ein("fg1t", [128, 8], F32)
    d["fbe1t"] = ein("fbe1t", [128, 8], F32)
    d["fg2r"] = ein("fg2r", [1, 256], F32)
    d["fbe2r"] = ein("fbe2r", [1, 256], F32)
    d["out_final"] = nc.dram_tensor("out_final", [SEGC, 256], F32,
                                    kind="ExternalOutput")
    # collective bounce buffers
    d["warm_i"] = nc.dram_tensor("warm_i", [16, 4], F32)
    d["warm_o"] = nc.dram_tensor("warm_o", [16, 4], F32)
    d["st1_i"] = nc.dram_tensor("st1_i", [16, 2], F32)
    d["st1_o"] = nc.dram_tensor("st1_o", [128, 2], F32)
    d["st2_i"] = nc.dram_tensor("st2_i", [8, 2], F32)
    d["st2_o"] = nc.dram_tensor("st2_o", [64, 2], F32)
    d["st3_i"] = nc.dram_tensor("st3_i", [1, 2], F32)
    d["st3_o"] = nc.dram_tensor("st3_o", [8, 2], F32)
    d["rs_iA"] = nc.dram_tensor("rs_iA", [B, RSW], BF16)
    d["rs_oA"] = nc.dram_tensor("rs_oA", [SEGC, RSW], BF16)
    d["rs_iB"] = nc.dram_tensor("rs_iB", [B, RSW], BF16)
    d["rs_oB"] = nc.dram_tensor("rs_oB", [SEGC, RSW], BF16)
    d["ag4_i"] = nc.dram_tensor("ag4_i", [128, 16], F32)
    d["ag4_o"] = nc.dram_tensor("ag4_o", [128 * NCORES, 16], F32)
    d["ag5_i"] = nc.dram_tensor("ag5_i", [1, 512], F32)
    d["ag5_o"] = nc.dram_tensor("ag5_o", [NCORES, 512], F32)

    with tile.TileContext(nc) as tc:
        _body(nc, tc, d)
    return nc


def _mkstats(nc, pool, mv, count, name):
    """mv [p,2]=(mean,var) -> (sum,sumsq) [p,2]."""
    p = mv.shape[0]
    ss = pool.tile([p, 2], F32, tag=f"ss_{name}")
    nc.vector.tensor_mul(ss[:, 1:2], mv[:, 0:1], mv[:, 0:1])
    nc.vector.tensor_add(ss[:, 1:2], ss[:, 1:2], mv[:, 1:2])
    nc.scalar.mul(ss[:, 0:1], mv[:, 0:1], float(count))
    nc.scalar.mul(ss[:, 1:2], ss[:, 1:2], float(count))
    return ss


def _mv_from_ss(nc, pool, ss, count, name):
    """(sum,sumsq) [p,2] over count -> (mean, rstd) [p,2]."""
    p = ss.shape[0]
    mr = pool.tile([p, 2], F32, tag=f"mr_{name}")
    epst = pool.tile([p, 1], F32, tag=f"eps_{name}")
    nc.vector.memset(epst[:], EPS_BN)
    nc.scalar.mul(mr[:, 0:1], ss[:, 0:1], 1.0 / count)
    nc.scalar.mul(mr[:, 1:2], ss[:, 1:2], 1.0 / count)
    m2 = pool.tile([p, 1], F32, tag=f"m2_{name}")
    nc.vector.tensor_mul(m2[:], mr[:, 0:1], mr[:, 0:1])
    nc.vector.tensor_sub(mr[:, 1:2], mr[:, 1:2], m2[:])
    nc.scalar.activation(mr[:, 1:2], mr[:, 1:2], AF.Sqrt, bias=epst[:])
    nc.vector.reciprocal(mr[:, 1:2], mr[:, 1:2])
    return mr


def _scale_bias(nc, pool, mrq, g, be, name):
    """scale = g*rstd ; bias = be - scale*mean  (all [p,1] per-partition)."""
    p = mrq.shape[0]
    sc = pool.tile([p, 1], F32, tag=f"sc_{name}")
    bi = pool.tile([p, 1], F32, tag=f"bi_{name}")
    nc.vector.tensor_mul(sc[:], g[:], mrq[:, 1:2])
    nc.vector.tensor_mul(bi[:], sc[:], mrq[:, 0:1])
    nc.vector.tensor_sub(bi[:], be[:], bi[:])
    return sc, bi


def _body(nc, tc, d):
    # collective warmup first. warm_i is never written (contents irrelevant),
    # so the op has NO dependencies and the ~55us ncfw startup begins at t=0,
    # overlapping the whole front of the kernel.
    nc.gpsimd.collective_compute(
        "AllReduce", mybir.AluOpType.add, replica_groups=RG,
        ins=[d["warm_i"][:]], outs=[d["warm_o"][:]])
    sing_cm = tc.tile_pool(name="sing", bufs=1)
    big_cm = tc.tile_pool(name="big", bufs=1)
    work_cm = tc.tile_pool(name="work", bufs=1)
    psA_cm = tc.tile_pool(name="psA", bufs=4, space="PSUM")
    psT_cm = tc.tile_pool(name="psT", bufs=2, space="PSUM")
    psS_cm = tc.tile_pool(name="psS", bufs=2, space="PSUM")
    sing = sing_cm.__enter__(); big = big_cm.__enter__()
    work = work_cm.__enter__()
    fw1p_cm = tc.tile_pool(name="fw1p", bufs=8)
    fw1p = fw1p_cm.__enter__()
    psA = psA_cm.__enter__(); psT = psT_cm.__enter__()
    psS = psS_cm.__enter__()

    # ---------------- constants
    def load(name, shape, dt=F32, pool=sing):
        t = pool.tile(shape, dt, tag=name)
        nc.sync.dma_start(t[:], d[name][:])
        return t

    w1D = load("w1D", [128, 128], BF16)
    w2D = load("w2D", [128, 128], BF16)
    w3D = load("w3D", [128, 128], BF16)
    f1s = load("f1", [128, 16])
    ft1s = load("ft1", [16, 128])
    f2s = load("f2", [128, 8])
    ft2s = load("ft2", [8, 128])
    f8_16s = load("f8_16", [128, 16])
    f8_8s = load("f8_8", [64, 8])
    g1 = load("g1q", [128, 1]); be1 = load("be1q", [128, 1])
    g2 = load("g2q", [128, 1]); be2 = load("be2q", [128, 1])
    g3 = load("g3s", [1, 1]); be3 = load("be3s", [1, 1])
    ones128 = sing.tile([128, 1], F32)
    nc.vector.memset(ones128[:], 1.0)
    ones8 = sing.tile([8, 1], F32)
    nc.vector.memset(ones8[:], 1.0)
    ones1x = sing.tile([1, 128], F32)
    nc.vector.memset(ones1x[:], 1.0)
    ones32h = sing.tile([32, 1], BF16)
    nc.vector.memset(ones32h[:], 1.0)
    ones1x32h = sing.tile([1, 32], BF16)
    nc.vector.memset(ones1x32h[:], 1.0)
    ident = sing.tile([128, 128], F32)
    make_identity(nc, ident[:])
    identh = sing.tile([SEGC, SEGC], BF16)
    make_identity(nc, identh[:])

    # ---------------- big loads
    xa = big.tile([128, QF], BF16, tag="slotA")       # slot A: xa -> y2 -> y3q
    nc.sync.dma_start(xa[:], d["xA4"][:])
    xb = big.tile([PH, C * 2 * B], BF16, tag="xb")
    nc.sync.dma_start(xb[:], d["xB"][:])
    xbv = xb[:].rearrange("p (c h s) -> p c h s", c=C, h=2, s=B)

    # fc1 weight prefetch: pool entered at the top so its slots exist from
    # t=0 and the 16.4MB stream overlaps all of stage A. 2-engine rotation.
    FW_CHUNKS = [8] * 8
    fwtiles = []
    _dge = [nc.sync, nc.scalar]
    _off = 0
    for gblk, nits in enumerate(FW_CHUNKS):
        fwt = fw1p.tile([PH, 8, 1024], F8, tag="fw", name=f"fw_{gblk}")
        _dge[gblk % 2].dma_start(fwt[:, :nits, :],
                                 d["fw1t"][:, _off : _off + nits, :])
        fwtiles.append((fwt, _off, nits))
        _off += nits
    fw2s = sing.tile([128, 8 * 256], BF16, tag="fw2s")
    nc.sync.dma_start(fw2s[:], d["fw2s"][:])
    fw2sv = fw2s[:].rearrange("p (t o) -> p t o", t=8)

    def layer_mms(ps, wD, krows, rhs_src, sl):
        nc.tensor.matmul(ps[:], wD[:], rhs_src[:, sl], start=True, stop=True)

    def stage_layer(rhs_src, wT, krows, fold, foldT, f8fold, st_i, st_o,
                    gq, beq, count_local, name, out_tag, wnext=None):
        """Single-pass layer: matmuls -> evict y fp16 (+bn_stats from y),
        fold+AllGather stats. The BN scale is folded into the next layer's
        weights (gamma>0), so the relu pass is a per-partition bias-shift
        split across scalar/vector/gpsimd."""
        y = big.tile([128, QF], BF16, tag=out_tag, name=f"y_{name}")
        stat = work.tile([128, NCHUNK, 6], F32, tag=f"stat_{name}")
        for j in range(NCHUNK):
            ps = psA.tile([128, NCH], F32, tag="psA", name=f"ps_{name}_{j}")
            layer_mms(ps, wT, krows, rhs_src, slice(j * NCH, (j + 1) * NCH))
            nc.scalar.copy(y[:, j * NCH : (j + 1) * NCH], ps[:])
            nc.vector.bn_stats(stat[:, j, :], y[:, j * NCH : (j + 1) * NCH])
        mv = work.tile([128, 2], F32, tag=f"mv_{name}")
        nc.vector.bn_aggr(mv[:], stat[:])
        ss = _mkstats(nc, work, mv, count_local, name)
        nfold = fold.shape[1]
        psf = psS.tile([128, 2], F32, tag="small", name=f"psf_{name}")
        nc.tensor.matmul(psf[:nfold, :], fold[:], ss[:], start=True, stop=True)
        sbf = work.tile([nfold, 2], F32, tag=f"sbf_{name}")
        nc.scalar.copy(sbf[:], psf[:nfold, :])
        nc.gpsimd.dma_start(st_i[:], sbf[:])
        nc.gpsimd.collective_compute(
            "AllGather", mybir.AluOpType.bypass, replica_groups=RG,
            ins=[st_i[:]], outs=[st_o[:]])
        agg = work.tile([nfold * NCORES, 2], F32, tag=f"agg_{name}")
        nc.gpsimd.dma_start(agg[:], st_o[:])
        psg = psS.tile([128, 2], F32, tag="small", name=f"psg_{name}")
        nc.tensor.matmul(psg[:nfold, :], f8fold[:], agg[:], start=True,
                         stop=True)
        ssg = work.tile([nfold, 2], F32, tag=f"ssg_{name}")
        nc.scalar.copy(ssg[:], psg[:nfold, :])
        mr = _mv_from_ss(nc, work, ssg, B * P, name)
        psb = psS.tile([128, 2], F32, tag="small", name=f"psb_{name}")
        nc.tensor.matmul(psb[:], foldT[:], mr[:], start=True, stop=True)
        mrq = work.tile([128, 2], F32, tag=f"mrq_{name}")
        nc.scalar.copy(mrq[:], psb[:])
        sc, bi = _scale_bias(nc, work, mrq, gq, beq, name)
        # fold the BN scale into the next layer's weights (gamma>0); the relu
        # pass becomes a bias-shift, split across scalar/vector/gpsimd. Clamp
        # sc away from 0 first: padded partition slots have gamma=0 and the
        # bare reciprocal would make bip = 0*inf = NaN there.
        isc = work.tile([128, 1], F32, tag=f"isc_{name}")
        nc.vector.tensor_scalar_max(isc[:], sc[:], 1e-30)
        nc.vector.reciprocal(isc[:], isc[:])
        bip = work.tile([128, 1], F32, tag=f"bip_{name}")
        nc.vector.tensor_mul(bip[:], bi[:], isc[:])
        wnf = None
        if wnext is not None:
            wnf = sing.tile([128, 128], BF16, tag=f"wnf_{name}")
            nc.vector.tensor_scalar_mul(wnf[:], wnext[:], sc[:])
        for j in range(NCHUNK):
            sl = slice(j * NCH, (j + 1) * NCH)
            if j % 2 == 0:
                nc.scalar.activation(y[:, sl], y[:, sl], AF.Relu, bias=bip[:])
            else:
                nc.vector.tensor_scalar(y[:, sl], y[:, sl], bip[:], 0.0,
                                        mybir.AluOpType.add,
                                        mybir.AluOpType.max)
        return y, wnf

    # ---------------- stage A layers 1 & 2
    h1, w2f = stage_layer(xa, w1D, 32, f1s, ft1s, f8_16s,
                          d["st1_i"], d["st1_o"], g1, be1, QF, "l1", "slotB",
                          wnext=w2D)
    # h2 reuses slot A (xa dead after L1 matmuls)
    h2, w3f = stage_layer(h1, w2f, 16, f2s, ft2s, f8_8s,
                          d["st2_i"], d["st2_o"], g2, be2, QF, "l2", "slotA",
                          wnext=w3D)

    # ---------------- stage A layer 3: scores straight from PSUM into
    # scoreS [128 segs, 2, 250] via per-chunk repack DMAs (rows {32a} real;
    # chunk j of quarter a covers segments 64a+2j..+1)
    y3q = big.tile([128, QF], BF16, tag="slotB", name="y3q")
    for j in range(NCHUNK):
        ps = psA.tile([128, NCH], F32, tag="psA", name=f"ps_l3_{j}")
        layer_mms(ps, w3f, 8, h2, slice(j * NCH, (j + 1) * NCH))
        nc.scalar.copy(y3q[:, j * NCH : (j + 1) * NCH], ps[:])
    scoreS = big.tile([128, 2, PL], BF16, tag="scoreS")
    for a in range(4):
        nc.sync.dma_start(
            scoreS[64 * (a % 2) : 64 * (a % 2) + 64, a // 2, :],
            y3q[32 * a : 32 * a + 1, :])
    # BN3 stats over all segments/points (all partitions real)
    stat3 = work.tile([128, 2, 6], F32, tag="stat3")
    nc.vector.bn_stats(stat3[:, 0, :], scoreS[:, 0, :])
    nc.vector.bn_stats(stat3[:, 1, :], scoreS[:, 1, :])
    mv3 = work.tile([128, 2], F32, tag="mv3")
    nc.vector.bn_aggr(mv3[:], stat3[:])
    ss3 = _mkstats(nc, work, mv3, 2 * PL, "l3")
    psf3 = psS.tile([128, 2], F32, tag="small", name="psf3")
    nc.tensor.matmul(psf3[:1, :], ones128[:], ss3[:], start=True, stop=True)
    sbf3 = work.tile([1, 2], F32, tag="sbf3")
    nc.scalar.copy(sbf3[:], psf3[:1, :])
    nc.gpsimd.dma_start(d["st3_i"][:], sbf3[:])
    nc.gpsimd.collective_compute(
        "AllGather", mybir.AluOpType.bypass, replica_groups=RG,
        ins=[d["st3_i"][:]], outs=[d["st3_o"][:]])
    agg3 = work.tile([8, 2], F32, tag="agg3")
    nc.gpsimd.dma_start(agg3[:], d["st3_o"][:])
    psg3 = psS.tile([128, 2], F32, tag="small", name="psg3")
    nc.tensor.matmul(psg3[:1, :], ones8[:], agg3[:], start=True, stop=True)
    ssg3 = work.tile([1, 2], F32, tag="ssg3")
    nc.scalar.copy(ssg3[:], psg3[:1, :])
    mr3 = _mv_from_ss(nc, work, ssg3, B * P, "l3")
    scb1 = work.tile([1, 2], F32, tag="scb1")
    nc.vector.tensor_mul(scb1[:, 0:1], g3[:], mr3[:, 1:2])
    nc.vector.tensor_mul(scb1[:, 1:2], scb1[:, 0:1], mr3[:, 0:1])
    nc.vector.tensor_sub(scb1[:, 1:2], be3[:], scb1[:, 1:2])
    psb3 = psS.tile([128, 2], F32, tag="small", name="psb3")
    nc.tensor.matmul(psb3[:], ones1x[:], scb1[:], start=True, stop=True)
    scb = work.tile([128, 2], F32, tag="scb")
    nc.scalar.copy(scb[:], psb3[:])
    # relu(BN3) in place, then exp
    expS = big.tile([128, 2, PL], F32, tag="expS")
    for tt in range(2):
        nc.scalar.activation(scoreS[:, tt, :], scoreS[:, tt, :], AF.Relu,
                             bias=scb[:, 1:2], scale=scb[:, 0:1])
        nc.scalar.activation(expS[:, tt, :], scoreS[:, tt, :], AF.Exp)
        # partial softmax denominators
    zloc = work.tile([128, 8], F32, tag="zloc")
    nc.vector.memset(zloc[:], 0.0)
    nc.vector.reduce_sum(zloc[:, 0:1], expS[:, 0, :], axis=mybir.AxisListType.X)
    nc.vector.reduce_sum(zloc[:, 4:5], expS[:, 1, :], axis=mybir.AxisListType.X)
    zpad = work.tile([128, 8], BF16, tag="zpad")
    nc.scalar.copy(zpad[:], zloc[:])
    # z (and zero pad) into columns 1024:1028 of rs_i: row (t p) = seg t*128+p
    nc.sync.dma_start(
        d["rs_iA"][:, 1024:1028].rearrange("(t p) c -> p t c", t=2),
        zpad[:].rearrange("p (t c) -> p t c", t=2))
    zero8 = work.tile([128, 8], BF16, tag="zero8")
    nc.vector.memset(zero8[:], 0.0)
    nc.sync.dma_start(
        d["rs_iB"][:, 1024:1028].rearrange("(t p) c -> p t c", t=2),
        zero8[:].rearrange("p (t c) -> p t c", t=2))
    # expT [125, 2, 256]: PE-transpose expS halves
    expT = big.tile([PH, 2, 256], BF16, tag="expT")
    for h in range(2):
        for tt in range(2):
            pt_ps = psT.tile([128, 128], F32, tag="psT")
            nc.tensor.transpose(pt_ps[:PH, :],
                                expS[:, tt, h * PH : h * PH + PH], ident[:])
            nc.scalar.copy(expT[:, h, tt * 128 : tt * 128 + 128],
                           pt_ps[:PH, :])

    psS_cm.__exit__(None, None, None)
    psT_cm.__exit__(None, None, None)
    psA_cm.__exit__(None, None, None)

    # ---------------- FC1 (contraction-sharded, out [256 segs, 1024] partial)
    # lhsT = pt seg-halves [125, 128]; rhs = fw1 it-chunk [125, 512]-halves.
    psF_cm = tc.tile_pool(name="psF", bufs=1, space="PSUM")
    ptp_cm = tc.tile_pool(name="ptp", bufs=4)
    psF = psF_cm.__enter__()
    ptp = ptp_cm.__enter__()
    r1ps = [psF.tile([128, 512], F32, name=f"r1ps_{m}", tag=f"r1_{m}")
            for m in range(4)]
    NIT = C * 2
    HIT = NIT // 2

    def fc1_evict(gen, rs_i):
        _dmaeng = [nc.sync, nc.scalar, nc.gpsimd, nc.sync]
        for m in range(4):
            s, f = m // 2, m % 2
            r1sb = big.tile([128, 512], BF16, tag=f"r1sb{gen}",
                            name=f"r1sb{gen}_{m}", bufs=2)
            if m % 2 == 0:
                nc.scalar.copy(r1sb[:], r1ps[m][:])
            else:
                nc.vector.tensor_copy(r1sb[:], r1ps[m][:])
            _dmaeng[m].dma_start(
                rs_i[s * 128 : (s + 1) * 128, f * 512 : (f + 1) * 512],
                r1sb[:])

    for ch in range(C):
        for h in range(2):
            it = ch * 2 + h
            gi = 0
            while not (fwtiles[gi][1] <= it < fwtiles[gi][1] + fwtiles[gi][2]):
                gi += 1
            fw = fwtiles[gi][0][:, it - fwtiles[gi][1], :]
            pt = ptp.tile([PH, 256], BF16, tag="pt", name=f"pt_{it}")
            nc.vector.tensor_mul(pt[:], xbv[:, ch, h, :], expT[:, h, :])
            for s in range(2):
                lhsT = pt[:, s * 128 : (s + 1) * 128]
                for f in range(2):
                    nc.tensor.matmul(
                        r1ps[s * 2 + f][:, :], lhsT,
                        fw[:, f * 512 : (f + 1) * 512],
                        start=(it in (0, HIT)), stop=(it in (HIT - 1, NIT - 1)))
            if it == HIT - 1:
                # first-half partials ship out mid-FC1 so RS#1 overlaps the
                # second half of the contraction.
                fc1_evict("A", d["rs_iA"])
                nc.gpsimd.collective_compute(
                    "ReduceScatter", mybir.AluOpType.add, replica_groups=RG,
                    ins=[d["rs_iA"][:]], outs=[d["rs_oA"][:]])
    fc1_evict("B", d["rs_iB"])
    nc.gpsimd.collective_compute(
        "ReduceScatter", mybir.AluOpType.add, replica_groups=RG,
        ins=[d["rs_iB"][:]], outs=[d["rs_oB"][:]])

    ptp_cm.__exit__(None, None, None)
    psF_cm.__exit__(None, None, None)
    fw1p_cm.__exit__(None, None, None)

    # ---------------- tail: this core owns segments [32c, 32c+32), complete.
    ps2_cm = tc.tile_pool(name="ps2", bufs=2, space="PSUM")
    ps2 = ps2_cm.__enter__()

    r1A = big.tile([SEGC, RSW], BF16, tag="r1A")
    nc.sync.dma_start(r1A[:], d["rs_oA"][:])
    r1B = big.tile([SEGC, RSW], BF16, tag="r1B")
    nc.sync.dma_start(r1B[:], d["rs_oB"][:])
    r1 = big.tile([SEGC, RSW], BF16, tag="r1")
    nc.vector.tensor_add(r1[:], r1A[:], r1B[:])
    zinv = work.tile([SEGC, 1], F32, tag="zinv")
    nc.scalar.mul(zinv[:], r1[:, 1024:1025], FW_SCALE)
    nc.vector.reciprocal(zinv[:], zinv[:])
    r1z = big.tile([SEGC, 1024], BF16, tag="r1z")
    nc.scalar.activation(r1z[:], r1[:, 0:1024], AF.Copy, scale=zinv[:])
    # transpose to 8 feature tiles [128, 32]; BN1 stats over local 32 segs
    r1T = big.tile([128, 8, SEGC], BF16, tag="r1T")
    stat8 = work.tile([128, 8, 6], F32, tag="stat8")
    for t in range(8):
        pt_ps = ps2.tile([128, SEGC], BF16, tag="tp", name=f"tp_{t}")
        nc.tensor.transpose(pt_ps[:], r1z[:, t * 128 : (t + 1) * 128],
                            identh[:])
        nc.scalar.copy(r1T[:, t, :], pt_ps[:])
        nc.vector.bn_stats(stat8[:, t, :], pt_ps[:])
    mv8 = work.tile([128, 8, 2], F32, tag="mv8")
    for t in range(8):
        nc.vector.bn_aggr(mv8[:, t, :], stat8[:, t : t + 1, :])
    ss8 = work.tile([128, 8, 2], F32, tag="ss8")
    nc.vector.tensor_mul(ss8[:, :, 1:2], mv8[:, :, 0:1], mv8[:, :, 0:1])
    nc.vector.tensor_add(ss8[:, :, 1:2], ss8[:, :, 1:2], mv8[:, :, 1:2])
    nc.scalar.mul(ss8[:, :, 0:1], mv8[:, :, 0:1], float(SEGC))
    nc.scalar.mul(ss8[:, :, 1:2], ss8[:, :, 1:2], float(SEGC))
    nc.gpsimd.dma_start(d["ag4_i"][:], ss8[:].rearrange("p t u -> p (t u)"))
    nc.gpsimd.collective_compute(
        "AllGather", mybir.AluOpType.bypass, replica_groups=RG,
        ins=[d["ag4_i"][:]], outs=[d["ag4_o"][:]])
    agg4 = work.tile([128, 8, 16], F32, tag="agg4")
    _age = [nc.sync, nc.scalar, nc.gpsimd]
    for c in range(8):
        _age[c % 3].dma_start(agg4[:, c, :],
                              d["ag4_o"][c * 128 : (c + 1) * 128, :])
    g8 = work.tile([128, 8, 2], F32, tag="g8")
    nc.vector.reduce_sum(g8[:], agg4[:].rearrange("p c w -> p w c"),
                         axis=mybir.AxisListType.X)
    # mean/rstd per feature ([128, 8] per-partition per-tile)
    epsf = work.tile([128, 1], F32, tag="epsf")
    nc.vector.memset(epsf[:], EPS_BN)
    nc.scalar.mul(g8[:, :, 0:1], g8[:, :, 0:1], 1.0 / B)
    nc.scalar.mul(g8[:, :, 1:2], g8[:, :, 1:2], 1.0 / B)
    m2t = work.tile([128, 8], F32, tag="m2t")
    nc.vector.tensor_mul(m2t[:], g8[:, :, 0:1], g8[:, :, 0:1])
    nc.vector.tensor_sub(g8[:, :, 1:2], g8[:, :, 1:2], m2t[:])
    nc.scalar.activation(g8[:, :, 1:2], g8[:, :, 1:2], AF.Sqrt, bias=epsf[:])
    nc.vector.reciprocal(g8[:, :, 1:2], g8[:, :, 1:2])
    fg1t = load("fg1t", [128, 8], pool=work)
    fbe1t = load("fbe1t", [128, 8], pool=work)
    sc1 = work.tile([128, 8], F32, tag="sc1")
    bi1 = work.tile([128, 8], F32, tag="bi1")
    nc.vector.tensor_mul(sc1[:], fg1t[:], g8[:, :, 1:2])
    nc.vector.tensor_mul(bi1[:], sc1[:], g8[:, :, 0:1])
    nc.vector.tensor_sub(bi1[:], fbe1t[:], bi1[:])
    r1Tr = big.tile([128, 8, SEGC], BF16, tag="r1Tr")
    for t in range(8):
        nc.scalar.activation(r1Tr[:, t, :], r1T[:, t, :], AF.Relu,
                             bias=bi1[:, t : t + 1], scale=sc1[:, t : t + 1])
    # FC2: out [32 segs, 256] complete (contraction over 1024 feats, local)
    ps_r2 = ps2.tile([SEGC, 256], F32, tag="r2", bufs=1)
    for t in range(8):
        nc.tensor.matmul(ps_r2[:], r1Tr[:, t, :], fw2sv[:, t, :],
                         start=(t == 0), stop=(t == 7))
    r2st = big.tile([SEGC, 512], BF16, tag="r2st")
    nc.scalar.copy(r2st[:, 0:256], ps_r2[:])
    nc.scalar.activation(r2st[:, 256:512], ps_r2[:], AF.Square)
    ps_s5 = ps2.tile([1, 512], F32, tag="s5", bufs=1)
    nc.tensor.matmul(ps_s5[:], ones32h[:], r2st[:], start=True, stop=True)
    s5 = work.tile([1, 512], F32, tag="s5sb")
    nc.scalar.copy(s5[:], ps_s5[:])
    nc.gpsimd.dma_start(d["ag5_i"][:], s5[:])
    nc.gpsimd.collective_compute(
        "AllGather", mybir.AluOpType.bypass, replica_groups=RG,
        ins=[d["ag5_i"][:]], outs=[d["ag5_o"][:]])
    agg5 = work.tile([8, 512], F32, tag="agg5")
    nc.sync.dma_start(agg5[:], d["ag5_o"][:])
    agg5h = work.tile([8, 512], BF16, tag="agg5h")
    nc.vector.tensor_copy(agg5h[:], agg5[:])
    ones8h = sing.tile([8, 1], BF16)
    nc.vector.memset(ones8h[:], 1.0)
    ps_g5 = ps2.tile([1, 512], F32, tag="g5", bufs=1)
    nc.tensor.matmul(ps_g5[:], ones8h[:], agg5h[:], start=True, stop=True)
    g5 = work.tile([1, 512], F32, tag="g5sb")
    nc.scalar.copy(g5[:], ps_g5[:])
    # scale/bias rows [1, 256] -> packed scb5 [1, 512] fp16 for PE broadcast
    eps1 = work.tile([1, 1], F32, tag="eps1")
    nc.vector.memset(eps1[:], EPS_BN)
    nc.scalar.mul(g5[:, 0:256], g5[:, 0:256], 1.0 / B)
    nc.scalar.mul(g5[:, 256:512], g5[:, 256:512], 1.0 / B)
    m2r = work.tile([1, 256], F32, tag="m2r")
    nc.vector.tensor_mul(m2r[:], g5[:, 0:256], g5[:, 0:256])
    nc.vector.tensor_sub(g5[:, 256:512], g5[:, 256:512], m2r[:])
    nc.scalar.activation(g5[:, 256:512], g5[:, 256:512], AF.Sqrt, bias=eps1[:])
    nc.vector.reciprocal(g5[:, 256:512], g5[:, 256:512])
    fg2r = load("fg2r", [1, 256], pool=work)
    fbe2r = load("fbe2r", [1, 256], pool=work)
    scb5 = work.tile([1, 512], BF16, tag="scb5")
    sc2f = work.tile([1, 256], F32, tag="sc2f")
    nc.vector.tensor_mul(sc2f[:], fg2r[:], g5[:, 256:512])
    nc.scalar.copy(scb5[:, 0:256], sc2f[:])
    bi2f = work.tile([1, 256], F32, tag="bi2f")
    nc.vector.tensor_mul(bi2f[:], sc2f[:], g5[:, 0:256])
    nc.vector.tensor_sub(bi2f[:], fbe2r[:], bi2f[:])
    nc.scalar.copy(scb5[:, 256:512], bi2f[:])
    ps_bc = ps2.tile([SEGC, 512], F32, tag="bc", bufs=1)
    nc.tensor.matmul(ps_bc[:], ones1x32h[:], scb5[:], start=True, stop=True)
    # apply BN2 + relu (per-column scale/bias via broadcast tiles)
    r2n = big.tile([SEGC, 256], BF16, tag="r2n")
    nc.vector.tensor_mul(r2n[:], r2st[:, 0:256], ps_bc[:, 0:256])
    nc.vector.tensor_add(r2n[:], r2n[:], ps_bc[:, 256:512])
    nc.vector.tensor_scalar_max(r2n[:], r2n[:], 0.0)
    # L2 normalize rows, write this core's [32, 256] slab
    nsq = work.tile([SEGC, 256], F32, tag="nsq")
    nc.scalar.activation(nsq[:], r2n[:], AF.Square)
    nrm = work.tile([SEGC, 1], F32, tag="nrm")
    nc.vector.reduce_sum(nrm[:], nsq[:], axis=mybir.AxisListType.X)
    nc.scalar.activation(nrm[:], nrm[:], AF.Sqrt)
    nc.vector.tensor_scalar_max(nrm[:], nrm[:], 1e-12)
    nc.vector.reciprocal(nrm[:], nrm[:])
    outf = big.tile([SEGC, 256], F32, tag="outf")
    nc.scalar.activation(outf[:], r2n[:], AF.Copy, scale=nrm[:])
    nc.sync.dma_start(d["out_final"][:], outf[:])

    ps2_cm.__exit__(None, None, None)
    work_cm.__exit__(None, None, None)
    big_cm.__exit__(None, None, None)
    sing_cm.__exit__(None, None, None)


# ------------------------------------------------------------------ host side
def _prep_core(x3, fw1, c):
    import ml_dtypes
    xs = x3[:, PL * c : PL * (c + 1), :]                       # [256,250,32]
    arr = np.ascontiguousarray(xs.transpose(2, 0, 1))          # [32,256,250]
    xA4 = arr.reshape(C, 4, QF).transpose(1, 0, 2).reshape(128, QF)
    xb = xs.reshape(B, 2, PH, C).transpose(2, 3, 1, 0)         # [125,32,2,256]
    xB = np.ascontiguousarray(xb).reshape(PH, C * 2 * B)
    fw = fw1.reshape(1024, P, C)[:, PL * c : PL * (c + 1), :]
    fw = fw.reshape(1024, 2, PH, C).transpose(2, 3, 1, 0)      # [125,32,2,1024]
    fw1t = np.ascontiguousarray(fw).reshape(PH, C * 2, 1024)
    bf = np.float16
    f8 = ml_dtypes.float8_e3m4
    return (np.ascontiguousarray(xA4).astype(bf), xB.astype(bf),
            (fw1t * 64.0).astype(f8))


def _qrep(v, rows):
    out = np.zeros((128, 1), np.float32)
    for a in range(4):
        out[32 * a : 32 * a + rows, 0] = v
    return out


def _wdiag(w):
    """w [out,in] -> block-diagonal lhsT [128, 128]: block a (32x32) holds
    w.T in its top-left corner."""
    t = np.zeros((128, 128), np.float32)
    wt = w.T  # [in, out]
    for a in range(4):
        t[32 * a : 32 * a + wt.shape[0], 32 * a : 32 * a + wt.shape[1]] = wt
    return t


def kernel(**inputs):
    import ml_dtypes

    if "nc" not in _cache:
        _cache["nc"] = _build()
    nc = _cache["nc"]
    bf = np.float16

    g = {k: np.asarray(v, np.float32) for k, v in inputs.items()
         if k != "length"}
    x3 = g["x"].reshape(B, P, C)

    f1 = np.zeros((128, 16), np.float32)
    f2 = np.zeros((128, 8), np.float32)
    for a in range(4):
        f1[32 * a : 32 * a + 16, :] = np.eye(16, dtype=np.float32)
        f2[32 * a : 32 * a + 8, :] = np.eye(8, dtype=np.float32)
    f8_16 = np.zeros((128, 16), np.float32)
    f8_8 = np.zeros((64, 8), np.float32)
    for k in range(8):
        f8_16[16 * k : 16 * k + 16, :] = np.eye(16, dtype=np.float32)
        f8_8[8 * k : 8 * k + 8, :] = np.eye(8, dtype=np.float32)

    shared = {
        "w1D": _wdiag(g["w1"]).astype(bf),
        "w2D": _wdiag(g["w2"]).astype(bf),
        "w3D": _wdiag(g["w3"]).astype(bf),
        "g1q": _qrep(g["g1"], 16), "be1q": _qrep(g["be1"], 16),
        "g2q": _qrep(g["g2"], 8), "be2q": _qrep(g["be2"], 8),
        "g3s": g["g3"].reshape(1, 1), "be3s": g["be3"].reshape(1, 1),
        "f1": f1, "ft1": np.ascontiguousarray(f1.T),
        "f2": f2, "ft2": np.ascontiguousarray(f2.T),
        "f8_16": f8_16, "f8_8": f8_8,
        "fw2s": np.ascontiguousarray(
            g["fw2"].reshape(256, 8, 128).transpose(2, 1, 0).reshape(
                128, 8 * 256)).astype(bf),
        "fg1t": np.ascontiguousarray(g["fg1"].reshape(8, 128).T),
        "fbe1t": np.ascontiguousarray(g["fbe1"].reshape(8, 128).T),
        "fg2r": g["fg2"].reshape(1, 256),
        "fbe2r": g["fbe2"].reshape(1, 256),
    }

    in_maps = []
    for c in range(NCORES):
        xA4, xB, fw1t = _prep_core(x3, g["fw1"], c)
        m = dict(shared)
        m["xA4"] = xA4
        m["xB"] = xB
        m["fw1t"] = fw1t
        in_maps.append(m)

    from concourse.bass_utils import run_bass_kernel_spmd

    res = run_bass_kernel_spmd(nc, in_maps, core_ids=list(range(NCORES)),
                               trace=bool(_cache.get("trace")))
    _cache["last_result"] = res
    return np.concatenate(
        [np.asarray(res.results[c]["out_final"], np.float32)
         for c in range(NCORES)], axis=0)


if __name__ == "__main__":
    nc = _build()
    print("build ok; instructions:",
          sum(len(bb.instructions) for bb in nc.main_func.blocks))


# revision 25
# speedup vs baseline: 1.0947x; 1.0947x over previous
"""Trainium2 Bass kernel for nn_FCGF_point_att3_sft_7000 (8 NeuronCores).

Model: pointwise attention MLP (32->16->8->1, BN+relu, BN stats over the full
512000-point batch), per-segment softmax over 2000 points, attention-weighted
pooling to [256, 64000], FC head 64000->1024->256 (BN+relu, stats over the
256-segment batch), final L2 row-normalize.

Sharding: points-within-segment. Core c owns points p in [250c, 250(c+1)) of
every segment. Stage A is data-parallel over points with AllGather'd BN stats;
fc1 is contraction-sharded (each core owns 8000 of the 64000 inputs and the
matching fw1 rows) with the output in [segs, feats] orientation so a
ReduceScatter over segments hands each core 32 complete segments; the softmax
denominators ride the same collective as an extra column. The whole tail
(BN1, fc2, BN2, L2-normalize) then runs locally per core on its 32 segments,
with two tiny AllGathers for the cross-segment BN statistics; each core emits
its own [32, 256] slab of the output.

Stage-A layout: "quartered" A-orientation. x.T is [128, 16000] with the
channels of free-quarter a on partitions [32a, 32a+32). Matmuls use
tile_position=(32a, 32a) so outputs land on partitions 32a+ch and every
eviction / BN / softmax op runs 128 partitions wide. Weight tiles are
zero-padded to M=32 so all PSUM rows are defined.

Training-mode BN is shift-invariant => conv/linear biases (b1,b2,b3,fb1,fb2)
drop out exactly; they are accepted and ignored.
"""

import sys

sys.path.insert(0, "/opt/trn_rl_repo")

import numpy as np

import concourse.bass as bass
import concourse.tile as tile
from concourse import mybir
from concourse.masks import make_identity

B = 256
P = 2000
C = 32
NCORES = 8
PL = P // NCORES           # 250
PH = PL // 2               # 125
NPTS = B * PL              # 64000 points per core
QF = NPTS // 4             # 16000 per quarter
NCH = 500                  # stage-A free chunk
NCHUNK = QF // NCH         # 32
SEGC = B // NCORES         # 32 segments per core after the ReduceScatter
RSW = 1028                 # rs payload width: 1024 feats + z + 3 pad
EPS_BN = 1e-5
F32 = mybir.dt.float32
BF16 = mybir.dt.float16  # fp16: same speed as bf16, 8x lower rounding noise
F8 = mybir.dt.float8e3   # e3m4: fc1 weight stream at half the HBM bytes
FW_SCALE = 64.0          # fw1*64 fits e3m4 range; undone in the z-normalize
RG = [list(range(NCORES))]
AF = mybir.ActivationFunctionType

_cache = {}


# ------------------------------------------------------------------ walrus fix
def _install_walrus_patch():
    """This container's walrus accepts only ONE semaphore wait per instruction.
    Spread Tile's end-of-kernel drain waits across single-wait nops, and split
    any instruction carrying >1 waits onto same-engine carrier nops."""
    if _cache.get("patched"):
        return
    from concourse.vector_clock import ScopedClock, VectorClock

    counter = [0]

    def split_waits(nc):
        for bb in nc.main_func.blocks:
            out = []
            changed = False
            for ins in bb.instructions:
                si = ins.sync_info
                waits = list(si.on_wait) if si and si.on_wait else []
                if len(waits) > 1:
                    changed = True
                    for w in waits[:-1]:
                        counter[0] += 1
                        out.append(mybir.InstNoOp(
                            name=f"I-wsplit-{counter[0]}",
                            engine=ins.engine, ins=[], outs=[],
                            sync_info=mybir.SyncInfo(on_wait=[w], on_update=[]),
                            bass_nofuse=True))
                    si.on_wait = waits[-1:]
                out.append(ins)
            if changed:
                try:
                    bb.instructions = out
                except Exception:
                    bb.instructions.clear()
                    for x in out:
                        bb.instructions.append(x)

    def _patched(self, tick_clock, wait_clock):
        nc = self.nc
        gc = tick_clock.global_clock
        n = len(gc)
        for i in range(n):
            if gc[i] > 0:
                vec = [0] * n
                vec[i] = gc[i]
                nop = nc.sync.nop(nofuse=True, hint=f"drain_wait_p{i}")
                wait_clock.add_sem_waits(
                    nop.ins, ScopedClock({None: VectorClock(vec)}))
        nc.sync.drain()
        nc.all_engine_barrier()
        assert self.sems is not None
        popped = nc._tile_sem_poison_stack.pop()
        assert popped is self._sem_poison
        nc.clear_and_free_semaphores(list(self.sems.allocated().values()))
        nc.all_engine_barrier()
        split_waits(nc)

    tile.TileContext._drain_and_barrier = _patched
    _cache["patched"] = True


# ------------------------------------------------------------------ bass build
def _build():
    _install_walrus_patch()
    nc = bass.Bass()

    def ein(name, shape, dt):
        return nc.dram_tensor(name, shape, dt, kind="ExternalInput")

    d = {}
    d["xA4"] = ein("xA4", [128, QF], BF16)
    d["xB"] = ein("xB", [PH, C * 2 * B], BF16)
    d["w1D"] = ein("w1D", [128, 128], BF16)
    d["w2D"] = ein("w2D", [128, 128], BF16)
    d["w3D"] = ein("w3D", [128, 128], BF16)
    for n in ("g1q", "be1q", "g2q", "be2q"):
        d[n] = ein(n, [128, 1], F32)
    d["g3s"] = ein("g3s", [1, 1], F32)
    d["be3s"] = ein("be3s", [1, 1], F32)
    d["f1"] = ein("f1", [128, 16], F32)
    d["ft1"] = ein("ft1", [16, 128], F32)
    d["f2"] = ein("f2", [128, 8], F32)
    d["ft2"] = ein("ft2", [8, 128], F32)
    d["f8_16"] = ein("f8_16", [128, 16], F32)
    d["f8_8"] = ein("f8_8", [64, 8], F32)
    d["fw1t"] = ein("fw1t", [PH, C * 2, 1024], F8)
    d["fw2s"] = ein("fw2s", [128, 8 * 256], BF16)
    d["fg1t"] = ein("fg1t", [128, 8], F32)
    d["fbe1t"] = ein("fbe1t", [128, 8], F32)
    d["fg2r"] = ein("fg2r", [1, 256], F32)
    d["fbe2r"] = ein("fbe2r", [1, 256], F32)
    d["out_final"] = nc.dram_tensor("out_final", [SEGC, 256], F32,
                                    kind="ExternalOutput")
    # collective bounce buffers
    d["warm_i"] = nc.dram_tensor("warm_i", [16, 4], F32)
    d["warm_o"] = nc.dram_tensor("warm_o", [16, 4], F32)
    d["st1_i"] = nc.dram_tensor("st1_i", [16, 2], F32)
    d["st1_o"] = nc.dram_tensor("st1_o", [128, 2], F32)
    d["st2_i"] = nc.dram_tensor("st2_i", [8, 2], F32)
    d["st2_o"] = nc.dram_tensor("st2_o", [64, 2], F32)
    d["st3_i"] = nc.dram_tensor("st3_i", [1, 2], F32)
    d["st3_o"] = nc.dram_tensor("st3_o", [8, 2], F32)
    d["rs_iA"] = nc.dram_tensor("rs_iA", [B, RSW], BF16)
    d["rs_oA"] = nc.dram_tensor("rs_oA", [SEGC, RSW], BF16)
    d["rs_iB"] = nc.dram_tensor("rs_iB", [B, RSW], BF16)
    d["rs_oB"] = nc.dram_tensor("rs_oB", [SEGC, RSW], BF16)
    d["ag4_i"] = nc.dram_tensor("ag4_i", [128, 16], F32)
    d["ag4_o"] = nc.dram_tensor("ag4_o", [128 * NCORES, 16], F32)
    d["ag5_i"] = nc.dram_tensor("ag5_i", [1, 512], F32)
    d["ag5_o"] = nc.dram_tensor("ag5_o", [NCORES, 512], F32)

    with tile.TileContext(nc) as tc:
        _body(nc, tc, d)
    return nc


def _mkstats(nc, pool, mv, count, name):
    """mv [p,2]=(mean,var) -> (sum,sumsq) [p,2]."""
    p = mv.shape[0]
    ss = pool.tile([p, 2], F32, tag=f"ss_{name}")
    nc.vector.tensor_mul(ss[:, 1:2], mv[:, 0:1], mv[:, 0:1])
    nc.vector.tensor_add(ss[:, 1:2], ss[:, 1:2], mv[:, 1:2])
    nc.scalar.mul(ss[:, 0:1], mv[:, 0:1], float(count))
    nc.scalar.mul(ss[:, 1:2], ss[:, 1:2], float(count))
    return ss


def _mv_from_ss(nc, pool, ss, count, name):
    """(sum,sumsq) [p,2] over count -> (mean, rstd) [p,2]."""
    p = ss.shape[0]
    mr = pool.tile([p, 2], F32, tag=f"mr_{name}")
    epst = pool.tile([p, 1], F32, tag=f"eps_{name}")
    nc.vector.memset(epst[:], EPS_BN)
    nc.scalar.mul(mr[:, 0:1], ss[:, 0:1], 1.0 / count)
    nc.scalar.mul(mr[:, 1:2], ss[:, 1:2], 1.0 / count)
    m2 = pool.tile([p, 1], F32, tag=f"m2_{name}")
    nc.vector.tensor_mul(m2[:], mr[:, 0:1], mr[:, 0:1])
    nc.vector.tensor_sub(mr[:, 1:2], mr[:, 1:2], m2[:])
    nc.scalar.activation(mr[:, 1:2], mr[:, 1:2], AF.Sqrt, bias=epst[:])
    nc.vector.reciprocal(mr[:, 1:2], mr[:, 1:2])
    return mr


def _scale_bias(nc, pool, mrq, g, be, name):
    """scale = g*rstd ; bias = be - scale*mean  (all [p,1] per-partition)."""
    p = mrq.shape[0]
    sc = pool.tile([p, 1], F32, tag=f"sc_{name}")
    bi = pool.tile([p, 1], F32, tag=f"bi_{name}")
    nc.vector.tensor_mul(sc[:], g[:], mrq[:, 1:2])
    nc.vector.tensor_mul(bi[:], sc[:], mrq[:, 0:1])
    nc.vector.tensor_sub(bi[:], be[:], bi[:])
    return sc, bi


def _body(nc, tc, d):
    # collective warmup first. warm_i is never written (contents irrelevant),
    # so the op has NO dependencies and the ~55us ncfw startup begins at t=0,
    # overlapping the whole front of the kernel.
    nc.gpsimd.collective_compute(
        "AllReduce", mybir.AluOpType.add, replica_groups=RG,
        ins=[d["warm_i"][:]], outs=[d["warm_o"][:]])
    sing_cm = tc.tile_pool(name="sing", bufs=1)
    big_cm = tc.tile_pool(name="big", bufs=1)
    work_cm = tc.tile_pool(name="work", bufs=1)
    psA_cm = tc.tile_pool(name="psA", bufs=4, space="PSUM")
    psT_cm = tc.tile_pool(name="psT", bufs=2, space="PSUM")
    psS_cm = tc.tile_pool(name="psS", bufs=2, space="PSUM")
    sing = sing_cm.__enter__(); big = big_cm.__enter__()
    work = work_cm.__enter__()
    fw1p_cm = tc.tile_pool(name="fw1p", bufs=8)
    fw1p = fw1p_cm.__enter__()
    psA = psA_cm.__enter__(); psT = psT_cm.__enter__()
    psS = psS_cm.__enter__()

    # ---------------- constants
    def load(name, shape, dt=F32, pool=sing):
        t = pool.tile(shape, dt, tag=name)
        nc.sync.dma_start(t[:], d[name][:])
        return t

    w1D = load("w1D", [128, 128], BF16)
    w2D = load("w2D", [128, 128], BF16)
    w3D = load("w3D", [128, 128], BF16)
    f1s = load("f1", [128, 16])
    ft1s = load("ft1", [16, 128])
    f2s = load("f2", [128, 8])
    ft2s = load("ft2", [8, 128])
    f8_16s = load("f8_16", [128, 16])
    f8_8s = load("f8_8", [64, 8])
    g1 = load("g1q", [128, 1]); be1 = load("be1q", [128, 1])
    g2 = load("g2q", [128, 1]); be2 = load("be2q", [128, 1])
    g3 = load("g3s", [1, 1]); be3 = load("be3s", [1, 1])
    ones128 = sing.tile([128, 1], F32)
    nc.vector.memset(ones128[:], 1.0)
    ones8 = sing.tile([8, 1], F32)
    nc.vector.memset(ones8[:], 1.0)
    ones1x = sing.tile([1, 128], F32)
    nc.vector.memset(ones1x[:], 1.0)
    ones32h = sing.tile([32, 1], BF16)
    nc.vector.memset(ones32h[:], 1.0)
    ones1x32h = sing.tile([1, 32], BF16)
    nc.vector.memset(ones1x32h[:], 1.0)
    ident = sing.tile([128, 128], F32)
    make_identity(nc, ident[:])
    identh = sing.tile([SEGC, SEGC], BF16)
    make_identity(nc, identh[:])

    # ---------------- big loads
    xa = big.tile([128, QF], BF16, tag="slotA")       # slot A: xa -> y2 -> y3q
    nc.sync.dma_start(xa[:], d["xA4"][:])
    xb = big.tile([PH, C * 2 * B], BF16, tag="xb")
    nc.sync.dma_start(xb[:], d["xB"][:])
    xbv = xb[:].rearrange("p (c h s) -> p c h s", c=C, h=2, s=B)

    # fc1 weight prefetch: pool entered at the top so its slots exist from
    # t=0 and the 16.4MB stream overlaps all of stage A. 2-engine rotation.
    FW_CHUNKS = [8] * 8
    fwtiles = []
    _dge = [nc.sync, nc.scalar]
    _off = 0
    for gblk, nits in enumerate(FW_CHUNKS):
        fwt = fw1p.tile([PH, 8, 1024], F8, tag="fw", name=f"fw_{gblk}")
        _dge[gblk % 2].dma_start(fwt[:, :nits, :],
                                 d["fw1t"][:, _off : _off + nits, :])
        fwtiles.append((fwt, _off, nits))
        _off += nits
    fw2s = sing.tile([128, 8 * 256], BF16, tag="fw2s")
    nc.sync.dma_start(fw2s[:], d["fw2s"][:])
    fw2sv = fw2s[:].rearrange("p (t o) -> p t o", t=8)

    def layer_mms(ps, wD, krows, rhs_src, sl):
        nc.tensor.matmul(ps[:], wD[:], rhs_src[:, sl], start=True, stop=True)

    def stage_layer(rhs_src, wT, krows, fold, foldT, f8fold, st_i, st_o,
                    gq, beq, count_local, name, out_tag, wnext=None):
        """Single-pass layer: matmuls -> evict y fp16 (+bn_stats from y),
        fold+AllGather stats. The BN scale is folded into the next layer's
        weights (gamma>0), so the relu pass is a per-partition bias-shift
        split across scalar/vector/gpsimd."""
        y = big.tile([128, QF], BF16, tag=out_tag, name=f"y_{name}")
        stat = work.tile([128, NCHUNK, 6], F32, tag=f"stat_{name}")
        for j in range(NCHUNK):
            ps = psA.tile([128, NCH], F32, tag="psA", name=f"ps_{name}_{j}")
            layer_mms(ps, wT, krows, rhs_src, slice(j * NCH, (j + 1) * NCH))
            nc.scalar.copy(y[:, j * NCH : (j + 1) * NCH], ps[:])
            nc.vector.bn_stats(stat[:, j, :], y[:, j * NCH : (j + 1) * NCH])
        mv = work.tile([128, 2], F32, tag=f"mv_{name}")
        nc.vector.bn_aggr(mv[:], stat[:])
        ss = _mkstats(nc, work, mv, count_local, name)
        nfold = fold.shape[1]
        psf = psS.tile([128, 2], F32, tag="small", name=f"psf_{name}")
        nc.tensor.matmul(psf[:nfold, :], fold[:], ss[:], start=True, stop=True)
        sbf = work.tile([nfold, 2], F32, tag=f"sbf_{name}")
        nc.scalar.copy(sbf[:], psf[:nfold, :])
        nc.gpsimd.dma_start(st_i[:], sbf[:])
        nc.gpsimd.collective_compute(
            "AllGather", mybir.AluOpType.bypass, replica_groups=RG,
            ins=[st_i[:]], outs=[st_o[:]])
        agg = work.tile([nfold * NCORES, 2], F32, tag=f"agg_{name}")
        nc.gpsimd.dma_start(agg[:], st_o[:])
        psg = psS.tile([128, 2], F32, tag="small", name=f"psg_{name}")
        nc.tensor.matmul(psg[:nfold, :], f8fold[:], agg[:], start=True,
                         stop=True)
        ssg = work.tile([nfold, 2], F32, tag=f"ssg_{name}")
        nc.scalar.copy(ssg[:], psg[:nfold, :])
        mr = _mv_from_ss(nc, work, ssg, B * P, name)
        psb = psS.tile([128, 2], F32, tag="small", name=f"psb_{name}")
        nc.tensor.matmul(psb[:], foldT[:], mr[:], start=True, stop=True)
        mrq = work.tile([128, 2], F32, tag=f"mrq_{name}")
        nc.scalar.copy(mrq[:], psb[:])
        sc, bi = _scale_bias(nc, work, mrq, gq, beq, name)
        # fold the BN scale into the next layer's weights (gamma>0); the relu
        # pass becomes a bias-shift, split across scalar/vector/gpsimd. Clamp
        # sc away from 0 first: padded partition slots have gamma=0 and the
        # bare reciprocal would make bip = 0*inf = NaN there.
        isc = work.tile([128, 1], F32, tag=f"isc_{name}")
        nc.vector.tensor_scalar_max(isc[:], sc[:], 1e-30)
        nc.vector.reciprocal(isc[:], isc[:])
        bip = work.tile([128, 1], F32, tag=f"bip_{name}")
        nc.vector.tensor_mul(bip[:], bi[:], isc[:])
        wnf = None
        if wnext is not None:
            wnf = sing.tile([128, 128], BF16, tag=f"wnf_{name}")
            nc.vector.tensor_scalar_mul(wnf[:], wnext[:], sc[:])
        for j in range(NCHUNK):
            sl = slice(j * NCH, (j + 1) * NCH)
            if j % 3 == 2:
                nc.scalar.activation(y[:, sl], y[:, sl], AF.Relu, bias=bip[:])
            else:
                nc.vector.tensor_scalar(y[:, sl], y[:, sl], bip[:], 0.0,
                                        mybir.AluOpType.add,
                                        mybir.AluOpType.max)
        return y, wnf

    # ---------------- stage A layers 1 & 2
    h1, w2f = stage_layer(xa, w1D, 32, f1s, ft1s, f8_16s,
                          d["st1_i"], d["st1_o"], g1, be1, QF, "l1", "slotB",
                          wnext=w2D)
    # h2 reuses slot A (xa dead after L1 matmuls)
    h2, w3f = stage_layer(h1, w2f, 16, f2s, ft2s, f8_8s,
                          d["st2_i"], d["st2_o"], g2, be2, QF, "l2", "slotA",
                          wnext=w3D)

    # ---------------- stage A layer 3: scores straight from PSUM into
    # scoreS [128 segs, 2, 250] via per-chunk repack DMAs (rows {32a} real;
    # chunk j of quarter a covers segments 64a+2j..+1)
    y3q = big.tile([128, QF], BF16, tag="slotB", name="y3q")
    for j in range(NCHUNK):
        ps = psA.tile([128, NCH], F32, tag="psA", name=f"ps_l3_{j}")
        layer_mms(ps, w3f, 8, h2, slice(j * NCH, (j + 1) * NCH))
        nc.scalar.copy(y3q[:, j * NCH : (j + 1) * NCH], ps[:])
    scoreS = big.tile([128, 2, PL], BF16, tag="scoreS")
    _sse = [nc.sync, nc.scalar, nc.gpsimd, nc.sync]
    for a in range(4):
        _sse[a].dma_start(
            scoreS[64 * (a % 2) : 64 * (a % 2) + 64, a // 2, :],
            y3q[32 * a : 32 * a + 1, :])
    # BN3 stats over all segments/points (all partitions real)
    stat3 = work.tile([128, 2, 6], F32, tag="stat3")
    nc.vector.bn_stats(stat3[:, 0, :], scoreS[:, 0, :])
    nc.vector.bn_stats(stat3[:, 1, :], scoreS[:, 1, :])
    mv3 = work.tile([128, 2], F32, tag="mv3")
    nc.vector.bn_aggr(mv3[:], stat3[:])
    ss3 = _mkstats(nc, work, mv3, 2 * PL, "l3")
    psf3 = psS.tile([128, 2], F32, tag="small", name="psf3")
    nc.tensor.matmul(psf3[:1, :], ones128[:], ss3[:], start=True, stop=True)
    sbf3 = work.tile([1, 2], F32, tag="sbf3")
    nc.scalar.copy(sbf3[:], psf3[:1, :])
    nc.gpsimd.dma_start(d["st3_i"][:], sbf3[:])
    nc.gpsimd.collective_compute(
        "AllGather", mybir.AluOpType.bypass, replica_groups=RG,
        ins=[d["st3_i"][:]], outs=[d["st3_o"][:]])
    agg3 = work.tile([8, 2], F32, tag="agg3")
    nc.gpsimd.dma_start(agg3[:], d["st3_o"][:])
    psg3 = psS.tile([128, 2], F32, tag="small", name="psg3")
    nc.tensor.matmul(psg3[:1, :], ones8[:], agg3[:], start=True, stop=True)
    ssg3 = work.tile([1, 2], F32, tag="ssg3")
    nc.scalar.copy(ssg3[:], psg3[:1, :])
    mr3 = _mv_from_ss(nc, work, ssg3, B * P, "l3")
    scb1 = work.tile([1, 2], F32, tag="scb1")
    nc.vector.tensor_mul(scb1[:, 0:1], g3[:], mr3[:, 1:2])
    nc.vector.tensor_mul(scb1[:, 1:2], scb1[:, 0:1], mr3[:, 0:1])
    nc.vector.tensor_sub(scb1[:, 1:2], be3[:], scb1[:, 1:2])
    psb3 = psS.tile([128, 2], F32, tag="small", name="psb3")
    nc.tensor.matmul(psb3[:], ones1x[:], scb1[:], start=True, stop=True)
    scb = work.tile([128, 2], F32, tag="scb")
    nc.scalar.copy(scb[:], psb3[:])
    # relu(BN3) in place, then exp
    expS = big.tile([128, 2, PL], F32, tag="expS")
    expT = big.tile([PH, 2, 256], BF16, tag="expT")
    for tt in range(2):
        nc.scalar.activation(scoreS[:, tt, :], scoreS[:, tt, :], AF.Relu,
                             bias=scb[:, 1:2], scale=scb[:, 0:1])
        nc.scalar.activation(expS[:, tt, :], scoreS[:, tt, :], AF.Exp)
        for h in range(2):
            pt_ps = psT.tile([128, 128], F32, tag="psT")
            nc.tensor.transpose(pt_ps[:PH, :],
                                expS[:, tt, h * PH : h * PH + PH], ident[:])
            nc.vector.tensor_copy(expT[:, h, tt * 128 : tt * 128 + 128],
                                  pt_ps[:PH, :])
        # partial softmax denominators
    zloc = work.tile([128, 8], F32, tag="zloc")
    nc.vector.memset(zloc[:], 0.0)
    nc.vector.reduce_sum(zloc[:, 0:1], expS[:, 0, :], axis=mybir.AxisListType.X)
    nc.vector.reduce_sum(zloc[:, 4:5], expS[:, 1, :], axis=mybir.AxisListType.X)
    zpad = work.tile([128, 8], BF16, tag="zpad")
    nc.scalar.copy(zpad[:], zloc[:])
    # z (and zero pad) into columns 1024:1028 of rs_i: row (t p) = seg t*128+p
    nc.sync.dma_start(
        d["rs_iA"][:, 1024:1028].rearrange("(t p) c -> p t c", t=2),
        zpad[:].rearrange("p (t c) -> p t c", t=2))
    zero8 = work.tile([128, 8], BF16, tag="zero8")
    nc.vector.memset(zero8[:], 0.0)
    nc.sync.dma_start(
        d["rs_iB"][:, 1024:1028].rearrange("(t p) c -> p t c", t=2),
        zero8[:].rearrange("p (t c) -> p t c", t=2))

    psS_cm.__exit__(None, None, None)
    psT_cm.__exit__(None, None, None)
    psA_cm.__exit__(None, None, None)

    # ---------------- FC1 (contraction-sharded, out [256 segs, 1024] partial)
    # lhsT = pt seg-halves [125, 128]; rhs = fw1 it-chunk [125, 512]-halves.
    psF_cm = tc.tile_pool(name="psF", bufs=1, space="PSUM")
    ptp_cm = tc.tile_pool(name="ptp", bufs=4)
    psF = psF_cm.__enter__()
    ptp = ptp_cm.__enter__()
    r1ps = [psF.tile([128, 512], F32, name=f"r1ps_{m}", tag=f"r1_{m}")
            for m in range(4)]
    NIT = C * 2
    HIT = NIT // 2

    def fc1_evict(gen, rs_i):
        _dmaeng = [nc.sync, nc.scalar, nc.gpsimd, nc.sync]
        for m in range(4):
            s, f = m // 2, m % 2
            r1sb = big.tile([128, 512], BF16, tag=f"r1sb{gen}",
                            name=f"r1sb{gen}_{m}", bufs=2)
            if m % 2 == 0:
                nc.scalar.copy(r1sb[:], r1ps[m][:])
            else:
                nc.vector.tensor_copy(r1sb[:], r1ps[m][:])
            _dmaeng[m].dma_start(
                rs_i[s * 128 : (s + 1) * 128, f * 512 : (f + 1) * 512],
                r1sb[:])

    for ch in range(C):
        for h in range(2):
            it = ch * 2 + h
            gi = 0
            while not (fwtiles[gi][1] <= it < fwtiles[gi][1] + fwtiles[gi][2]):
                gi += 1
            fw = fwtiles[gi][0][:, it - fwtiles[gi][1], :]
            pt = ptp.tile([PH, 256], BF16, tag="pt", name=f"pt_{it}")
            nc.vector.tensor_mul(pt[:], xbv[:, ch, h, :], expT[:, h, :])
            for s in range(2):
                lhsT = pt[:, s * 128 : (s + 1) * 128]
                for f in range(2):
                    nc.tensor.matmul(
                        r1ps[s * 2 + f][:, :], lhsT,
                        fw[:, f * 512 : (f + 1) * 512],
                        start=(it in (0, HIT)), stop=(it in (HIT - 1, NIT - 1)))
            if it == HIT - 1:
                # first-half partials ship out mid-FC1 so RS#1 overlaps the
                # second half of the contraction.
                fc1_evict("A", d["rs_iA"])
                nc.gpsimd.collective_compute(
                    "ReduceScatter", mybir.AluOpType.add, replica_groups=RG,
                    ins=[d["rs_iA"][:]], outs=[d["rs_oA"][:]])
    fc1_evict("B", d["rs_iB"])
    nc.gpsimd.collective_compute(
        "ReduceScatter", mybir.AluOpType.add, replica_groups=RG,
        ins=[d["rs_iB"][:]], outs=[d["rs_oB"][:]])

    ptp_cm.__exit__(None, None, None)
    psF_cm.__exit__(None, None, None)
    fw1p_cm.__exit__(None, None, None)

    # ---------------- tail: this core owns segments [32c, 32c+32), complete.
    ps2_cm = tc.tile_pool(name="ps2", bufs=2, space="PSUM")
    ps2 = ps2_cm.__enter__()

    r1A = big.tile([SEGC, RSW], BF16, tag="r1A")
    nc.sync.dma_start(r1A[:], d["rs_oA"][:])
    r1B = big.tile([SEGC, RSW], BF16, tag="r1B")
    nc.sync.dma_start(r1B[:], d["rs_oB"][:])
    r1 = big.tile([SEGC, RSW], BF16, tag="r1")
    nc.vector.tensor_add(r1[:], r1A[:], r1B[:])
    zinv = work.tile([SEGC, 1], F32, tag="zinv")
    nc.scalar.mul(zinv[:], r1[:, 1024:1025], FW_SCALE)
    nc.vector.reciprocal(zinv[:], zinv[:])
    r1z = big.tile([SEGC, 1024], BF16, tag="r1z")
    nc.scalar.activation(r1z[:], r1[:, 0:1024], AF.Copy, scale=zinv[:])
    # transpose to 8 feature tiles [128, 32]; BN1 stats over local 32 segs
    r1T = big.tile([128, 8, SEGC], BF16, tag="r1T")
    stat8 = work.tile([128, 8, 6], F32, tag="stat8")
    for t in range(8):
        pt_ps = ps2.tile([128, SEGC], BF16, tag="tp", name=f"tp_{t}")
        nc.tensor.transpose(pt_ps[:], r1z[:, t * 128 : (t + 1) * 128],
                            identh[:])
        nc.scalar.copy(r1T[:, t, :], pt_ps[:])
        nc.vector.bn_stats(stat8[:, t, :], pt_ps[:])
    mv8 = work.tile([128, 8, 2], F32, tag="mv8")
    for t in range(8):
        nc.vector.bn_aggr(mv8[:, t, :], stat8[:, t : t + 1, :])
    ss8 = work.tile([128, 8, 2], F32, tag="ss8")
    nc.vector.tensor_mul(ss8[:, :, 1:2], mv8[:, :, 0:1], mv8[:, :, 0:1])
    nc.vector.tensor_add(ss8[:, :, 1:2], ss8[:, :, 1:2], mv8[:, :, 1:2])
    nc.scalar.mul(ss8[:, :, 0:1], mv8[:, :, 0:1], float(SEGC))
    nc.scalar.mul(ss8[:, :, 1:2], ss8[:, :, 1:2], float(SEGC))
    nc.gpsimd.dma_start(d["ag4_i"][:], ss8[:].rearrange("p t u -> p (t u)"))
    nc.gpsimd.collective_compute(
        "AllGather", mybir.AluOpType.bypass, replica_groups=RG,
        ins=[d["ag4_i"][:]], outs=[d["ag4_o"][:]])
    agg4 = work.tile([128, 8, 16], F32, tag="agg4")
    _age = [nc.sync, nc.scalar, nc.gpsimd]
    for c in range(8):
        _age[c % 3].dma_start(agg4[:, c, :],
                              d["ag4_o"][c * 128 : (c + 1) * 128, :])
    g8 = work.tile([128, 8, 2], F32, tag="g8")
    nc.vector.reduce_sum(g8[:], agg4[:].rearrange("p c w -> p w c"),
                         axis=mybir.AxisListType.X)
    # mean/rstd per feature ([128, 8] per-partition per-tile)
    epsf = work.tile([128, 1], F32, tag="epsf")
    nc.vector.memset(epsf[:], EPS_BN)
    nc.scalar.mul(g8[:, :, 0:1], g8[:, :, 0:1], 1.0 / B)
    nc.scalar.mul(g8[:, :, 1:2], g8[:, :, 1:2], 1.0 / B)
    m2t = work.tile([128, 8], F32, tag="m2t")
    nc.vector.tensor_mul(m2t[:], g8[:, :, 0:1], g8[:, :, 0:1])
    nc.vector.tensor_sub(g8[:, :, 1:2], g8[:, :, 1:2], m2t[:])
    nc.scalar.activation(g8[:, :, 1:2], g8[:, :, 1:2], AF.Sqrt, bias=epsf[:])
    nc.vector.reciprocal(g8[:, :, 1:2], g8[:, :, 1:2])
    fg1t = load("fg1t", [128, 8], pool=work)
    fbe1t = load("fbe1t", [128, 8], pool=work)
    sc1 = work.tile([128, 8], F32, tag="sc1")
    bi1 = work.tile([128, 8], F32, tag="bi1")
    nc.vector.tensor_mul(sc1[:], fg1t[:], g8[:, :, 1:2])
    nc.vector.tensor_mul(bi1[:], sc1[:], g8[:, :, 0:1])
    nc.vector.tensor_sub(bi1[:], fbe1t[:], bi1[:])
    r1Tr = big.tile([128, 8, SEGC], BF16, tag="r1Tr")
    for t in range(8):
        nc.scalar.activation(r1Tr[:, t, :], r1T[:, t, :], AF.Relu,
                             bias=bi1[:, t : t + 1], scale=sc1[:, t : t + 1])
    # FC2: out [32 segs, 256] complete (contraction over 1024 feats, local)
    ps_r2 = ps2.tile([SEGC, 256], F32, tag="r2", bufs=1)
    for t in range(8):
        nc.tensor.matmul(ps_r2[:], r1Tr[:, t, :], fw2sv[:, t, :],
                         start=(t == 0), stop=(t == 7))
    r2st = big.tile([SEGC, 512], BF16, tag="r2st")
    nc.scalar.copy(r2st[:, 0:256], ps_r2[:])
    nc.scalar.activation(r2st[:, 256:512], ps_r2[:], AF.Square)
    ps_s5 = ps2.tile([1, 512], F32, tag="s5", bufs=1)
    nc.tensor.matmul(ps_s5[:], ones32h[:], r2st[:], start=True, stop=True)
    s5 = work.tile([1, 512], F32, tag="s5sb")
    nc.scalar.copy(s5[:], ps_s5[:])
    nc.gpsimd.dma_start(d["ag5_i"][:], s5[:])
    nc.gpsimd.collective_compute(
        "AllGather", mybir.AluOpType.bypass, replica_groups=RG,
        ins=[d["ag5_i"][:]], outs=[d["ag5_o"][:]])
    agg5 = work.tile([8, 512], F32, tag="agg5")
    nc.sync.dma_start(agg5[:], d["ag5_o"][:])
    agg5h = work.tile([8, 512], BF16, tag="agg5h")
    nc.vector.tensor_copy(agg5h[:], agg5[:])
    ones8h = sing.tile([8, 1], BF16)
    nc.vector.memset(ones8h[:], 1.0)
    ps_g5 = ps2.tile([1, 512], F32, tag="g5", bufs=1)
    nc.tensor.matmul(ps_g5[:], ones8h[:], agg5h[:], start=True, stop=True)
    g5 = work.tile([1, 512], F32, tag="g5sb")
    nc.scalar.copy(g5[:], ps_g5[:])
    # scale/bias rows [1, 256] -> packed scb5 [1, 512] fp16 for PE broadcast
    eps1 = work.tile([1, 1], F32, tag="eps1")
    nc.vector.memset(eps1[:], EPS_BN)
    nc.scalar.mul(g5[:, 0:256], g5[:, 0:256], 1.0 / B)
    nc.scalar.mul(g5[:, 256:512], g5[:, 256:512], 1.0 / B)
    m2r = work.tile([1, 256], F32, tag="m2r")
    nc.vector.tensor_mul(m2r[:], g5[:, 0:256], g5[:, 0:256])
    nc.vector.tensor_sub(g5[:, 256:512], g5[:, 256:512], m2r[:])
    nc.scalar.activation(g5[:, 256:512], g5[:, 256:512], AF.Sqrt, bias=eps1[:])
    nc.vector.reciprocal(g5[:, 256:512], g5[:, 256:512])
    fg2r = load("fg2r", [1, 256], pool=work)
    fbe2r = load("fbe2r", [1, 256], pool=work)
    scb5 = work.tile([1, 512], BF16, tag="scb5")
    sc2f = work.tile([1, 256], F32, tag="sc2f")
    nc.vector.tensor_mul(sc2f[:], fg2r[:], g5[:, 256:512])
    nc.scalar.copy(scb5[:, 0:256], sc2f[:])
    bi2f = work.tile([1, 256], F32, tag="bi2f")
    nc.vector.tensor_mul(bi2f[:], sc2f[:], g5[:, 0:256])
    nc.vector.tensor_sub(bi2f[:], fbe2r[:], bi2f[:])
    nc.scalar.copy(scb5[:, 256:512], bi2f[:])
    ps_bc = ps2.tile([SEGC, 512], F32, tag="bc", bufs=1)
    nc.tensor.matmul(ps_bc[:], ones1x32h[:], scb5[:], start=True, stop=True)
    # apply BN2 + relu (per-column scale/bias via broadcast tiles)
    r2n = big.tile([SEGC, 256], BF16, tag="r2n")
    nc.vector.tensor_mul(r2n[:], r2st[:, 0:256], ps_bc[:, 0:256])
    nc.vector.tensor_add(r2n[:], r2n[:], ps_bc[:, 256:512])
    nc.vector.tensor_scalar_max(r2n[:], r2n[:], 0.0)
    # L2 normalize rows, write this core's [32, 256] slab
    nsq = work.tile([SEGC, 256], F32, tag="nsq")
    nc.scalar.activation(nsq[:], r2n[:], AF.Square)
    nrm = work.tile([SEGC, 1], F32, tag="nrm")
    nc.vector.reduce_sum(nrm[:], nsq[:], axis=mybir.AxisListType.X)
    nc.scalar.activation(nrm[:], nrm[:], AF.Sqrt)
    nc.vector.tensor_scalar_max(nrm[:], nrm[:], 1e-12)
    nc.vector.reciprocal(nrm[:], nrm[:])
    outf = big.tile([SEGC, 256], F32, tag="outf")
    nc.scalar.activation(outf[:], r2n[:], AF.Copy, scale=nrm[:])
    nc.sync.dma_start(d["out_final"][:], outf[:])

    ps2_cm.__exit__(None, None, None)
    work_cm.__exit__(None, None, None)
    big_cm.__exit__(None, None, None)
    sing_cm.__exit__(None, None, None)


# ------------------------------------------------------------------ host side
def _prep_core(x3, fw1, c):
    import ml_dtypes
    xs = x3[:, PL * c : PL * (c + 1), :]                       # [256,250,32]
    arr = np.ascontiguousarray(xs.transpose(2, 0, 1))          # [32,256,250]
    xA4 = arr.reshape(C, 4, QF).transpose(1, 0, 2).reshape(128, QF)
    xb = xs.reshape(B, 2, PH, C).transpose(2, 3, 1, 0)         # [125,32,2,256]
    xB = np.ascontiguousarray(xb).reshape(PH, C * 2 * B)
    fw = fw1.reshape(1024, P, C)[:, PL * c : PL * (c + 1), :]
    fw = fw.reshape(1024, 2, PH, C).transpose(2, 3, 1, 0)      # [125,32,2,1024]
    fw1t = np.ascontiguousarray(fw).reshape(PH, C * 2, 1024)
    bf = np.float16
    f8 = ml_dtypes.float8_e3m4
    return (np.ascontiguousarray(xA4).astype(bf), xB.astype(bf),
            (fw1t * 64.0).astype(f8))


def _qrep(v, rows):
    out = np.zeros((128, 1), np.float32)
    for a in range(4):
        out[32 * a : 32 * a + rows, 0] = v
    return out


def _wdiag(w):
    """w [out,in] -> block-diagonal lhsT [128, 128]: block a (32x32) holds
    w.T in its top-left corner."""
    t = np.zeros((128, 128), np.float32)
    wt = w.T  # [in, out]
    for a in range(4):
        t[32 * a : 32 * a + wt.shape[0], 32 * a : 32 * a + wt.shape[1]] = wt
    return t


def kernel(**inputs):
    import ml_dtypes

    if "nc" not in _cache:
        _cache["nc"] = _build()
    nc = _cache["nc"]
    bf = np.float16

    g = {k: np.asarray(v, np.float32) for k, v in inputs.items()
         if k != "length"}
    x3 = g["x"].reshape(B, P, C)

    f1 = np.zeros((128, 16), np.float32)
    f2 = np.zeros((128, 8), np.float32)
    for a in range(4):
        f1[32 * a : 32 * a + 16, :] = np.eye(16, dtype=np.float32)
        f2[32 * a : 32 * a + 8, :] = np.eye(8, dtype=np.float32)
    f8_16 = np.zeros((128, 16), np.float32)
    f8_8 = np.zeros((64, 8), np.float32)
    for k in range(8):
        f8_16[16 * k : 16 * k + 16, :] = np.eye(16, dtype=np.float32)
        f8_8[8 * k : 8 * k + 8, :] = np.eye(8, dtype=np.float32)

    shared = {
        "w1D": _wdiag(g["w1"]).astype(bf),
        "w2D": _wdiag(g["w2"]).astype(bf),
        "w3D": _wdiag(g["w3"]).astype(bf),
        "g1q": _qrep(g["g1"], 16), "be1q": _qrep(g["be1"], 16),
        "g2q": _qrep(g["g2"], 8), "be2q": _qrep(g["be2"], 8),
        "g3s": g["g3"].reshape(1, 1), "be3s": g["be3"].reshape(1, 1),
        "f1": f1, "ft1": np.ascontiguousarray(f1.T),
        "f2": f2, "ft2": np.ascontiguousarray(f2.T),
        "f8_16": f8_16, "f8_8": f8_8,
        "fw2s": np.ascontiguousarray(
            g["fw2"].reshape(256, 8, 128).transpose(2, 1, 0).reshape(
                128, 8 * 256)).astype(bf),
        "fg1t": np.ascontiguousarray(g["fg1"].reshape(8, 128).T),
        "fbe1t": np.ascontiguousarray(g["fbe1"].reshape(8, 128).T),
        "fg2r": g["fg2"].reshape(1, 256),
        "fbe2r": g["fbe2"].reshape(1, 256),
    }

    in_maps = []
    for c in range(NCORES):
        xA4, xB, fw1t = _prep_core(x3, g["fw1"], c)
        m = dict(shared)
        m["xA4"] = xA4
        m["xB"] = xB
        m["fw1t"] = fw1t
        in_maps.append(m)

    from concourse.bass_utils import run_bass_kernel_spmd

    res = run_bass_kernel_spmd(nc, in_maps, core_ids=list(range(NCORES)),
                               trace=bool(_cache.get("trace")))
    _cache["last_result"] = res
    return np.concatenate(
        [np.asarray(res.results[c]["out_final"], np.float32)
         for c in range(NCORES)], axis=0)


if __name__ == "__main__":
    nc = _build()
    print("build ok; instructions:",
          sum(len(bb.instructions) for bb in nc.main_func.blocks))


# revision 28
# speedup vs baseline: 1.1814x; 1.0792x over previous
"""Trainium2 Bass kernel for nn_FCGF_point_att3_sft_7000 (8 NeuronCores).

Model: pointwise attention MLP (32->16->8->1, BN+relu, BN stats over the full
512000-point batch), per-segment softmax over 2000 points, attention-weighted
pooling to [256, 64000], FC head 64000->1024->256 (BN+relu, stats over the
256-segment batch), final L2 row-normalize.

Sharding: points-within-segment. Core c owns points p in [250c, 250(c+1)) of
every segment. Stage A is data-parallel over points with AllGather'd BN stats;
fc1 is contraction-sharded (each core owns 8000 of the 64000 inputs and the
matching fw1 rows) with the output in [segs, feats] orientation so a
ReduceScatter over segments hands each core 32 complete segments; the softmax
denominators ride the same collective as an extra column. The whole tail
(BN1, fc2, BN2, L2-normalize) then runs locally per core on its 32 segments,
with two tiny AllGathers for the cross-segment BN statistics; each core emits
its own [32, 256] slab of the output.

Stage-A layout: "quartered" A-orientation. x.T is [128, 16000] with the
channels of free-quarter a on partitions [32a, 32a+32). Matmuls use
tile_position=(32a, 32a) so outputs land on partitions 32a+ch and every
eviction / BN / softmax op runs 128 partitions wide. Weight tiles are
zero-padded to M=32 so all PSUM rows are defined.

Training-mode BN is shift-invariant => conv/linear biases (b1,b2,b3,fb1,fb2)
drop out exactly; they are accepted and ignored.
"""

import sys

sys.path.insert(0, "/opt/trn_rl_repo")

import numpy as np

import concourse.bass as bass
import concourse.tile as tile
from concourse import mybir
from concourse.masks import make_identity

B = 256
P = 2000
C = 32
NCORES = 8
PL = P // NCORES           # 250
PH = PL // 2               # 125
NPTS = B * PL              # 64000 points per core
QF = NPTS // 4             # 16000 per quarter
NCH = 500                  # stage-A free chunk
NCHUNK = QF // NCH         # 32
SEGC = B // NCORES         # 32 segments per core after the ReduceScatter
RSW = 1028                 # rs payload width: 1024 feats + z + 3 pad
EPS_BN = 1e-5
F32 = mybir.dt.float32
BF16 = mybir.dt.float16  # fp16: same speed as bf16, 8x lower rounding noise
F8 = mybir.dt.float8e3   # e3m4: fc1 weight stream at half the HBM bytes
FW_SCALE = 64.0          # fw1*64 fits e3m4 range; undone in the z-normalize
RG = [list(range(NCORES))]
AF = mybir.ActivationFunctionType

_cache = {}


# ------------------------------------------------------------------ walrus fix
def _install_walrus_patch():
    """This container's walrus accepts only ONE semaphore wait per instruction.
    Spread Tile's end-of-kernel drain waits across single-wait nops, and split
    any instruction carrying >1 waits onto same-engine carrier nops."""
    if _cache.get("patched"):
        return
    from concourse.vector_clock import ScopedClock, VectorClock

    counter = [0]

    def split_waits(nc):
        for bb in nc.main_func.blocks:
            out = []
            changed = False
            for ins in bb.instructions:
                si = ins.sync_info
                waits = list(si.on_wait) if si and si.on_wait else []
                if len(waits) > 1:
                    changed = True
                    for w in waits[:-1]:
                        counter[0] += 1
                        out.append(mybir.InstNoOp(
                            name=f"I-wsplit-{counter[0]}",
                            engine=ins.engine, ins=[], outs=[],
                            sync_info=mybir.SyncInfo(on_wait=[w], on_update=[]),
                            bass_nofuse=True))
                    si.on_wait = waits[-1:]
                out.append(ins)
            if changed:
                try:
                    bb.instructions = out
                except Exception:
                    bb.instructions.clear()
                    for x in out:
                        bb.instructions.append(x)

    def _patched(self, tick_clock, wait_clock):
        nc = self.nc
        gc = tick_clock.global_clock
        n = len(gc)
        for i in range(n):
            if gc[i] > 0:
                vec = [0] * n
                vec[i] = gc[i]
                nop = nc.sync.nop(nofuse=True, hint=f"drain_wait_p{i}")
                wait_clock.add_sem_waits(
                    nop.ins, ScopedClock({None: VectorClock(vec)}))
        nc.sync.drain()
        nc.all_engine_barrier()
        assert self.sems is not None
        popped = nc._tile_sem_poison_stack.pop()
        assert popped is self._sem_poison
        nc.clear_and_free_semaphores(list(self.sems.allocated().values()))
        nc.all_engine_barrier()
        split_waits(nc)

    tile.TileContext._drain_and_barrier = _patched
    _cache["patched"] = True


# ------------------------------------------------------------------ bass build
def _build():
    _install_walrus_patch()
    nc = bass.Bass()

    def ein(name, shape, dt):
        return nc.dram_tensor(name, shape, dt, kind="ExternalInput")

    d = {}
    d["xA4"] = ein("xA4", [128, QF], BF16)
    d["xB"] = ein("xB", [PH, C * 2 * B], BF16)
    d["w1D"] = ein("w1D", [128, 128], BF16)
    d["w2D"] = ein("w2D", [128, 128], BF16)
    d["w3D"] = ein("w3D", [128, 128], BF16)
    for n in ("g1q", "be1q", "g2q", "be2q"):
        d[n] = ein(n, [128, 1], F32)
    d["g3s"] = ein("g3s", [1, 1], F32)
    d["be3s"] = ein("be3s", [1, 1], F32)
    d["f1"] = ein("f1", [128, 16], F32)
    d["ft1"] = ein("ft1", [16, 128], F32)
    d["f2"] = ein("f2", [128, 8], F32)
    d["ft2"] = ein("ft2", [8, 128], F32)
    d["f8_16"] = ein("f8_16", [128, 16], F32)
    d["f8_8"] = ein("f8_8", [64, 8], F32)
    d["fw1t"] = ein("fw1t", [PH, C * 2, 1024], F8)
    d["fw2s"] = ein("fw2s", [128, 8 * 256], BF16)
    d["fg1t"] = ein("fg1t", [128, 8], F32)
    d["fbe1t"] = ein("fbe1t", [128, 8], F32)
    d["fg2r"] = ein("fg2r", [1, 256], F32)
    d["fbe2r"] = ein("fbe2r", [1, 256], F32)
    d["out_final"] = nc.dram_tensor("out_final", [SEGC, 256], F32,
                                    kind="ExternalOutput")
    # collective bounce buffers
    d["warm_i"] = nc.dram_tensor("warm_i", [16, 4], F32)
    d["warm_o"] = nc.dram_tensor("warm_o", [16, 4], F32)
    d["st1_i"] = nc.dram_tensor("st1_i", [16, 2], F32)
    d["st1_o"] = nc.dram_tensor("st1_o", [128, 2], F32)
    d["st2_i"] = nc.dram_tensor("st2_i", [8, 2], F32)
    d["st2_o"] = nc.dram_tensor("st2_o", [64, 2], F32)
    d["st3_i"] = nc.dram_tensor("st3_i", [1, 2], F32)
    d["st3_o"] = nc.dram_tensor("st3_o", [8, 2], F32)
    d["rs_iA"] = nc.dram_tensor("rs_iA", [B, RSW], BF16)
    d["rs_oA"] = nc.dram_tensor("rs_oA", [SEGC, RSW], BF16)
    d["rs_iB"] = nc.dram_tensor("rs_iB", [B, RSW], BF16)
    d["rs_oB"] = nc.dram_tensor("rs_oB", [SEGC, RSW], BF16)
    d["ag4_i"] = nc.dram_tensor("ag4_i", [128, 16], F32)
    d["ag4_o"] = nc.dram_tensor("ag4_o", [128 * NCORES, 16], F32)
    d["ag5_i"] = nc.dram_tensor("ag5_i", [1, 512], F32)
    d["ag5_o"] = nc.dram_tensor("ag5_o", [NCORES, 512], F32)

    with tile.TileContext(nc) as tc:
        _body(nc, tc, d)
    return nc


def _mkstats(nc, pool, mv, count, name):
    """mv [p,2]=(mean,var) -> (sum,sumsq) [p,2]."""
    p = mv.shape[0]
    ss = pool.tile([p, 2], F32, tag=f"ss_{name}")
    nc.vector.tensor_mul(ss[:, 1:2], mv[:, 0:1], mv[:, 0:1])
    nc.vector.tensor_add(ss[:, 1:2], ss[:, 1:2], mv[:, 1:2])
    nc.scalar.mul(ss[:, 0:1], mv[:, 0:1], float(count))
    nc.scalar.mul(ss[:, 1:2], ss[:, 1:2], float(count))
    return ss


def _mv_from_ss(nc, pool, ss, count, name):
    """(sum,sumsq) [p,2] over count -> (mean, rstd) [p,2]."""
    p = ss.shape[0]
    mr = pool.tile([p, 2], F32, tag=f"mr_{name}")
    epst = pool.tile([p, 1], F32, tag=f"eps_{name}")
    nc.vector.memset(epst[:], EPS_BN)
    nc.scalar.mul(mr[:, 0:1], ss[:, 0:1], 1.0 / count)
    nc.scalar.mul(mr[:, 1:2], ss[:, 1:2], 1.0 / count)
    m2 = pool.tile([p, 1], F32, tag=f"m2_{name}")
    nc.vector.tensor_mul(m2[:], mr[:, 0:1], mr[:, 0:1])
    nc.vector.tensor_sub(mr[:, 1:2], mr[:, 1:2], m2[:])
    nc.scalar.activation(mr[:, 1:2], mr[:, 1:2], AF.Sqrt, bias=epst[:])
    nc.vector.reciprocal(mr[:, 1:2], mr[:, 1:2])
    return mr


def _scale_bias(nc, pool, mrq, g, be, name):
    """scale = g*rstd ; bias = be - scale*mean  (all [p,1] per-partition)."""
    p = mrq.shape[0]
    sc = pool.tile([p, 1], F32, tag=f"sc_{name}")
    bi = pool.tile([p, 1], F32, tag=f"bi_{name}")
    nc.vector.tensor_mul(sc[:], g[:], mrq[:, 1:2])
    nc.vector.tensor_mul(bi[:], sc[:], mrq[:, 0:1])
    nc.vector.tensor_sub(bi[:], be[:], bi[:])
    return sc, bi


def _body(nc, tc, d):
    # collective warmup first. warm_i is never written (contents irrelevant),
    # so the op has NO dependencies and the ~55us ncfw startup begins at t=0,
    # overlapping the whole front of the kernel.
    nc.gpsimd.collective_compute(
        "AllReduce", mybir.AluOpType.add, replica_groups=RG,
        ins=[d["warm_i"][:]], outs=[d["warm_o"][:]])
    sing_cm = tc.tile_pool(name="sing", bufs=1)
    big_cm = tc.tile_pool(name="big", bufs=1)
    work_cm = tc.tile_pool(name="work", bufs=1)
    psA_cm = tc.tile_pool(name="psA", bufs=4, space="PSUM")
    psT_cm = tc.tile_pool(name="psT", bufs=2, space="PSUM")
    psS_cm = tc.tile_pool(name="psS", bufs=2, space="PSUM")
    sing = sing_cm.__enter__(); big = big_cm.__enter__()
    work = work_cm.__enter__()
    fw1p_cm = tc.tile_pool(name="fw1p", bufs=8)
    fw1p = fw1p_cm.__enter__()
    psA = psA_cm.__enter__(); psT = psT_cm.__enter__()
    psS = psS_cm.__enter__()

    # ---------------- constants
    def load(name, shape, dt=F32, pool=sing):
        t = pool.tile(shape, dt, tag=name)
        nc.sync.dma_start(t[:], d[name][:])
        return t

    w1D = load("w1D", [128, 128], BF16)
    w2D = load("w2D", [128, 128], BF16)
    w3D = load("w3D", [128, 128], BF16)
    f1s = load("f1", [128, 16])
    ft1s = load("ft1", [16, 128])
    f2s = load("f2", [128, 8])
    ft2s = load("ft2", [8, 128])
    f8_16s = load("f8_16", [128, 16])
    f8_8s = load("f8_8", [64, 8])
    g1 = load("g1q", [128, 1]); be1 = load("be1q", [128, 1])
    g2 = load("g2q", [128, 1]); be2 = load("be2q", [128, 1])
    g3 = load("g3s", [1, 1]); be3 = load("be3s", [1, 1])
    ones128 = sing.tile([128, 1], F32)
    nc.vector.memset(ones128[:], 1.0)
    ones8 = sing.tile([8, 1], F32)
    nc.vector.memset(ones8[:], 1.0)
    ones1x = sing.tile([1, 128], F32)
    nc.vector.memset(ones1x[:], 1.0)
    ones32h = sing.tile([32, 1], BF16)
    nc.vector.memset(ones32h[:], 1.0)
    ones1x32h = sing.tile([1, 32], BF16)
    nc.vector.memset(ones1x32h[:], 1.0)
    ident = sing.tile([128, 128], F32)
    make_identity(nc, ident[:])
    identh = sing.tile([SEGC, SEGC], BF16)
    make_identity(nc, identh[:])

    # ---------------- big loads
    xa = big.tile([128, QF], BF16, tag="slotA")       # slot A: xa -> y2 -> y3q
    nc.sync.dma_start(xa[:], d["xA4"][:])
    xb = big.tile([PH, C * 2 * B], BF16, tag="xb")
    nc.sync.dma_start(xb[:], d["xB"][:])
    xbv = xb[:].rearrange("p (c h s) -> p c h s", c=C, h=2, s=B)

    # fc1 weight prefetch: pool entered at the top so its slots exist from
    # t=0 and the 16.4MB stream overlaps all of stage A. 2-engine rotation.
    FW_CHUNKS = [8] * 8
    fwtiles = []
    _dge = [nc.sync, nc.scalar]
    _off = 0
    for gblk, nits in enumerate(FW_CHUNKS):
        fwt = fw1p.tile([PH, 8, 1024], F8, tag="fw", name=f"fw_{gblk}")
        _dge[gblk % 2].dma_start(fwt[:, :nits, :],
                                 d["fw1t"][:, _off : _off + nits, :])
        fwtiles.append((fwt, _off, nits))
        _off += nits
    fw2s = sing.tile([128, 8 * 256], BF16, tag="fw2s")
    nc.sync.dma_start(fw2s[:], d["fw2s"][:])
    fw2sv = fw2s[:].rearrange("p (t o) -> p t o", t=8)

    def layer_mms(ps, wD, krows, rhs_src, sl):
        nc.tensor.matmul(ps[:], wD[:], rhs_src[:, sl], start=True, stop=True)

    def stage_layer(rhs_src, wT, krows, fold, foldT, f8fold, st_i, st_o,
                    gq, beq, count_local, name, out_tag, wnext=None):
        """Single-pass layer: matmuls -> evict y fp16 (+bn_stats from y),
        fold+AllGather stats. The BN scale is folded into the next layer's
        weights (gamma>0), so the relu pass is a per-partition bias-shift
        split across scalar/vector/gpsimd."""
        y = big.tile([128, QF], BF16, tag=out_tag, name=f"y_{name}")
        stat = work.tile([128, NCHUNK, 6], F32, tag=f"stat_{name}")
        for j in range(NCHUNK):
            ps = psA.tile([128, NCH], F32, tag="psA", name=f"ps_{name}_{j}")
            layer_mms(ps, wT, krows, rhs_src, slice(j * NCH, (j + 1) * NCH))
            nc.scalar.copy(y[:, j * NCH : (j + 1) * NCH], ps[:])
            nc.vector.bn_stats(stat[:, j, :], ps[:])
        mv = work.tile([128, 2], F32, tag=f"mv_{name}")
        nc.vector.bn_aggr(mv[:], stat[:])
        ss = _mkstats(nc, work, mv, count_local, name)
        nfold = fold.shape[1]
        psf = psS.tile([128, 2], F32, tag="small", name=f"psf_{name}")
        nc.tensor.matmul(psf[:nfold, :], fold[:], ss[:], start=True, stop=True)
        sbf = work.tile([nfold, 2], F32, tag=f"sbf_{name}")
        nc.scalar.copy(sbf[:], psf[:nfold, :])
        nc.gpsimd.dma_start(st_i[:], sbf[:])
        nc.gpsimd.collective_compute(
            "AllGather", mybir.AluOpType.bypass, replica_groups=RG,
            ins=[st_i[:]], outs=[st_o[:]])
        agg = work.tile([nfold * NCORES, 2], F32, tag=f"agg_{name}")
        nc.gpsimd.dma_start(agg[:], st_o[:])
        psg = psS.tile([128, 2], F32, tag="small", name=f"psg_{name}")
        nc.tensor.matmul(psg[:nfold, :], f8fold[:], agg[:], start=True,
                         stop=True)
        ssg = work.tile([nfold, 2], F32, tag=f"ssg_{name}")
        nc.scalar.copy(ssg[:], psg[:nfold, :])
        mr = _mv_from_ss(nc, work, ssg, B * P, name)
        psb = psS.tile([128, 2], F32, tag="small", name=f"psb_{name}")
        nc.tensor.matmul(psb[:], foldT[:], mr[:], start=True, stop=True)
        mrq = work.tile([128, 2], F32, tag=f"mrq_{name}")
        nc.scalar.copy(mrq[:], psb[:])
        sc, bi = _scale_bias(nc, work, mrq, gq, beq, name)
        # fold the BN scale into the next layer's weights (gamma>0); the relu
        # pass becomes a bias-shift, split across scalar/vector/gpsimd. Clamp
        # sc away from 0 first: padded partition slots have gamma=0 and the
        # bare reciprocal would make bip = 0*inf = NaN there.
        isc = work.tile([128, 1], F32, tag=f"isc_{name}")
        nc.vector.tensor_scalar_max(isc[:], sc[:], 1e-30)
        nc.vector.reciprocal(isc[:], isc[:])
        bip = work.tile([128, 1], F32, tag=f"bip_{name}")
        nc.vector.tensor_mul(bip[:], bi[:], isc[:])
        wnf = None
        if wnext is not None:
            wnf = sing.tile([128, 128], BF16, tag=f"wnf_{name}")
            nc.vector.tensor_scalar_mul(wnf[:], wnext[:], sc[:])
        for j in range(NCHUNK):
            sl = slice(j * NCH, (j + 1) * NCH)
            if j % 3 == 2:
                nc.scalar.activation(y[:, sl], y[:, sl], AF.Relu, bias=bip[:])
            else:
                nc.vector.tensor_scalar(y[:, sl], y[:, sl], bip[:], 0.0,
                                        mybir.AluOpType.add,
                                        mybir.AluOpType.max)
        return y, wnf

    # ---------------- stage A layers 1 & 2
    h1, w2f = stage_layer(xa, w1D, 32, f1s, ft1s, f8_16s,
                          d["st1_i"], d["st1_o"], g1, be1, QF, "l1", "slotB",
                          wnext=w2D)
    # h2 reuses slot A (xa dead after L1 matmuls)
    h2, w3f = stage_layer(h1, w2f, 16, f2s, ft2s, f8_8s,
                          d["st2_i"], d["st2_o"], g2, be2, QF, "l2", "slotA",
                          wnext=w3D)

    # ---------------- stage A layer 3: scores straight from PSUM into
    # scoreS [128 segs, 2, 250] via per-chunk repack DMAs (rows {32a} real;
    # chunk j of quarter a covers segments 64a+2j..+1)
    y3q = big.tile([128, QF], BF16, tag="slotB", name="y3q")
    for j in range(NCHUNK):
        ps = psA.tile([128, NCH], F32, tag="psA", name=f"ps_l3_{j}")
        layer_mms(ps, w3f, 8, h2, slice(j * NCH, (j + 1) * NCH))
        nc.scalar.copy(y3q[:, j * NCH : (j + 1) * NCH], ps[:])
    scoreS = big.tile([128, 2, PL], BF16, tag="scoreS")
    _sse = [nc.sync, nc.scalar, nc.gpsimd, nc.sync]
    for a in range(4):
        _sse[a].dma_start(
            scoreS[64 * (a % 2) : 64 * (a % 2) + 64, a // 2, :],
            y3q[32 * a : 32 * a + 1, :])
    # BN3 stats over all segments/points (all partitions real)
    stat3 = work.tile([128, 2, 6], F32, tag="stat3")
    nc.vector.bn_stats(stat3[:, 0, :], scoreS[:, 0, :])
    nc.vector.bn_stats(stat3[:, 1, :], scoreS[:, 1, :])
    mv3 = work.tile([128, 2], F32, tag="mv3")
    nc.vector.bn_aggr(mv3[:], stat3[:])
    ss3 = _mkstats(nc, work, mv3, 2 * PL, "l3")
    psf3 = psS.tile([128, 2], F32, tag="small", name="psf3")
    nc.tensor.matmul(psf3[:1, :], ones128[:], ss3[:], start=True, stop=True)
    sbf3 = work.tile([1, 2], F32, tag="sbf3")
    nc.scalar.copy(sbf3[:], psf3[:1, :])
    nc.gpsimd.dma_start(d["st3_i"][:], sbf3[:])
    nc.gpsimd.collective_compute(
        "AllGather", mybir.AluOpType.bypass, replica_groups=RG,
        ins=[d["st3_i"][:]], outs=[d["st3_o"][:]])
    agg3 = work.tile([8, 2], F32, tag="agg3")
    nc.gpsimd.dma_start(agg3[:], d["st3_o"][:])
    psg3 = psS.tile([128, 2], F32, tag="small", name="psg3")
    nc.tensor.matmul(psg3[:1, :], ones8[:], agg3[:], start=True, stop=True)
    ssg3 = work.tile([1, 2], F32, tag="ssg3")
    nc.scalar.copy(ssg3[:], psg3[:1, :])
    mr3 = _mv_from_ss(nc, work, ssg3, B * P, "l3")
    scb1 = work.tile([1, 2], F32, tag="scb1")
    nc.vector.tensor_mul(scb1[:, 0:1], g3[:], mr3[:, 1:2])
    nc.vector.tensor_mul(scb1[:, 1:2], scb1[:, 0:1], mr3[:, 0:1])
    nc.vector.tensor_sub(scb1[:, 1:2], be3[:], scb1[:, 1:2])
    psb3 = psS.tile([128, 2], F32, tag="small", name="psb3")
    nc.tensor.matmul(psb3[:], ones1x[:], scb1[:], start=True, stop=True)
    scb = work.tile([128, 2], F32, tag="scb")
    nc.scalar.copy(scb[:], psb3[:])
    # relu(BN3) in place, then exp
    expS = big.tile([128, 2, PL], F32, tag="expS")
    expT = big.tile([PH, 2, 256], BF16, tag="expT")
    for tt in range(2):
        nc.scalar.activation(scoreS[:, tt, :], scoreS[:, tt, :], AF.Relu,
                             bias=scb[:, 1:2], scale=scb[:, 0:1])
        nc.scalar.activation(expS[:, tt, :], scoreS[:, tt, :], AF.Exp)
        for h in range(2):
            pt_ps = psT.tile([128, 128], F32, tag="psT")
            nc.tensor.transpose(pt_ps[:PH, :],
                                expS[:, tt, h * PH : h * PH + PH], ident[:])
            nc.vector.tensor_copy(expT[:, h, tt * 128 : tt * 128 + 128],
                                  pt_ps[:PH, :])
        # partial softmax denominators
    zloc = work.tile([128, 8], F32, tag="zloc")
    nc.vector.memset(zloc[:], 0.0)
    nc.vector.reduce_sum(zloc[:, 0:1], expS[:, 0, :], axis=mybir.AxisListType.X)
    nc.vector.reduce_sum(zloc[:, 4:5], expS[:, 1, :], axis=mybir.AxisListType.X)
    zpad = work.tile([128, 8], BF16, tag="zpad")
    nc.scalar.copy(zpad[:], zloc[:])
    # z (and zero pad) into columns 1024:1028 of rs_i: row (t p) = seg t*128+p
    nc.sync.dma_start(
        d["rs_iA"][:, 1024:1028].rearrange("(t p) c -> p t c", t=2),
        zpad[:].rearrange("p (t c) -> p t c", t=2))
    zero8 = work.tile([128, 8], BF16, tag="zero8")
    nc.vector.memset(zero8[:], 0.0)
    nc.sync.dma_start(
        d["rs_iB"][:, 1024:1028].rearrange("(t p) c -> p t c", t=2),
        zero8[:].rearrange("p (t c) -> p t c", t=2))

    psS_cm.__exit__(None, None, None)
    psT_cm.__exit__(None, None, None)
    psA_cm.__exit__(None, None, None)

    # ---------------- FC1 (contraction-sharded, out [256 segs, 1024] partial)
    # lhsT = pt seg-halves [125, 128]; rhs = fw1 it-chunk [125, 512]-halves.
    psF_cm = tc.tile_pool(name="psF", bufs=1, space="PSUM")
    ptp_cm = tc.tile_pool(name="ptp", bufs=4)
    psF = psF_cm.__enter__()
    ptp = ptp_cm.__enter__()
    r1ps = [psF.tile([128, 512], F32, name=f"r1ps_{m}", tag=f"r1_{m}")
            for m in range(4)]
    NIT = C * 2
    HIT = NIT // 2

    def fc1_evict(gen, rs_i):
        _dmaeng = [nc.sync, nc.scalar, nc.gpsimd, nc.sync]
        for m in range(4):
            s, f = m // 2, m % 2
            r1sb = big.tile([128, 512], BF16, tag=f"r1sb{gen}",
                            name=f"r1sb{gen}_{m}", bufs=2)
            if m % 2 == 0:
                nc.scalar.copy(r1sb[:], r1ps[m][:])
            else:
                nc.vector.tensor_copy(r1sb[:], r1ps[m][:])
            _dmaeng[m].dma_start(
                rs_i[s * 128 : (s + 1) * 128, f * 512 : (f + 1) * 512],
                r1sb[:])

    for ch in range(C):
        for h in range(2):
            it = ch * 2 + h
            gi = 0
            while not (fwtiles[gi][1] <= it < fwtiles[gi][1] + fwtiles[gi][2]):
                gi += 1
            fw = fwtiles[gi][0][:, it - fwtiles[gi][1], :]
            pt = ptp.tile([PH, 256], BF16, tag="pt", name=f"pt_{it}")
            nc.vector.tensor_mul(pt[:], xbv[:, ch, h, :], expT[:, h, :])
            for s in range(2):
                lhsT = pt[:, s * 128 : (s + 1) * 128]
                for f in range(2):
                    nc.tensor.matmul(
                        r1ps[s * 2 + f][:, :], lhsT,
                        fw[:, f * 512 : (f + 1) * 512],
                        start=(it in (0, HIT)), stop=(it in (HIT - 1, NIT - 1)))
            if it == HIT - 1:
                # first-half partials ship out mid-FC1 so RS#1 overlaps the
                # second half of the contraction.
                fc1_evict("A", d["rs_iA"])
                nc.gpsimd.collective_compute(
                    "ReduceScatter", mybir.AluOpType.add, replica_groups=RG,
                    ins=[d["rs_iA"][:]], outs=[d["rs_oA"][:]])
    fc1_evict("B", d["rs_iB"])
    nc.gpsimd.collective_compute(
        "ReduceScatter", mybir.AluOpType.add, replica_groups=RG,
        ins=[d["rs_iB"][:]], outs=[d["rs_oB"][:]])

    ptp_cm.__exit__(None, None, None)
    psF_cm.__exit__(None, None, None)
    fw1p_cm.__exit__(None, None, None)

    # ---------------- tail: this core owns segments [32c, 32c+32), complete.
    ps2_cm = tc.tile_pool(name="ps2", bufs=2, space="PSUM")
    ps2 = ps2_cm.__enter__()

    r1A = big.tile([SEGC, RSW], BF16, tag="r1A")
    nc.sync.dma_start(r1A[:], d["rs_oA"][:])
    r1B = big.tile([SEGC, RSW], BF16, tag="r1B")
    nc.sync.dma_start(r1B[:], d["rs_oB"][:])
    r1 = big.tile([SEGC, RSW], BF16, tag="r1")
    nc.vector.tensor_add(r1[:], r1A[:], r1B[:])
    zinv = work.tile([SEGC, 1], F32, tag="zinv")
    nc.scalar.mul(zinv[:], r1[:, 1024:1025], FW_SCALE)
    nc.vector.reciprocal(zinv[:], zinv[:])
    r1z = big.tile([SEGC, 1024], BF16, tag="r1z")
    nc.scalar.activation(r1z[:], r1[:, 0:1024], AF.Copy, scale=zinv[:])
    # transpose to 8 feature tiles [128, 32]; BN1 stats over local 32 segs
    r1T = big.tile([128, 8, SEGC], BF16, tag="r1T")
    stat8 = work.tile([128, 8, 6], F32, tag="stat8")
    for t in range(8):
        pt_ps = ps2.tile([128, SEGC], BF16, tag="tp", name=f"tp_{t}")
        nc.tensor.transpose(pt_ps[:], r1z[:, t * 128 : (t + 1) * 128],
                            identh[:])
        nc.scalar.copy(r1T[:, t, :], pt_ps[:])
        nc.vector.bn_stats(stat8[:, t, :], pt_ps[:])
    mv8 = work.tile([128, 8, 2], F32, tag="mv8")
    for t in range(8):
        nc.vector.bn_aggr(mv8[:, t, :], stat8[:, t : t + 1, :])
    ss8 = work.tile([128, 8, 2], F32, tag="ss8")
    nc.vector.tensor_mul(ss8[:, :, 1:2], mv8[:, :, 0:1], mv8[:, :, 0:1])
    nc.vector.tensor_add(ss8[:, :, 1:2], ss8[:, :, 1:2], mv8[:, :, 1:2])
    nc.scalar.mul(ss8[:, :, 0:1], mv8[:, :, 0:1], float(SEGC))
    nc.scalar.mul(ss8[:, :, 1:2], ss8[:, :, 1:2], float(SEGC))
    nc.gpsimd.dma_start(d["ag4_i"][:], ss8[:].rearrange("p t u -> p (t u)"))
    nc.gpsimd.collective_compute(
        "AllGather", mybir.AluOpType.bypass, replica_groups=RG,
        ins=[d["ag4_i"][:]], outs=[d["ag4_o"][:]])
    agg4 = work.tile([128, 8, 16], F32, tag="agg4")
    _age = [nc.sync, nc.scalar, nc.gpsimd]
    for c in range(8):
        _age[c % 3].dma_start(agg4[:, c, :],
                              d["ag4_o"][c * 128 : (c + 1) * 128, :])
    g8 = work.tile([128, 8, 2], F32, tag="g8")
    nc.vector.reduce_sum(g8[:], agg4[:].rearrange("p c w -> p w c"),
                         axis=mybir.AxisListType.X)
    # mean/rstd per feature ([128, 8] per-partition per-tile)
    epsf = work.tile([128, 1], F32, tag="epsf")
    nc.vector.memset(epsf[:], EPS_BN)
    nc.scalar.mul(g8[:, :, 0:1], g8[:, :, 0:1], 1.0 / B)
    nc.scalar.mul(g8[:, :, 1:2], g8[:, :, 1:2], 1.0 / B)
    m2t = work.tile([128, 8], F32, tag="m2t")
    nc.vector.tensor_mul(m2t[:], g8[:, :, 0:1], g8[:, :, 0:1])
    nc.vector.tensor_sub(g8[:, :, 1:2], g8[:, :, 1:2], m2t[:])
    nc.scalar.activation(g8[:, :, 1:2], g8[:, :, 1:2], AF.Sqrt, bias=epsf[:])
    nc.vector.reciprocal(g8[:, :, 1:2], g8[:, :, 1:2])
    fg1t = load("fg1t", [128, 8], pool=work)
    fbe1t = load("fbe1t", [128, 8], pool=work)
    sc1 = work.tile([128, 8], F32, tag="sc1")
    bi1 = work.tile([128, 8], F32, tag="bi1")
    nc.vector.tensor_mul(sc1[:], fg1t[:], g8[:, :, 1:2])
    nc.vector.tensor_mul(bi1[:], sc1[:], g8[:, :, 0:1])
    nc.vector.tensor_sub(bi1[:], fbe1t[:], bi1[:])
    r1Tr = big.tile([128, 8, SEGC], BF16, tag="r1Tr")
    for t in range(8):
        nc.scalar.activation(r1Tr[:, t, :], r1T[:, t, :], AF.Relu,
                             bias=bi1[:, t : t + 1], scale=sc1[:, t : t + 1])
    # FC2: out [32 segs, 256] complete (contraction over 1024 feats, local)
    ps_r2 = ps2.tile([SEGC, 256], F32, tag="r2", bufs=1)
    for t in range(8):
        nc.tensor.matmul(ps_r2[:], r1Tr[:, t, :], fw2sv[:, t, :],
                         start=(t == 0), stop=(t == 7))
    r2st = big.tile([SEGC, 512], BF16, tag="r2st")
    nc.scalar.copy(r2st[:, 0:256], ps_r2[:])
    nc.scalar.activation(r2st[:, 256:512], ps_r2[:], AF.Square)
    ps_s5 = ps2.tile([1, 512], F32, tag="s5", bufs=1)
    nc.tensor.matmul(ps_s5[:], ones32h[:], r2st[:], start=True, stop=True)
    s5 = work.tile([1, 512], F32, tag="s5sb")
    nc.scalar.copy(s5[:], ps_s5[:])
    nc.gpsimd.dma_start(d["ag5_i"][:], s5[:])
    nc.gpsimd.collective_compute(
        "AllGather", mybir.AluOpType.bypass, replica_groups=RG,
        ins=[d["ag5_i"][:]], outs=[d["ag5_o"][:]])
    agg5 = work.tile([8, 512], F32, tag="agg5")
    nc.sync.dma_start(agg5[:], d["ag5_o"][:])
    agg5h = work.tile([8, 512], BF16, tag="agg5h")
    nc.vector.tensor_copy(agg5h[:], agg5[:])
    ones8h = sing.tile([8, 1], BF16)
    nc.vector.memset(ones8h[:], 1.0)
    ps_g5 = ps2.tile([1, 512], F32, tag="g5", bufs=1)
    nc.tensor.matmul(ps_g5[:], ones8h[:], agg5h[:], start=True, stop=True)
    g5 = work.tile([1, 512], F32, tag="g5sb")
    nc.scalar.copy(g5[:], ps_g5[:])
    # scale/bias rows [1, 256] -> packed scb5 [1, 512] fp16 for PE broadcast
    eps1 = work.tile([1, 1], F32, tag="eps1")
    nc.vector.memset(eps1[:], EPS_BN)
    nc.scalar.mul(g5[:, 0:256], g5[:, 0:256], 1.0 / B)
    nc.scalar.mul(g5[:, 256:512], g5[:, 256:512], 1.0 / B)
    m2r = work.tile([1, 256], F32, tag="m2r")
    nc.vector.tensor_mul(m2r[:], g5[:, 0:256], g5[:, 0:256])
    nc.vector.tensor_sub(g5[:, 256:512], g5[:, 256:512], m2r[:])
    nc.scalar.activation(g5[:, 256:512], g5[:, 256:512], AF.Sqrt, bias=eps1[:])
    nc.vector.reciprocal(g5[:, 256:512], g5[:, 256:512])
    fg2r = load("fg2r", [1, 256], pool=work)
    fbe2r = load("fbe2r", [1, 256], pool=work)
    scb5 = work.tile([1, 512], BF16, tag="scb5")
    sc2f = work.tile([1, 256], F32, tag="sc2f")
    nc.vector.tensor_mul(sc2f[:], fg2r[:], g5[:, 256:512])
    nc.scalar.copy(scb5[:, 0:256], sc2f[:])
    bi2f = work.tile([1, 256], F32, tag="bi2f")
    nc.vector.tensor_mul(bi2f[:], sc2f[:], g5[:, 0:256])
    nc.vector.tensor_sub(bi2f[:], fbe2r[:], bi2f[:])
    nc.scalar.copy(scb5[:, 256:512], bi2f[:])
    ps_bc = ps2.tile([SEGC, 512], F32, tag="bc", bufs=1)
    nc.tensor.matmul(ps_bc[:], ones1x32h[:], scb5[:], start=True, stop=True)
    # apply BN2 + relu (per-column scale/bias via broadcast tiles)
    r2n = big.tile([SEGC, 256], BF16, tag="r2n")
    nc.vector.tensor_mul(r2n[:], r2st[:, 0:256], ps_bc[:, 0:256])
    nc.vector.tensor_add(r2n[:], r2n[:], ps_bc[:, 256:512])
    nc.vector.tensor_scalar_max(r2n[:], r2n[:], 0.0)
    # L2 normalize rows, write this core's [32, 256] slab
    nsq = work.tile([SEGC, 256], F32, tag="nsq")
    nc.scalar.activation(nsq[:], r2n[:], AF.Square)
    nrm = work.tile([SEGC, 1], F32, tag="nrm")
    nc.vector.reduce_sum(nrm[:], nsq[:], axis=mybir.AxisListType.X)
    nc.scalar.activation(nrm[:], nrm[:], AF.Sqrt)
    nc.vector.tensor_scalar_max(nrm[:], nrm[:], 1e-12)
    nc.vector.reciprocal(nrm[:], nrm[:])
    outf = big.tile([SEGC, 256], F32, tag="outf")
    nc.scalar.activation(outf[:], r2n[:], AF.Copy, scale=nrm[:])
    nc.sync.dma_start(d["out_final"][:], outf[:])

    ps2_cm.__exit__(None, None, None)
    work_cm.__exit__(None, None, None)
    big_cm.__exit__(None, None, None)
    sing_cm.__exit__(None, None, None)


# ------------------------------------------------------------------ host side
def _prep_core(x3, fw1, c):
    import ml_dtypes
    xs = x3[:, PL * c : PL * (c + 1), :]                       # [256,250,32]
    arr = np.ascontiguousarray(xs.transpose(2, 0, 1))          # [32,256,250]
    xA4 = arr.reshape(C, 4, QF).transpose(1, 0, 2).reshape(128, QF)
    xb = xs.reshape(B, 2, PH, C).transpose(2, 3, 1, 0)         # [125,32,2,256]
    xB = np.ascontiguousarray(xb).reshape(PH, C * 2 * B)
    fw = fw1.reshape(1024, P, C)[:, PL * c : PL * (c + 1), :]
    fw = fw.reshape(1024, 2, PH, C).transpose(2, 3, 1, 0)      # [125,32,2,1024]
    fw1t = np.ascontiguousarray(fw).reshape(PH, C * 2, 1024)
    bf = np.float16
    f8 = ml_dtypes.float8_e3m4
    return (np.ascontiguousarray(xA4).astype(bf), xB.astype(bf),
            (fw1t * 64.0).astype(f8))


def _qrep(v, rows):
    out = np.zeros((128, 1), np.float32)
    for a in range(4):
        out[32 * a : 32 * a + rows, 0] = v
    return out


def _wdiag(w):
    """w [out,in] -> block-diagonal lhsT [128, 128]: block a (32x32) holds
    w.T in its top-left corner."""
    t = np.zeros((128, 128), np.float32)
    wt = w.T  # [in, out]
    for a in range(4):
        t[32 * a : 32 * a + wt.shape[0], 32 * a : 32 * a + wt.shape[1]] = wt
    return t


def kernel(**inputs):
    import ml_dtypes

    if "nc" not in _cache:
        _cache["nc"] = _build()
    nc = _cache["nc"]
    bf = np.float16

    g = {k: np.asarray(v, np.float32) for k, v in inputs.items()
         if k != "length"}
    x3 = g["x"].reshape(B, P, C)

    f1 = np.zeros((128, 16), np.float32)
    f2 = np.zeros((128, 8), np.float32)
    for a in range(4):
        f1[32 * a : 32 * a + 16, :] = np.eye(16, dtype=np.float32)
        f2[32 * a : 32 * a + 8, :] = np.eye(8, dtype=np.float32)
    f8_16 = np.zeros((128, 16), np.float32)
    f8_8 = np.zeros((64, 8), np.float32)
    for k in range(8):
        f8_16[16 * k : 16 * k + 16, :] = np.eye(16, dtype=np.float32)
        f8_8[8 * k : 8 * k + 8, :] = np.eye(8, dtype=np.float32)

    shared = {
        "w1D": _wdiag(g["w1"]).astype(bf),
        "w2D": _wdiag(g["w2"]).astype(bf),
        "w3D": _wdiag(g["w3"]).astype(bf),
        "g1q": _qrep(g["g1"], 16), "be1q": _qrep(g["be1"], 16),
        "g2q": _qrep(g["g2"], 8), "be2q": _qrep(g["be2"], 8),
        "g3s": g["g3"].reshape(1, 1), "be3s": g["be3"].reshape(1, 1),
        "f1": f1, "ft1": np.ascontiguousarray(f1.T),
        "f2": f2, "ft2": np.ascontiguousarray(f2.T),
        "f8_16": f8_16, "f8_8": f8_8,
        "fw2s": np.ascontiguousarray(
            g["fw2"].reshape(256, 8, 128).transpose(2, 1, 0).reshape(
                128, 8 * 256)).astype(bf),
        "fg1t": np.ascontiguousarray(g["fg1"].reshape(8, 128).T),
        "fbe1t": np.ascontiguousarray(g["fbe1"].reshape(8, 128).T),
        "fg2r": g["fg2"].reshape(1, 256),
        "fbe2r": g["fbe2"].reshape(1, 256),
    }

    in_maps = []
    for c in range(NCORES):
        xA4, xB, fw1t = _prep_core(x3, g["fw1"], c)
        m = dict(shared)
        m["xA4"] = xA4
        m["xB"] = xB
        m["fw1t"] = fw1t
        in_maps.append(m)

    from concourse.bass_utils import run_bass_kernel_spmd

    res = run_bass_kernel_spmd(nc, in_maps, core_ids=list(range(NCORES)),
                               trace=bool(_cache.get("trace")))
    _cache["last_result"] = res
    return np.concatenate(
        [np.asarray(res.results[c]["out_final"], np.float32)
         for c in range(NCORES)], axis=0)


if __name__ == "__main__":
    nc = _build()
    print("build ok; instructions:",
          sum(len(bb.instructions) for bb in nc.main_func.blocks))


# revision 32
# speedup vs baseline: 1.2403x; 1.0499x over previous
"""Trainium2 Bass kernel for nn_FCGF_point_att3_sft_7000 (8 NeuronCores).

Model: pointwise attention MLP (32->16->8->1, BN+relu, BN stats over the full
512000-point batch), per-segment softmax over 2000 points, attention-weighted
pooling to [256, 64000], FC head 64000->1024->256 (BN+relu, stats over the
256-segment batch), final L2 row-normalize.

Sharding: points-within-segment. Core c owns points p in [250c, 250(c+1)) of
every segment. Stage A is data-parallel over points with AllGather'd BN stats;
fc1 is contraction-sharded (each core owns 8000 of the 64000 inputs and the
matching fw1 rows) with the output in [segs, feats] orientation so a
ReduceScatter over segments hands each core 32 complete segments; the softmax
denominators ride the same collective as an extra column. The whole tail
(BN1, fc2, BN2, L2-normalize) then runs locally per core on its 32 segments,
with two tiny AllGathers for the cross-segment BN statistics; each core emits
its own [32, 256] slab of the output.

Stage-A layout: "quartered" A-orientation. x.T is [128, 16000] with the
channels of free-quarter a on partitions [32a, 32a+32). Matmuls use
tile_position=(32a, 32a) so outputs land on partitions 32a+ch and every
eviction / BN / softmax op runs 128 partitions wide. Weight tiles are
zero-padded to M=32 so all PSUM rows are defined.

Training-mode BN is shift-invariant => conv/linear biases (b1,b2,b3,fb1,fb2)
drop out exactly; they are accepted and ignored.
"""

import sys

sys.path.insert(0, "/opt/trn_rl_repo")

import numpy as np

import concourse.bass as bass
import concourse.tile as tile
from concourse import mybir
from concourse.masks import make_identity

B = 256
P = 2000
C = 32
NCORES = 8
PL = P // NCORES           # 250
PH = PL // 2               # 125
NPTS = B * PL              # 64000 points per core
QF = NPTS // 4             # 16000 per quarter
NCH = 500                  # stage-A free chunk
NCHUNK = QF // NCH         # 32
SEGC = B // NCORES         # 32 segments per core after the ReduceScatter
RSW = 1028                 # rs payload width: 1024 feats + z + 3 pad
EPS_BN = 1e-5
F32 = mybir.dt.float32
BF16 = mybir.dt.float16  # fp16: same speed as bf16, 8x lower rounding noise
F8 = mybir.dt.float8e3   # e3m4: fc1 weight stream at half the HBM bytes
FW_SCALE = 64.0          # fw1*64 fits e3m4 range; undone in the z-normalize
RG = [list(range(NCORES))]
AF = mybir.ActivationFunctionType

_cache = {}


# ------------------------------------------------------------------ walrus fix
def _install_walrus_patch():
    """This container's walrus accepts only ONE semaphore wait per instruction.
    Spread Tile's end-of-kernel drain waits across single-wait nops, and split
    any instruction carrying >1 waits onto same-engine carrier nops."""
    if _cache.get("patched"):
        return
    from concourse.vector_clock import ScopedClock, VectorClock

    counter = [0]

    def split_waits(nc):
        for bb in nc.main_func.blocks:
            out = []
            changed = False
            for ins in bb.instructions:
                si = ins.sync_info
                waits = list(si.on_wait) if si and si.on_wait else []
                if len(waits) > 1:
                    changed = True
                    for w in waits[:-1]:
                        counter[0] += 1
                        out.append(mybir.InstNoOp(
                            name=f"I-wsplit-{counter[0]}",
                            engine=ins.engine, ins=[], outs=[],
                            sync_info=mybir.SyncInfo(on_wait=[w], on_update=[]),
                            bass_nofuse=True))
                    si.on_wait = waits[-1:]
                out.append(ins)
            if changed:
                try:
                    bb.instructions = out
                except Exception:
                    bb.instructions.clear()
                    for x in out:
                        bb.instructions.append(x)

    def _patched(self, tick_clock, wait_clock):
        nc = self.nc
        gc = tick_clock.global_clock
        n = len(gc)
        for i in range(n):
            if gc[i] > 0:
                vec = [0] * n
                vec[i] = gc[i]
                nop = nc.sync.nop(nofuse=True, hint=f"drain_wait_p{i}")
                wait_clock.add_sem_waits(
                    nop.ins, ScopedClock({None: VectorClock(vec)}))
        nc.sync.drain()
        nc.all_engine_barrier()
        assert self.sems is not None
        popped = nc._tile_sem_poison_stack.pop()
        assert popped is self._sem_poison
        nc.clear_and_free_semaphores(list(self.sems.allocated().values()))
        nc.all_engine_barrier()
        split_waits(nc)

    tile.TileContext._drain_and_barrier = _patched
    _cache["patched"] = True


# ------------------------------------------------------------------ bass build
def _build():
    _install_walrus_patch()
    nc = bass.Bass()

    def ein(name, shape, dt):
        return nc.dram_tensor(name, shape, dt, kind="ExternalInput")

    d = {}
    d["xA4"] = ein("xA4", [128, QF], BF16)
    d["xB"] = ein("xB", [PH, C * 2 * B], BF16)
    d["w1D"] = ein("w1D", [128, 128], BF16)
    d["w2K"] = ein("w2K", [128, 4 * 128], BF16)
    d["w3D"] = ein("w3D", [128, 128], BF16)
    for n in ("g1q", "be1q", "g2q", "be2q"):
        d[n] = ein(n, [128, 1], F32)
    d["g3s"] = ein("g3s", [1, 1], F32)
    d["be3s"] = ein("be3s", [1, 1], F32)
    d["f1"] = ein("f1", [128, 16], F32)
    d["ft1"] = ein("ft1", [16, 128], F32)
    d["f2"] = ein("f2", [128, 8], F32)
    d["ft2"] = ein("ft2", [8, 128], F32)
    d["f8_16"] = ein("f8_16", [128, 16], F32)
    d["f8_8"] = ein("f8_8", [64, 8], F32)
    d["fw1t"] = ein("fw1t", [PH, C * 2, 1024], F8)
    d["fw2s"] = ein("fw2s", [128, 8 * 256], BF16)
    d["fg1t"] = ein("fg1t", [128, 8], F32)
    d["fbe1t"] = ein("fbe1t", [128, 8], F32)
    d["fg2r"] = ein("fg2r", [1, 256], F32)
    d["fbe2r"] = ein("fbe2r", [1, 256], F32)
    d["out_final"] = nc.dram_tensor("out_final", [SEGC, 256], F32,
                                    kind="ExternalOutput")
    # collective bounce buffers
    d["warm_i"] = nc.dram_tensor("warm_i", [16, 4], F32)
    d["warm_o"] = nc.dram_tensor("warm_o", [16, 4], F32)
    d["st1_i"] = nc.dram_tensor("st1_i", [16, 2], F32)
    d["st1_o"] = nc.dram_tensor("st1_o", [128, 2], F32)
    d["st2_i"] = nc.dram_tensor("st2_i", [8, 2], F32)
    d["st2_o"] = nc.dram_tensor("st2_o", [64, 2], F32)
    d["st3_i"] = nc.dram_tensor("st3_i", [1, 2], F32)
    d["st3_o"] = nc.dram_tensor("st3_o", [8, 2], F32)
    d["rs_iA"] = nc.dram_tensor("rs_iA", [B, RSW], BF16)
    d["rs_oA"] = nc.dram_tensor("rs_oA", [SEGC, RSW], BF16)
    d["rs_iB"] = nc.dram_tensor("rs_iB", [B, RSW], BF16)
    d["rs_oB"] = nc.dram_tensor("rs_oB", [SEGC, RSW], BF16)
    d["ag4_i"] = nc.dram_tensor("ag4_i", [128, 16], F32)
    d["ag4_o"] = nc.dram_tensor("ag4_o", [128 * NCORES, 16], F32)
    d["ag5_i"] = nc.dram_tensor("ag5_i", [1, 512], F32)
    d["ag5_o"] = nc.dram_tensor("ag5_o", [NCORES, 512], F32)

    with tile.TileContext(nc) as tc:
        _body(nc, tc, d)
    return nc


def _mkstats(nc, pool, mv, count, name):
    """mv [p,2]=(mean,var) -> (sum,sumsq) [p,2]."""
    p = mv.shape[0]
    ss = pool.tile([p, 2], F32, tag=f"ss_{name}")
    nc.vector.tensor_mul(ss[:, 1:2], mv[:, 0:1], mv[:, 0:1])
    nc.vector.tensor_add(ss[:, 1:2], ss[:, 1:2], mv[:, 1:2])
    nc.scalar.mul(ss[:, 0:1], mv[:, 0:1], float(count))
    nc.scalar.mul(ss[:, 1:2], ss[:, 1:2], float(count))
    return ss


def _mv_from_ss(nc, pool, ss, count, name):
    """(sum,sumsq) [p,2] over count -> (mean, rstd) [p,2]."""
    p = ss.shape[0]
    mr = pool.tile([p, 2], F32, tag=f"mr_{name}")
    epst = pool.tile([p, 1], F32, tag=f"eps_{name}")
    nc.vector.memset(epst[:], EPS_BN)
    nc.scalar.mul(mr[:, 0:1], ss[:, 0:1], 1.0 / count)
    nc.scalar.mul(mr[:, 1:2], ss[:, 1:2], 1.0 / count)
    m2 = pool.tile([p, 1], F32, tag=f"m2_{name}")
    nc.vector.tensor_mul(m2[:], mr[:, 0:1], mr[:, 0:1])
    nc.vector.tensor_sub(mr[:, 1:2], mr[:, 1:2], m2[:])
    nc.scalar.activation(mr[:, 1:2], mr[:, 1:2], AF.Sqrt, bias=epst[:])
    nc.vector.reciprocal(mr[:, 1:2], mr[:, 1:2])
    return mr


def _scale_bias(nc, pool, mrq, g, be, name):
    """scale = g*rstd ; bias = be - scale*mean  (all [p,1] per-partition)."""
    p = mrq.shape[0]
    sc = pool.tile([p, 1], F32, tag=f"sc_{name}")
    bi = pool.tile([p, 1], F32, tag=f"bi_{name}")
    nc.vector.tensor_mul(sc[:], g[:], mrq[:, 1:2])
    nc.vector.tensor_mul(bi[:], sc[:], mrq[:, 0:1])
    nc.vector.tensor_sub(bi[:], be[:], bi[:])
    return sc, bi


def _body(nc, tc, d):
    # collective warmup first. warm_i is never written (contents irrelevant),
    # so the op has NO dependencies and the ~55us ncfw startup begins at t=0,
    # overlapping the whole front of the kernel.
    nc.gpsimd.collective_compute(
        "AllReduce", mybir.AluOpType.add, replica_groups=RG,
        ins=[d["warm_i"][:]], outs=[d["warm_o"][:]])
    sing_cm = tc.tile_pool(name="sing", bufs=1)
    big_cm = tc.tile_pool(name="big", bufs=1)
    work_cm = tc.tile_pool(name="work", bufs=1)
    psA_cm = tc.tile_pool(name="psA", bufs=4, space="PSUM")
    psT_cm = tc.tile_pool(name="psT", bufs=2, space="PSUM")
    psS_cm = tc.tile_pool(name="psS", bufs=2, space="PSUM")
    sing = sing_cm.__enter__(); big = big_cm.__enter__()
    work = work_cm.__enter__()
    fw1p_cm = tc.tile_pool(name="fw1p", bufs=6)
    fw1p = fw1p_cm.__enter__()
    psA = psA_cm.__enter__(); psT = psT_cm.__enter__()
    psS = psS_cm.__enter__()

    # ---------------- constants
    def load(name, shape, dt=F32, pool=sing):
        t = pool.tile(shape, dt, tag=name)
        nc.sync.dma_start(t[:], d[name][:])
        return t

    w1D = load("w1D", [128, 128], BF16)
    w2Kt = load("w2K", [128, 4 * 128], BF16)
    w3D = load("w3D", [128, 128], BF16)
    f1s = load("f1", [128, 16])
    ft1s = load("ft1", [16, 128])
    f2s = load("f2", [128, 8])
    ft2s = load("ft2", [8, 128])
    f8_16s = load("f8_16", [128, 16])
    f8_8s = load("f8_8", [64, 8])
    g1 = load("g1q", [128, 1]); be1 = load("be1q", [128, 1])
    g2 = load("g2q", [128, 1]); be2 = load("be2q", [128, 1])
    g3 = load("g3s", [1, 1]); be3 = load("be3s", [1, 1])
    ones128 = sing.tile([128, 1], F32)
    nc.vector.memset(ones128[:], 1.0)
    ones8 = sing.tile([8, 1], F32)
    nc.vector.memset(ones8[:], 1.0)
    ones1x = sing.tile([1, 128], F32)
    nc.vector.memset(ones1x[:], 1.0)
    ones32h = sing.tile([32, 1], BF16)
    nc.vector.memset(ones32h[:], 1.0)
    ones1x32h = sing.tile([1, 32], BF16)
    nc.vector.memset(ones1x32h[:], 1.0)
    ident = sing.tile([128, 128], F32)
    make_identity(nc, ident[:])
    identh = sing.tile([SEGC, SEGC], BF16)
    make_identity(nc, identh[:])

    # ---------------- big loads
    xa = big.tile([128, QF], BF16, tag="slotA")       # slot A: xa -> y2 -> y3q
    nc.sync.dma_start(xa[:], d["xA4"][:])
    xb = big.tile([PH, C * 2 * B], BF16, tag="xb")
    nc.sync.dma_start(xb[:], d["xB"][:])
    xbv = xb[:].rearrange("p (c h s) -> p c h s", c=C, h=2, s=B)

    # fc1 weight prefetch: pool entered at the top so its slots exist from
    # t=0 and the 16.4MB stream overlaps all of stage A. 2-engine rotation.
    FW_CHUNKS = [8] * 8
    fwtiles = []
    _dge = [nc.sync, nc.scalar]
    _off = 0
    for gblk, nits in enumerate(FW_CHUNKS):
        fwt = fw1p.tile([PH, 8, 1024], F8, tag="fw", name=f"fw_{gblk}")
        _dge[gblk % 2].dma_start(fwt[:, :nits, :],
                                 d["fw1t"][:, _off : _off + nits, :])
        fwtiles.append((fwt, _off, nits))
        _off += nits
    fw2s = sing.tile([128, 8 * 256], BF16, tag="fw2s")
    nc.sync.dma_start(fw2s[:], d["fw2s"][:])
    fw2sv = fw2s[:].rearrange("p (t o) -> p t o", t=8)

    def layer_mms(ps, wD, krows, rhs_src, sl):
        nc.tensor.matmul(ps[:], wD[:], rhs_src[:, sl], start=True, stop=True)

    def stage_layer(mm_emit, nchunks, fold, foldT, f8fold, st_i, st_o,
                    gq, beq, count_local, name, out_tag, wnext=None):
        """Single-pass layer: matmuls -> evict y fp16 (+bn_stats from PSUM),
        fold+AllGather stats. The BN scale is folded into the next layer's
        weights (gamma>0), so the relu pass is a per-partition bias-shift
        split across scalar/vector."""
        y = big.tile([128, nchunks * NCH], BF16, tag=out_tag,
                     name=f"y_{name}")
        stat = work.tile([128, nchunks, 6], F32, tag=f"stat_{name}")
        for j in range(nchunks):
            ps = psA.tile([128, NCH], F32, tag="psA", name=f"ps_{name}_{j}")
            mm_emit(ps, j)
            nc.scalar.copy(y[:, j * NCH : (j + 1) * NCH], ps[:])
            nc.vector.bn_stats(stat[:, j, :], ps[:])
        mv = work.tile([128, 2], F32, tag=f"mv_{name}")
        nc.vector.bn_aggr(mv[:], stat[:])
        ss = _mkstats(nc, work, mv, count_local, name)
        nfold = fold.shape[1]
        psf = psS.tile([128, 2], F32, tag="small", name=f"psf_{name}")
        nc.tensor.matmul(psf[:nfold, :], fold[:], ss[:], start=True, stop=True)
        sbf = work.tile([nfold, 2], F32, tag=f"sbf_{name}")
        nc.scalar.copy(sbf[:], psf[:nfold, :])
        nc.gpsimd.dma_start(st_i[:], sbf[:])
        nc.gpsimd.collective_compute(
            "AllGather", mybir.AluOpType.bypass, replica_groups=RG,
            ins=[st_i[:]], outs=[st_o[:]])
        agg = work.tile([nfold * NCORES, 2], F32, tag=f"agg_{name}")
        nc.gpsimd.dma_start(agg[:], st_o[:])
        psg = psS.tile([128, 2], F32, tag="small", name=f"psg_{name}")
        nc.tensor.matmul(psg[:nfold, :], f8fold[:], agg[:], start=True,
                         stop=True)
        ssg = work.tile([nfold, 2], F32, tag=f"ssg_{name}")
        nc.scalar.copy(ssg[:], psg[:nfold, :])
        mr = _mv_from_ss(nc, work, ssg, B * P, name)
        psb = psS.tile([128, 2], F32, tag="small", name=f"psb_{name}")
        nc.tensor.matmul(psb[:], foldT[:], mr[:], start=True, stop=True)
        mrq = work.tile([128, 2], F32, tag=f"mrq_{name}")
        nc.scalar.copy(mrq[:], psb[:])
        sc, bi = _scale_bias(nc, work, mrq, gq, beq, name)
        # fold the BN scale into the next layer's weights (gamma>0); the relu
        # pass becomes a bias-shift, split across scalar/vector/gpsimd. Clamp
        # sc away from 0 first: padded partition slots have gamma=0 and the
        # bare reciprocal would make bip = 0*inf = NaN there.
        isc = work.tile([128, 1], F32, tag=f"isc_{name}")
        nc.vector.tensor_scalar_max(isc[:], sc[:], 1e-30)
        nc.vector.reciprocal(isc[:], isc[:])
        bip = work.tile([128, 1], F32, tag=f"bip_{name}")
        nc.vector.tensor_mul(bip[:], bi[:], isc[:])
        wnf = None
        if wnext is not None:
            wnf = sing.tile(list(wnext.shape), BF16, tag=f"wnf_{name}")
            nc.vector.tensor_scalar_mul(wnf[:], wnext[:], sc[:])
        for j in range(nchunks):
            sl = slice(j * NCH, (j + 1) * NCH)
            if j % 3 == 2:
                nc.scalar.activation(y[:, sl], y[:, sl], AF.Relu, bias=bip[:])
            else:
                nc.vector.tensor_scalar(y[:, sl], y[:, sl], bip[:], 0.0,
                                        mybir.AluOpType.add,
                                        mybir.AluOpType.max)
        return y, wnf

    # ---------------- stage A layers 1 & 2
    def l1_mm(ps, j):
        nc.tensor.matmul(ps[:], w1D[:], xa[:, j * NCH : (j + 1) * NCH],
                         start=True, stop=True)

    h1, w2f = stage_layer(l1_mm, NCHUNK, f1s, ft1s, f8_16s,
                          d["st1_i"], d["st1_o"], g1, be1, QF, "l1", "slotB",
                          wnext=w2Kt)
    w2fv = w2f[:].rearrange("p (k m) -> p k m", k=4)

    def l2_mm(ps, t):
        # 4 column-shifted weight variants stack 4 point-chunks into the
        # 4x(4x8) partition rows of one psum tile (accumulation unions them).
        # k-strided chunk assignment keeps each score row's segments
        # contiguous for the scoreS extraction DMA.
        for k in range(4):
            j = t + 8 * k
            nc.tensor.matmul(ps[:], w2fv[:, k, :],
                             h1[:, j * NCH : (j + 1) * NCH],
                             start=(k == 0), stop=(k == 3))

    h2, w3f = stage_layer(l2_mm, NCHUNK // 4, f2s, ft2s, f8_8s,
                          d["st2_i"], d["st2_o"], g2, be2, QF // 4, "l2",
                          "y2s", wnext=w3D)

    # ---------------- stage A layer 3: scores straight from PSUM into
    # scoreS [128 segs, 2, 250] via per-chunk repack DMAs (rows {32a} real;
    # chunk j of quarter a covers segments 64a+2j..+1)
    y3q = big.tile([128, QF // 4], BF16, tag="y3s", name="y3q")
    for t in range(NCHUNK // 4):
        ps = psA.tile([128, NCH], F32, tag="psA", name=f"ps_l3_{t}")
        nc.tensor.matmul(ps[:], w3f[:], h2[:, t * NCH : (t + 1) * NCH],
                         start=True, stop=True)
        nc.scalar.copy(y3q[:, t * NCH : (t + 1) * NCH], ps[:])
    scoreS = big.tile([128, 2, PL], BF16, tag="scoreS")
    _sse = [nc.sync, nc.scalar, nc.gpsimd, nc.sync]
    for a in range(4):
        for k in range(4):
            _sse[(4 * a + k) % 4].dma_start(
                scoreS[64 * (a % 2) + 16 * k : 64 * (a % 2) + 16 * k + 16,
                       a // 2, :],
                y3q[32 * a + 8 * k : 32 * a + 8 * k + 1, :])
    # BN3 stats over all segments/points (all partitions real)
    stat3 = work.tile([128, 2, 6], F32, tag="stat3")
    nc.vector.bn_stats(stat3[:, 0, :], scoreS[:, 0, :])
    nc.vector.bn_stats(stat3[:, 1, :], scoreS[:, 1, :])
    mv3 = work.tile([128, 2], F32, tag="mv3")
    nc.vector.bn_aggr(mv3[:], stat3[:])
    ss3 = _mkstats(nc, work, mv3, 2 * PL, "l3")
    psf3 = psS.tile([128, 2], F32, tag="small", name="psf3")
    nc.tensor.matmul(psf3[:1, :], ones128[:], ss3[:], start=True, stop=True)
    sbf3 = work.tile([1, 2], F32, tag="sbf3")
    nc.scalar.copy(sbf3[:], psf3[:1, :])
    nc.gpsimd.dma_start(d["st3_i"][:], sbf3[:])
    nc.gpsimd.collective_compute(
        "AllGather", mybir.AluOpType.bypass, replica_groups=RG,
        ins=[d["st3_i"][:]], outs=[d["st3_o"][:]])
    agg3 = work.tile([8, 2], F32, tag="agg3")
    nc.gpsimd.dma_start(agg3[:], d["st3_o"][:])
    psg3 = psS.tile([128, 2], F32, tag="small", name="psg3")
    nc.tensor.matmul(psg3[:1, :], ones8[:], agg3[:], start=True, stop=True)
    ssg3 = work.tile([1, 2], F32, tag="ssg3")
    nc.scalar.copy(ssg3[:], psg3[:1, :])
    mr3 = _mv_from_ss(nc, work, ssg3, B * P, "l3")
    scb1 = work.tile([1, 2], F32, tag="scb1")
    nc.vector.tensor_mul(scb1[:, 0:1], g3[:], mr3[:, 1:2])
    nc.vector.tensor_mul(scb1[:, 1:2], scb1[:, 0:1], mr3[:, 0:1])
    nc.vector.tensor_sub(scb1[:, 1:2], be3[:], scb1[:, 1:2])
    psb3 = psS.tile([128, 2], F32, tag="small", name="psb3")
    nc.tensor.matmul(psb3[:], ones1x[:], scb1[:], start=True, stop=True)
    scb = work.tile([128, 2], F32, tag="scb")
    nc.scalar.copy(scb[:], psb3[:])
    # relu(BN3) in place, then exp
    expS = big.tile([128, 2, PL], F32, tag="expS")
    expT = big.tile([PH, 2, 256], BF16, tag="expT")
    for tt in range(2):
        nc.scalar.activation(scoreS[:, tt, :], scoreS[:, tt, :], AF.Relu,
                             bias=scb[:, 1:2], scale=scb[:, 0:1])
        nc.scalar.activation(expS[:, tt, :], scoreS[:, tt, :], AF.Exp)
        for h in range(2):
            pt_ps = psT.tile([128, 128], F32, tag="psT")
            nc.tensor.transpose(pt_ps[:PH, :],
                                expS[:, tt, h * PH : h * PH + PH], ident[:])
            nc.vector.tensor_copy(expT[:, h, tt * 128 : tt * 128 + 128],
                                  pt_ps[:PH, :])
        # partial softmax denominators
    zloc = work.tile([128, 8], F32, tag="zloc")
    nc.vector.memset(zloc[:], 0.0)
    nc.vector.reduce_sum(zloc[:, 0:1], expS[:, 0, :], axis=mybir.AxisListType.X)
    nc.vector.reduce_sum(zloc[:, 4:5], expS[:, 1, :], axis=mybir.AxisListType.X)
    zpad = work.tile([128, 8], BF16, tag="zpad")
    nc.scalar.copy(zpad[:], zloc[:])
    # z (and zero pad) into columns 1024:1028 of rs_i: row (t p) = seg t*128+p
    nc.sync.dma_start(
        d["rs_iA"][:, 1024:1028].rearrange("(t p) c -> p t c", t=2),
        zpad[:].rearrange("p (t c) -> p t c", t=2))
    zero8 = work.tile([128, 8], BF16, tag="zero8")
    nc.vector.memset(zero8[:], 0.0)
    nc.sync.dma_start(
        d["rs_iB"][:, 1024:1028].rearrange("(t p) c -> p t c", t=2),
        zero8[:].rearrange("p (t c) -> p t c", t=2))

    psS_cm.__exit__(None, None, None)
    psT_cm.__exit__(None, None, None)
    psA_cm.__exit__(None, None, None)

    # ---------------- FC1 (contraction-sharded, out [256 segs, 1024] partial)
    # lhsT = pt seg-halves [125, 128]; rhs = fw1 it-chunk [125, 512]-halves.
    psF_cm = tc.tile_pool(name="psF", bufs=1, space="PSUM")
    ptp_cm = tc.tile_pool(name="ptp", bufs=4)
    psF = psF_cm.__enter__()
    ptp = ptp_cm.__enter__()
    r1ps = [psF.tile([128, 512], F32, name=f"r1ps_{m}", tag=f"r1_{m}")
            for m in range(4)]
    NIT = C * 2
    HIT = NIT // 2

    def fc1_evict(gen, rs_i):
        _dmaeng = [nc.sync, nc.scalar, nc.gpsimd, nc.sync]
        for m in range(4):
            s, f = m // 2, m % 2
            r1sb = big.tile([128, 512], BF16, tag=f"r1sb{gen}",
                            name=f"r1sb{gen}_{m}", bufs=2)
            if m % 2 == 0:
                nc.scalar.copy(r1sb[:], r1ps[m][:])
            else:
                nc.vector.tensor_copy(r1sb[:], r1ps[m][:])
            _dmaeng[m].dma_start(
                rs_i[s * 128 : (s + 1) * 128, f * 512 : (f + 1) * 512],
                r1sb[:])

    for ch in range(C):
        for h in range(2):
            it = ch * 2 + h
            gi = 0
            while not (fwtiles[gi][1] <= it < fwtiles[gi][1] + fwtiles[gi][2]):
                gi += 1
            fw = fwtiles[gi][0][:, it - fwtiles[gi][1], :]
            pt = ptp.tile([PH, 256], BF16, tag="pt", name=f"pt_{it}")
            nc.vector.tensor_mul(pt[:], xbv[:, ch, h, :], expT[:, h, :])
            for s in range(2):
                lhsT = pt[:, s * 128 : (s + 1) * 128]
                for f in range(2):
                    nc.tensor.matmul(
                        r1ps[s * 2 + f][:, :], lhsT,
                        fw[:, f * 512 : (f + 1) * 512],
                        start=(it in (0, HIT)), stop=(it in (HIT - 1, NIT - 1)))
            if it == HIT - 1:
                # first-half partials ship out mid-FC1 so RS#1 overlaps the
                # second half of the contraction.
                fc1_evict("A", d["rs_iA"])
                nc.gpsimd.collective_compute(
                    "ReduceScatter", mybir.AluOpType.add, replica_groups=RG,
                    ins=[d["rs_iA"][:]], outs=[d["rs_oA"][:]])
    fc1_evict("B", d["rs_iB"])
    nc.gpsimd.collective_compute(
        "ReduceScatter", mybir.AluOpType.add, replica_groups=RG,
        ins=[d["rs_iB"][:]], outs=[d["rs_oB"][:]])

    ptp_cm.__exit__(None, None, None)
    psF_cm.__exit__(None, None, None)
    fw1p_cm.__exit__(None, None, None)

    # ---------------- tail: this core owns segments [32c, 32c+32), complete.
    ps2_cm = tc.tile_pool(name="ps2", bufs=2, space="PSUM")
    ps2 = ps2_cm.__enter__()

    r1A = big.tile([SEGC, RSW], BF16, tag="r1A")
    nc.sync.dma_start(r1A[:], d["rs_oA"][:])
    r1B = big.tile([SEGC, RSW], BF16, tag="r1B")
    nc.sync.dma_start(r1B[:], d["rs_oB"][:])
    r1 = big.tile([SEGC, RSW], BF16, tag="r1")
    nc.vector.tensor_add(r1[:], r1A[:], r1B[:])
    zinv = work.tile([SEGC, 1], F32, tag="zinv")
    nc.scalar.mul(zinv[:], r1[:, 1024:1025], FW_SCALE)
    nc.vector.reciprocal(zinv[:], zinv[:])
    r1z = big.tile([SEGC, 1024], BF16, tag="r1z")
    nc.scalar.activation(r1z[:], r1[:, 0:1024], AF.Copy, scale=zinv[:])
    # transpose to 8 feature tiles [128, 32]; BN1 stats over local 32 segs
    r1T = big.tile([128, 8, SEGC], BF16, tag="r1T")
    stat8 = work.tile([128, 8, 6], F32, tag="stat8")
    for t in range(8):
        pt_ps = ps2.tile([128, SEGC], BF16, tag="tp", name=f"tp_{t}")
        nc.tensor.transpose(pt_ps[:], r1z[:, t * 128 : (t + 1) * 128],
                            identh[:])
        nc.scalar.copy(r1T[:, t, :], pt_ps[:])
        nc.vector.bn_stats(stat8[:, t, :], pt_ps[:])
    mv8 = work.tile([128, 8, 2], F32, tag="mv8")
    for t in range(8):
        nc.vector.bn_aggr(mv8[:, t, :], stat8[:, t : t + 1, :])
    ss8 = work.tile([128, 8, 2], F32, tag="ss8")
    nc.vector.tensor_mul(ss8[:, :, 1:2], mv8[:, :, 0:1], mv8[:, :, 0:1])
    nc.vector.tensor_add(ss8[:, :, 1:2], ss8[:, :, 1:2], mv8[:, :, 1:2])
    nc.scalar.mul(ss8[:, :, 0:1], mv8[:, :, 0:1], float(SEGC))
    nc.scalar.mul(ss8[:, :, 1:2], ss8[:, :, 1:2], float(SEGC))
    nc.gpsimd.dma_start(d["ag4_i"][:], ss8[:].rearrange("p t u -> p (t u)"))
    nc.gpsimd.collective_compute(
        "AllGather", mybir.AluOpType.bypass, replica_groups=RG,
        ins=[d["ag4_i"][:]], outs=[d["ag4_o"][:]])
    agg4 = work.tile([128, 8, 16], F32, tag="agg4")
    _age = [nc.sync, nc.scalar, nc.gpsimd]
    for c in range(8):
        _age[c % 3].dma_start(agg4[:, c, :],
                              d["ag4_o"][c * 128 : (c + 1) * 128, :])
    g8 = work.tile([128, 8, 2], F32, tag="g8")
    nc.vector.reduce_sum(g8[:], agg4[:].rearrange("p c w -> p w c"),
                         axis=mybir.AxisListType.X)
    # mean/rstd per feature ([128, 8] per-partition per-tile)
    epsf = work.tile([128, 1], F32, tag="epsf")
    nc.vector.memset(epsf[:], EPS_BN)
    nc.scalar.mul(g8[:, :, 0:1], g8[:, :, 0:1], 1.0 / B)
    nc.scalar.mul(g8[:, :, 1:2], g8[:, :, 1:2], 1.0 / B)
    m2t = work.tile([128, 8], F32, tag="m2t")
    nc.vector.tensor_mul(m2t[:], g8[:, :, 0:1], g8[:, :, 0:1])
    nc.vector.tensor_sub(g8[:, :, 1:2], g8[:, :, 1:2], m2t[:])
    nc.scalar.activation(g8[:, :, 1:2], g8[:, :, 1:2], AF.Sqrt, bias=epsf[:])
    nc.vector.reciprocal(g8[:, :, 1:2], g8[:, :, 1:2])
    fg1t = load("fg1t", [128, 8], pool=work)
    fbe1t = load("fbe1t", [128, 8], pool=work)
    sc1 = work.tile([128, 8], F32, tag="sc1")
    bi1 = work.tile([128, 8], F32, tag="bi1")
    nc.vector.tensor_mul(sc1[:], fg1t[:], g8[:, :, 1:2])
    nc.vector.tensor_mul(bi1[:], sc1[:], g8[:, :, 0:1])
    nc.vector.tensor_sub(bi1[:], fbe1t[:], bi1[:])
    r1Tr = big.tile([128, 8, SEGC], BF16, tag="r1Tr")
    for t in range(8):
        nc.scalar.activation(r1Tr[:, t, :], r1T[:, t, :], AF.Relu,
                             bias=bi1[:, t : t + 1], scale=sc1[:, t : t + 1])
    # FC2: out [32 segs, 256] complete (contraction over 1024 feats, local)
    ps_r2 = ps2.tile([SEGC, 256], F32, tag="r2", bufs=1)
    for t in range(8):
        nc.tensor.matmul(ps_r2[:], r1Tr[:, t, :], fw2sv[:, t, :],
                         start=(t == 0), stop=(t == 7))
    r2st = big.tile([SEGC, 512], BF16, tag="r2st")
    nc.scalar.copy(r2st[:, 0:256], ps_r2[:])
    nc.scalar.activation(r2st[:, 256:512], ps_r2[:], AF.Square)
    ps_s5 = ps2.tile([1, 512], F32, tag="s5", bufs=1)
    nc.tensor.matmul(ps_s5[:], ones32h[:], r2st[:], start=True, stop=True)
    s5 = work.tile([1, 512], F32, tag="s5sb")
    nc.scalar.copy(s5[:], ps_s5[:])
    nc.gpsimd.dma_start(d["ag5_i"][:], s5[:])
    nc.gpsimd.collective_compute(
        "AllGather", mybir.AluOpType.bypass, replica_groups=RG,
        ins=[d["ag5_i"][:]], outs=[d["ag5_o"][:]])
    agg5 = work.tile([8, 512], F32, tag="agg5")
    nc.sync.dma_start(agg5[:], d["ag5_o"][:])
    agg5h = work.tile([8, 512], BF16, tag="agg5h")
    nc.vector.tensor_copy(agg5h[:], agg5[:])
    ones8h = sing.tile([8, 1], BF16)
    nc.vector.memset(ones8h[:], 1.0)
    ps_g5 = ps2.tile([1, 512], F32, tag="g5", bufs=1)
    nc.tensor.matmul(ps_g5[:], ones8h[:], agg5h[:], start=True, stop=True)
    g5 = work.tile([1, 512], F32, tag="g5sb")
    nc.scalar.copy(g5[:], ps_g5[:])
    # scale/bias rows [1, 256] -> packed scb5 [1, 512] fp16 for PE broadcast
    eps1 = work.tile([1, 1], F32, tag="eps1")
    nc.vector.memset(eps1[:], EPS_BN)
    nc.scalar.mul(g5[:, 0:256], g5[:, 0:256], 1.0 / B)
    nc.scalar.mul(g5[:, 256:512], g5[:, 256:512], 1.0 / B)
    m2r = work.tile([1, 256], F32, tag="m2r")
    nc.vector.tensor_mul(m2r[:], g5[:, 0:256], g5[:, 0:256])
    nc.vector.tensor_sub(g5[:, 256:512], g5[:, 256:512], m2r[:])
    nc.scalar.activation(g5[:, 256:512], g5[:, 256:512], AF.Sqrt, bias=eps1[:])
    nc.vector.reciprocal(g5[:, 256:512], g5[:, 256:512])
    fg2r = load("fg2r", [1, 256], pool=work)
    fbe2r = load("fbe2r", [1, 256], pool=work)
    scb5 = work.tile([1, 512], BF16, tag="scb5")
    sc2f = work.tile([1, 256], F32, tag="sc2f")
    nc.vector.tensor_mul(sc2f[:], fg2r[:], g5[:, 256:512])
    nc.scalar.copy(scb5[:, 0:256], sc2f[:])
    bi2f = work.tile([1, 256], F32, tag="bi2f")
    nc.vector.tensor_mul(bi2f[:], sc2f[:], g5[:, 0:256])
    nc.vector.tensor_sub(bi2f[:], fbe2r[:], bi2f[:])
    nc.scalar.copy(scb5[:, 256:512], bi2f[:])
    ps_bc = ps2.tile([SEGC, 512], F32, tag="bc", bufs=1)
    nc.tensor.matmul(ps_bc[:], ones1x32h[:], scb5[:], start=True, stop=True)
    # apply BN2 + relu (per-column scale/bias via broadcast tiles)
    r2n = big.tile([SEGC, 256], BF16, tag="r2n")
    nc.vector.tensor_mul(r2n[:], r2st[:, 0:256], ps_bc[:, 0:256])
    nc.vector.tensor_add(r2n[:], r2n[:], ps_bc[:, 256:512])
    nc.vector.tensor_scalar_max(r2n[:], r2n[:], 0.0)
    # L2 normalize rows, write this core's [32, 256] slab
    nsq = work.tile([SEGC, 256], F32, tag="nsq")
    nc.scalar.activation(nsq[:], r2n[:], AF.Square)
    nrm = work.tile([SEGC, 1], F32, tag="nrm")
    nc.vector.reduce_sum(nrm[:], nsq[:], axis=mybir.AxisListType.X)
    nc.scalar.activation(nrm[:], nrm[:], AF.Sqrt)
    nc.vector.tensor_scalar_max(nrm[:], nrm[:], 1e-12)
    nc.vector.reciprocal(nrm[:], nrm[:])
    outf = big.tile([SEGC, 256], F32, tag="outf")
    nc.scalar.activation(outf[:], r2n[:], AF.Copy, scale=nrm[:])
    nc.sync.dma_start(d["out_final"][:], outf[:])

    ps2_cm.__exit__(None, None, None)
    work_cm.__exit__(None, None, None)
    big_cm.__exit__(None, None, None)
    sing_cm.__exit__(None, None, None)


# ------------------------------------------------------------------ host side
def _prep_core(x3, fw1, c):
    import ml_dtypes
    xs = x3[:, PL * c : PL * (c + 1), :]                       # [256,250,32]
    arr = np.ascontiguousarray(xs.transpose(2, 0, 1))          # [32,256,250]
    xA4 = arr.reshape(C, 4, QF).transpose(1, 0, 2).reshape(128, QF)
    xb = xs.reshape(B, 2, PH, C).transpose(2, 3, 1, 0)         # [125,32,2,256]
    xB = np.ascontiguousarray(xb).reshape(PH, C * 2 * B)
    fw = fw1.reshape(1024, P, C)[:, PL * c : PL * (c + 1), :]
    fw = fw.reshape(1024, 2, PH, C).transpose(2, 3, 1, 0)      # [125,32,2,1024]
    fw1t = np.ascontiguousarray(fw).reshape(PH, C * 2, 1024)
    bf = np.float16
    f8 = ml_dtypes.float8_e3m4
    return (np.ascontiguousarray(xA4).astype(bf), xB.astype(bf),
            (fw1t * 64.0).astype(f8))


def _qrep(v, rows):
    out = np.zeros((128, 1), np.float32)
    for a in range(4):
        out[32 * a : 32 * a + rows, 0] = v
    return out


def _wdiag(w):
    """w [out,in] -> block-diagonal lhsT [128, 128]: block a (32x32) holds
    w.T in its top-left corner."""
    t = np.zeros((128, 128), np.float32)
    wt = w.T  # [in, out]
    for a in range(4):
        t[32 * a : 32 * a + wt.shape[0], 32 * a : 32 * a + wt.shape[1]] = wt
    return t


def _w2k(w2):
    """Four column-shifted L2 lhsT variants: variant k maps quarter a's
    outputs to partition rows 32a+8k..+8, so four point-chunks stack into
    one psum tile."""
    out = np.zeros((128, 4, 128), np.float32)
    wt = w2.T  # [16, 8]
    for k in range(4):
        for a in range(4):
            out[32 * a : 32 * a + 16, k,
                32 * a + 8 * k : 32 * a + 8 * k + 8] = wt
    return out.reshape(128, 512)


def _w3k(w3):
    """L3 lhsT on the packed (a,k,c) row space: block (a,k) contracts rows
    32a+8k+c into the single score row 32a+8k."""
    t = np.zeros((128, 128), np.float32)
    wt = w3.T  # [8, 1]
    for a in range(4):
        for k in range(4):
            r = 32 * a + 8 * k
            t[r : r + 8, r : r + 1] = wt
    return t


def _rep8(v):
    """Per-partition vector on the packed (a,k,c) rows: row 32a+8k+c = v[c]."""
    out = np.zeros((128, 1), np.float32)
    for b in range(16):
        out[8 * b : 8 * b + 8, 0] = v
    return out


def kernel(**inputs):
    import ml_dtypes

    if "nc" not in _cache:
        _cache["nc"] = _build()
    nc = _cache["nc"]
    bf = np.float16

    g = {k: np.asarray(v, np.float32) for k, v in inputs.items()
         if k != "length"}
    x3 = g["x"].reshape(B, P, C)

    f1 = np.zeros((128, 16), np.float32)
    f2 = np.zeros((128, 8), np.float32)
    for a in range(4):
        f1[32 * a : 32 * a + 16, :] = np.eye(16, dtype=np.float32)
    for b in range(16):
        f2[8 * b : 8 * b + 8, :] = np.eye(8, dtype=np.float32)
    f8_16 = np.zeros((128, 16), np.float32)
    f8_8 = np.zeros((64, 8), np.float32)
    for k in range(8):
        f8_16[16 * k : 16 * k + 16, :] = np.eye(16, dtype=np.float32)
        f8_8[8 * k : 8 * k + 8, :] = np.eye(8, dtype=np.float32)

    shared = {
        "w1D": _wdiag(g["w1"]).astype(bf),
        "w2K": _w2k(g["w2"]).astype(bf),
        "w3D": _w3k(g["w3"]).astype(bf),
        "g1q": _qrep(g["g1"], 16), "be1q": _qrep(g["be1"], 16),
        "g2q": _rep8(g["g2"]), "be2q": _rep8(g["be2"]),
        "g3s": g["g3"].reshape(1, 1), "be3s": g["be3"].reshape(1, 1),
        "f1": f1, "ft1": np.ascontiguousarray(f1.T),
        "f2": f2, "ft2": np.ascontiguousarray(f2.T),
        "f8_16": f8_16, "f8_8": f8_8,
        "fw2s": np.ascontiguousarray(
            g["fw2"].reshape(256, 8, 128).transpose(2, 1, 0).reshape(
                128, 8 * 256)).astype(bf),
        "fg1t": np.ascontiguousarray(g["fg1"].reshape(8, 128).T),
        "fbe1t": np.ascontiguousarray(g["fbe1"].reshape(8, 128).T),
        "fg2r": g["fg2"].reshape(1, 256),
        "fbe2r": g["fbe2"].reshape(1, 256),
    }

    in_maps = []
    for c in range(NCORES):
        xA4, xB, fw1t = _prep_core(x3, g["fw1"], c)
        m = dict(shared)
        m["xA4"] = xA4
        m["xB"] = xB
        m["fw1t"] = fw1t
        in_maps.append(m)

    from concourse.bass_utils import run_bass_kernel_spmd

    res = run_bass_kernel_spmd(nc, in_maps, core_ids=list(range(NCORES)),
                               trace=bool(_cache.get("trace")))
    _cache["last_result"] = res
    return np.concatenate(
        [np.asarray(res.results[c]["out_final"], np.float32)
         for c in range(NCORES)], axis=0)


if __name__ == "__main__":
    nc = _build()
    print("build ok; instructions:",
          sum(len(bb.instructions) for bb in nc.main_func.blocks))


# revision 35
# speedup vs baseline: 1.2819x; 1.0335x over previous
"""Trainium2 Bass kernel for nn_FCGF_point_att3_sft_7000 (8 NeuronCores).

Model: pointwise attention MLP (32->16->8->1, BN+relu, BN stats over the full
512000-point batch), per-segment softmax over 2000 points, attention-weighted
pooling to [256, 64000], FC head 64000->1024->256 (BN+relu, stats over the
256-segment batch), final L2 row-normalize.

Sharding: points-within-segment. Core c owns points p in [250c, 250(c+1)) of
every segment. Stage A is data-parallel over points with AllGather'd BN stats;
fc1 is contraction-sharded (each core owns 8000 of the 64000 inputs and the
matching fw1 rows) with the output in [segs, feats] orientation so a
ReduceScatter over segments hands each core 32 complete segments; the softmax
denominators ride the same collective as an extra column. The whole tail
(BN1, fc2, BN2, L2-normalize) then runs locally per core on its 32 segments,
with two tiny AllGathers for the cross-segment BN statistics; each core emits
its own [32, 256] slab of the output.

Stage-A layout: "quartered" A-orientation. x.T is [128, 16000] with the
channels of free-quarter a on partitions [32a, 32a+32). Matmuls use
tile_position=(32a, 32a) so outputs land on partitions 32a+ch and every
eviction / BN / softmax op runs 128 partitions wide. Weight tiles are
zero-padded to M=32 so all PSUM rows are defined.

Training-mode BN is shift-invariant => conv/linear biases (b1,b2,b3,fb1,fb2)
drop out exactly; they are accepted and ignored.
"""

import sys

sys.path.insert(0, "/opt/trn_rl_repo")

import numpy as np

import concourse.bass as bass
import concourse.tile as tile
from concourse import mybir
from concourse.masks import make_identity

B = 256
P = 2000
C = 32
NCORES = 8
PL = P // NCORES           # 250
PH = PL // 2               # 125
NPTS = B * PL              # 64000 points per core
QF = NPTS // 4             # 16000 per quarter
NCH = 500                  # stage-A free chunk
NCHUNK = QF // NCH         # 32
SEGC = B // NCORES         # 32 segments per core after the ReduceScatter
RSW = 1028                 # rs payload width: 1024 feats + z + 3 pad
EPS_BN = 1e-5
F32 = mybir.dt.float32
BF16 = mybir.dt.float16  # fp16: same speed as bf16, 8x lower rounding noise
F8 = mybir.dt.float8e3   # e3m4: fc1 weight stream at half the HBM bytes
FW_SCALE = 64.0          # fw1*64 fits e3m4 range; undone in the z-normalize
RG = [list(range(NCORES))]
AF = mybir.ActivationFunctionType

_cache = {}


# ------------------------------------------------------------------ walrus fix
def _install_walrus_patch():
    """This container's walrus accepts only ONE semaphore wait per instruction.
    Spread Tile's end-of-kernel drain waits across single-wait nops, and split
    any instruction carrying >1 waits onto same-engine carrier nops."""
    if _cache.get("patched"):
        return
    from concourse.vector_clock import ScopedClock, VectorClock

    counter = [0]

    def split_waits(nc):
        for bb in nc.main_func.blocks:
            out = []
            changed = False
            for ins in bb.instructions:
                si = ins.sync_info
                waits = list(si.on_wait) if si and si.on_wait else []
                if len(waits) > 1:
                    changed = True
                    for w in waits[:-1]:
                        counter[0] += 1
                        out.append(mybir.InstNoOp(
                            name=f"I-wsplit-{counter[0]}",
                            engine=ins.engine, ins=[], outs=[],
                            sync_info=mybir.SyncInfo(on_wait=[w], on_update=[]),
                            bass_nofuse=True))
                    si.on_wait = waits[-1:]
                out.append(ins)
            if changed:
                try:
                    bb.instructions = out
                except Exception:
                    bb.instructions.clear()
                    for x in out:
                        bb.instructions.append(x)

    def _patched(self, tick_clock, wait_clock):
        nc = self.nc
        gc = tick_clock.global_clock
        n = len(gc)
        for i in range(n):
            if gc[i] > 0:
                vec = [0] * n
                vec[i] = gc[i]
                nop = nc.sync.nop(nofuse=True, hint=f"drain_wait_p{i}")
                wait_clock.add_sem_waits(
                    nop.ins, ScopedClock({None: VectorClock(vec)}))
        nc.sync.drain()
        nc.all_engine_barrier()
        assert self.sems is not None
        popped = nc._tile_sem_poison_stack.pop()
        assert popped is self._sem_poison
        nc.clear_and_free_semaphores(list(self.sems.allocated().values()))
        nc.all_engine_barrier()
        split_waits(nc)

    tile.TileContext._drain_and_barrier = _patched
    _cache["patched"] = True


# ------------------------------------------------------------------ bass build
def _build():
    _install_walrus_patch()
    nc = bass.Bass()

    def ein(name, shape, dt):
        return nc.dram_tensor(name, shape, dt, kind="ExternalInput")

    d = {}
    d["xA4"] = ein("xA4", [128, QF], BF16)
    d["xB"] = ein("xB", [PH, C * 2 * B], BF16)
    d["w1D"] = ein("w1D", [128, 128], BF16)
    d["w2K"] = ein("w2K", [128, 4 * 128], BF16)
    d["w3D"] = ein("w3D", [128, 128], BF16)
    for n in ("g1q", "be1q", "g2q", "be2q"):
        d[n] = ein(n, [128, 1], F32)
    d["g3s"] = ein("g3s", [1, 1], F32)
    d["be3s"] = ein("be3s", [1, 1], F32)
    d["f1"] = ein("f1", [128, 16], F32)
    d["ft1"] = ein("ft1", [16, 128], F32)
    d["f2"] = ein("f2", [128, 8], F32)
    d["ft2"] = ein("ft2", [8, 128], F32)
    d["f8_16"] = ein("f8_16", [128, 16], F32)
    d["f8_8"] = ein("f8_8", [64, 8], F32)
    d["fw1t"] = ein("fw1t", [PH, C * 2, 1024], F8)
    d["fw2s"] = ein("fw2s", [128, 8 * 256], BF16)
    d["fg1t"] = ein("fg1t", [128, 8], F32)
    d["fbe1t"] = ein("fbe1t", [128, 8], F32)
    d["fg2r"] = ein("fg2r", [1, 256], F32)
    d["fbe2r"] = ein("fbe2r", [1, 256], F32)
    d["out_final"] = nc.dram_tensor("out_final", [SEGC, 256], F32,
                                    kind="ExternalOutput")
    # collective bounce buffers
    d["warm_i"] = nc.dram_tensor("warm_i", [16, 4], F32)
    d["warm_o"] = nc.dram_tensor("warm_o", [16, 4], F32)
    d["st1_i"] = nc.dram_tensor("st1_i", [16, 2], F32)
    d["st1_o"] = nc.dram_tensor("st1_o", [128, 2], F32)
    d["st2_i"] = nc.dram_tensor("st2_i", [8, 2], F32)
    d["st2_o"] = nc.dram_tensor("st2_o", [64, 2], F32)
    d["st3_i"] = nc.dram_tensor("st3_i", [1, 2], F32)
    d["st3_o"] = nc.dram_tensor("st3_o", [8, 2], F32)
    d["rs_iA"] = nc.dram_tensor("rs_iA", [B, RSW], BF16)
    d["rs_oA"] = nc.dram_tensor("rs_oA", [SEGC, RSW], BF16)
    d["rs_iB"] = nc.dram_tensor("rs_iB", [B, RSW], BF16)
    d["rs_oB"] = nc.dram_tensor("rs_oB", [SEGC, RSW], BF16)
    d["ag4_i"] = nc.dram_tensor("ag4_i", [128, 16], F32)
    d["ag4_o"] = nc.dram_tensor("ag4_o", [128 * NCORES, 16], F32)
    d["ag5_i"] = nc.dram_tensor("ag5_i", [1, 512], F32)
    d["ag5_o"] = nc.dram_tensor("ag5_o", [NCORES, 512], F32)

    with tile.TileContext(nc) as tc:
        _body(nc, tc, d)
    return nc


def _mkstats(nc, pool, mv, count, name):
    """mv [p,2]=(mean,var) -> (sum,sumsq) [p,2]."""
    p = mv.shape[0]
    ss = pool.tile([p, 2], F32, tag=f"ss_{name}")
    nc.vector.tensor_mul(ss[:, 1:2], mv[:, 0:1], mv[:, 0:1])
    nc.vector.tensor_add(ss[:, 1:2], ss[:, 1:2], mv[:, 1:2])
    nc.scalar.mul(ss[:, 0:1], mv[:, 0:1], float(count))
    nc.scalar.mul(ss[:, 1:2], ss[:, 1:2], float(count))
    return ss


def _mv_from_ss(nc, pool, ss, count, name):
    """(sum,sumsq) [p,2] over count -> (mean, rstd) [p,2]."""
    p = ss.shape[0]
    mr = pool.tile([p, 2], F32, tag=f"mr_{name}")
    epst = pool.tile([p, 1], F32, tag=f"eps_{name}")
    nc.vector.memset(epst[:], EPS_BN)
    nc.scalar.mul(mr[:, 0:1], ss[:, 0:1], 1.0 / count)
    nc.scalar.mul(mr[:, 1:2], ss[:, 1:2], 1.0 / count)
    m2 = pool.tile([p, 1], F32, tag=f"m2_{name}")
    nc.vector.tensor_mul(m2[:], mr[:, 0:1], mr[:, 0:1])
    nc.vector.tensor_sub(mr[:, 1:2], mr[:, 1:2], m2[:])
    nc.scalar.activation(mr[:, 1:2], mr[:, 1:2], AF.Sqrt, bias=epst[:])
    nc.vector.reciprocal(mr[:, 1:2], mr[:, 1:2])
    return mr


def _scale_bias(nc, pool, mrq, g, be, name):
    """scale = g*rstd ; bias = be - scale*mean  (all [p,1] per-partition)."""
    p = mrq.shape[0]
    sc = pool.tile([p, 1], F32, tag=f"sc_{name}")
    bi = pool.tile([p, 1], F32, tag=f"bi_{name}")
    nc.vector.tensor_mul(sc[:], g[:], mrq[:, 1:2])
    nc.vector.tensor_mul(bi[:], sc[:], mrq[:, 0:1])
    nc.vector.tensor_sub(bi[:], be[:], bi[:])
    return sc, bi


def _body(nc, tc, d):
    # collective warmup first. warm_i is never written (contents irrelevant),
    # so the op has NO dependencies and the ~55us ncfw startup begins at t=0,
    # overlapping the whole front of the kernel.
    nc.gpsimd.collective_compute(
        "AllReduce", mybir.AluOpType.add, replica_groups=RG,
        ins=[d["warm_i"][:]], outs=[d["warm_o"][:]])
    sing_cm = tc.tile_pool(name="sing", bufs=1)
    big_cm = tc.tile_pool(name="big", bufs=1)
    work_cm = tc.tile_pool(name="work", bufs=1)
    psA_cm = tc.tile_pool(name="psA", bufs=4, space="PSUM")
    psT_cm = tc.tile_pool(name="psT", bufs=2, space="PSUM")
    psS_cm = tc.tile_pool(name="psS", bufs=2, space="PSUM")
    sing = sing_cm.__enter__(); big = big_cm.__enter__()
    work = work_cm.__enter__()
    fw1p_cm = tc.tile_pool(name="fw1p", bufs=6)
    fw1p = fw1p_cm.__enter__()
    psA = psA_cm.__enter__(); psT = psT_cm.__enter__()
    psS = psS_cm.__enter__()

    # ---------------- constants
    def load(name, shape, dt=F32, pool=sing):
        t = pool.tile(shape, dt, tag=name)
        nc.sync.dma_start(t[:], d[name][:])
        return t

    w1D = load("w1D", [128, 128], BF16)
    w2Kt = load("w2K", [128, 4 * 128], BF16)
    w3D = load("w3D", [128, 128], BF16)
    f1s = load("f1", [128, 16])
    ft1s = load("ft1", [16, 128])
    f2s = load("f2", [128, 8])
    ft2s = load("ft2", [8, 128])
    f8_16s = load("f8_16", [128, 16])
    f8_8s = load("f8_8", [64, 8])
    g1 = load("g1q", [128, 1]); be1 = load("be1q", [128, 1])
    g2 = load("g2q", [128, 1]); be2 = load("be2q", [128, 1])
    g3 = load("g3s", [1, 1]); be3 = load("be3s", [1, 1])
    ones128 = sing.tile([128, 1], F32)
    nc.vector.memset(ones128[:], 1.0)
    ones8 = sing.tile([8, 1], F32)
    nc.vector.memset(ones8[:], 1.0)
    ones1x = sing.tile([1, 128], F32)
    nc.vector.memset(ones1x[:], 1.0)
    ones32h = sing.tile([32, 1], BF16)
    nc.vector.memset(ones32h[:], 1.0)
    ones1x32h = sing.tile([1, 32], BF16)
    nc.vector.memset(ones1x32h[:], 1.0)
    ident = sing.tile([128, 128], F32)
    make_identity(nc, ident[:])
    identh = sing.tile([SEGC, SEGC], BF16)
    make_identity(nc, identh[:])

    # ---------------- big loads
    xa = big.tile([128, QF], BF16, tag="slotA")       # slot A: xa -> y2 -> y3q
    nc.sync.dma_start(xa[:], d["xA4"][:])
    xb = big.tile([PH, C * 2 * B], BF16, tag="xb")
    nc.sync.dma_start(xb[:], d["xB"][:])
    xbv = xb[:].rearrange("p (c h s) -> p c h s", c=C, h=2, s=B)

    # fc1 weight prefetch: pool entered at the top so its slots exist from
    # t=0 and the 16.4MB stream overlaps all of stage A. 2-engine rotation.
    FW_CHUNKS = [8] * 8
    fwtiles = []
    _dge = [nc.sync, nc.scalar]
    _off = 0
    for gblk, nits in enumerate(FW_CHUNKS):
        fwt = fw1p.tile([PH, 8, 1024], F8, tag="fw", name=f"fw_{gblk}")
        _dge[gblk % 2].dma_start(fwt[:, :nits, :],
                                 d["fw1t"][:, _off : _off + nits, :])
        fwtiles.append((fwt, _off, nits))
        _off += nits
    fw2s = sing.tile([128, 8 * 256], BF16, tag="fw2s")
    nc.sync.dma_start(fw2s[:], d["fw2s"][:])
    fw2sv = fw2s[:].rearrange("p (t o) -> p t o", t=8)

    def layer_mms(ps, wD, krows, rhs_src, sl):
        nc.tensor.matmul(ps[:], wD[:], rhs_src[:, sl], start=True, stop=True)

    def stage_layer(mm_emit, nchunks, fold, foldT, f8fold, st_i, st_o,
                    gq, beq, count_local, name, out_tag, wnext=None):
        """Single-pass layer: matmuls -> evict y fp16 (+bn_stats from PSUM),
        fold+AllGather stats. The BN scale is folded into the next layer's
        weights (gamma>0), so the relu pass is a per-partition bias-shift
        split across scalar/vector."""
        y = big.tile([128, nchunks * NCH], BF16, tag=out_tag,
                     name=f"y_{name}")
        stat = work.tile([128, nchunks, 6], F32, tag=f"stat_{name}")
        for j in range(nchunks):
            ps = psA.tile([128, NCH], F32, tag="psA", name=f"ps_{name}_{j}")
            mm_emit(ps, j)
            nc.scalar.copy(y[:, j * NCH : (j + 1) * NCH], ps[:])
            nc.vector.bn_stats(stat[:, j, :], y[:, j * NCH : (j + 1) * NCH])
        mv = work.tile([128, 2], F32, tag=f"mv_{name}")
        nc.vector.bn_aggr(mv[:], stat[:])
        ss = _mkstats(nc, work, mv, count_local, name)
        nfold = fold.shape[1]
        psf = psS.tile([128, 2], F32, tag="small", name=f"psf_{name}")
        nc.tensor.matmul(psf[:nfold, :], fold[:], ss[:], start=True, stop=True)
        sbf = work.tile([nfold, 2], F32, tag=f"sbf_{name}")
        nc.scalar.copy(sbf[:], psf[:nfold, :])
        nc.gpsimd.dma_start(st_i[:], sbf[:])
        nc.gpsimd.collective_compute(
            "AllGather", mybir.AluOpType.bypass, replica_groups=RG,
            ins=[st_i[:]], outs=[st_o[:]])
        agg = work.tile([nfold * NCORES, 2], F32, tag=f"agg_{name}")
        nc.gpsimd.dma_start(agg[:], st_o[:])
        psg = psS.tile([128, 2], F32, tag="small", name=f"psg_{name}")
        nc.tensor.matmul(psg[:nfold, :], f8fold[:], agg[:], start=True,
                         stop=True)
        ssg = work.tile([nfold, 2], F32, tag=f"ssg_{name}")
        nc.scalar.copy(ssg[:], psg[:nfold, :])
        mr = _mv_from_ss(nc, work, ssg, B * P, name)
        psb = psS.tile([128, 2], F32, tag="small", name=f"psb_{name}")
        nc.tensor.matmul(psb[:], foldT[:], mr[:], start=True, stop=True)
        mrq = work.tile([128, 2], F32, tag=f"mrq_{name}")
        nc.scalar.copy(mrq[:], psb[:])
        sc, bi = _scale_bias(nc, work, mrq, gq, beq, name)
        # fold the BN scale into the next layer's weights (gamma>0); the relu
        # pass becomes a bias-shift, split across scalar/vector/gpsimd. Clamp
        # sc away from 0 first: padded partition slots have gamma=0 and the
        # bare reciprocal would make bip = 0*inf = NaN there.
        isc = work.tile([128, 1], F32, tag=f"isc_{name}")
        nc.vector.tensor_scalar_max(isc[:], sc[:], 1e-30)
        nc.vector.reciprocal(isc[:], isc[:])
        bip = work.tile([128, 1], F32, tag=f"bip_{name}")
        nc.vector.tensor_mul(bip[:], bi[:], isc[:])
        wnf = None
        if wnext is not None:
            wnf = sing.tile(list(wnext.shape), BF16, tag=f"wnf_{name}")
            nc.vector.tensor_scalar_mul(wnf[:], wnext[:], sc[:])
        for j in range(nchunks):
            sl = slice(j * NCH, (j + 1) * NCH)
            if j % 3 == 2:
                nc.scalar.activation(y[:, sl], y[:, sl], AF.Relu, bias=bip[:])
            else:
                nc.vector.tensor_scalar(y[:, sl], y[:, sl], bip[:], 0.0,
                                        mybir.AluOpType.add,
                                        mybir.AluOpType.max)
        return y, wnf

    # ---------------- stage A layers 1 & 2
    def l1_mm(ps, j):
        nc.tensor.matmul(ps[:], w1D[:], xa[:, j * NCH : (j + 1) * NCH],
                         start=True, stop=True)

    h1, w2f = stage_layer(l1_mm, NCHUNK, f1s, ft1s, f8_16s,
                          d["st1_i"], d["st1_o"], g1, be1, QF, "l1", "slotB",
                          wnext=w2Kt)
    w2fv = w2f[:].rearrange("p (k m) -> p k m", k=4)

    def l2_mm(ps, t):
        # 4 column-shifted weight variants stack 4 point-chunks into the
        # 4x(4x8) partition rows of one psum tile (accumulation unions them).
        # k-strided chunk assignment keeps each score row's segments
        # contiguous for the scoreS extraction DMA.
        for k in range(4):
            j = t + 8 * k
            nc.tensor.matmul(ps[:], w2fv[:, k, :],
                             h1[:, j * NCH : (j + 1) * NCH],
                             start=(k == 0), stop=(k == 3))

    h2, w3f = stage_layer(l2_mm, NCHUNK // 4, f2s, ft2s, f8_8s,
                          d["st2_i"], d["st2_o"], g2, be2, QF // 4, "l2",
                          "y2s", wnext=w3D)

    # ---------------- stage A layer 3: scores straight from PSUM into
    # scoreS [128 segs, 2, 250] via per-chunk repack DMAs (rows {32a} real;
    # chunk j of quarter a covers segments 64a+2j..+1)
    y3q = big.tile([128, QF // 4], BF16, tag="y3s", name="y3q")
    for t in range(NCHUNK // 4):
        ps = psA.tile([128, NCH], F32, tag="psA", name=f"ps_l3_{t}")
        nc.tensor.matmul(ps[:], w3f[:], h2[:, t * NCH : (t + 1) * NCH],
                         start=True, stop=True)
        if t % 2 == 0:
            nc.scalar.copy(y3q[:, t * NCH : (t + 1) * NCH], ps[:])
        else:
            nc.vector.tensor_copy(y3q[:, t * NCH : (t + 1) * NCH], ps[:])
    scoreS = big.tile([128, 2, PL], BF16, tag="scoreS")
    _sse = [nc.sync, nc.scalar, nc.gpsimd, nc.sync]
    for a in range(4):
        for k in range(4):
            _sse[(4 * a + k) % 3].dma_start(
                scoreS[64 * (a % 2) + 16 * k : 64 * (a % 2) + 16 * k + 16,
                       a // 2, :],
                y3q[32 * a + 8 * k : 32 * a + 8 * k + 1, :])
    # BN3 stats over all segments/points (all partitions real)
    stat3 = work.tile([128, 2, 6], F32, tag="stat3")
    nc.vector.bn_stats(stat3[:, 0, :], scoreS[:, 0, :])
    nc.vector.bn_stats(stat3[:, 1, :], scoreS[:, 1, :])
    mv3 = work.tile([128, 2], F32, tag="mv3")
    nc.vector.bn_aggr(mv3[:], stat3[:])
    ss3 = _mkstats(nc, work, mv3, 2 * PL, "l3")
    psf3 = psS.tile([128, 2], F32, tag="small", name="psf3")
    nc.tensor.matmul(psf3[:1, :], ones128[:], ss3[:], start=True, stop=True)
    sbf3 = work.tile([1, 2], F32, tag="sbf3")
    nc.scalar.copy(sbf3[:], psf3[:1, :])
    nc.gpsimd.dma_start(d["st3_i"][:], sbf3[:])
    nc.gpsimd.collective_compute(
        "AllGather", mybir.AluOpType.bypass, replica_groups=RG,
        ins=[d["st3_i"][:]], outs=[d["st3_o"][:]])
    agg3 = work.tile([8, 2], F32, tag="agg3")
    nc.gpsimd.dma_start(agg3[:], d["st3_o"][:])
    psg3 = psS.tile([128, 2], F32, tag="small", name="psg3")
    nc.tensor.matmul(psg3[:1, :], ones8[:], agg3[:], start=True, stop=True)
    ssg3 = work.tile([1, 2], F32, tag="ssg3")
    nc.scalar.copy(ssg3[:], psg3[:1, :])
    mr3 = _mv_from_ss(nc, work, ssg3, B * P, "l3")
    scb1 = work.tile([1, 2], F32, tag="scb1")
    nc.vector.tensor_mul(scb1[:, 0:1], g3[:], mr3[:, 1:2])
    nc.vector.tensor_mul(scb1[:, 1:2], scb1[:, 0:1], mr3[:, 0:1])
    nc.vector.tensor_sub(scb1[:, 1:2], be3[:], scb1[:, 1:2])
    psb3 = psS.tile([128, 2], F32, tag="small", name="psb3")
    nc.tensor.matmul(psb3[:], ones1x[:], scb1[:], start=True, stop=True)
    scb = work.tile([128, 2], F32, tag="scb")
    nc.scalar.copy(scb[:], psb3[:])
    # relu(BN3) in place, then exp
    expS = big.tile([128, 2, PL], F32, tag="expS")
    expT = big.tile([PH, 2, 256], BF16, tag="expT")
    for tt in range(2):
        nc.scalar.activation(scoreS[:, tt, :], scoreS[:, tt, :], AF.Relu,
                             bias=scb[:, 1:2], scale=scb[:, 0:1])
        nc.scalar.activation(expS[:, tt, :], scoreS[:, tt, :], AF.Exp)
        for h in range(2):
            pt_ps = psT.tile([128, 128], F32, tag="psT")
            nc.tensor.transpose(pt_ps[:PH, :],
                                expS[:, tt, h * PH : h * PH + PH], ident[:])
            nc.vector.tensor_copy(expT[:, h, tt * 128 : tt * 128 + 128],
                                  pt_ps[:PH, :])
        # partial softmax denominators
    zloc = work.tile([128, 8], F32, tag="zloc")
    nc.vector.memset(zloc[:], 0.0)
    nc.vector.reduce_sum(zloc[:, 0:1], expS[:, 0, :], axis=mybir.AxisListType.X)
    nc.vector.reduce_sum(zloc[:, 4:5], expS[:, 1, :], axis=mybir.AxisListType.X)
    zpad = work.tile([128, 8], BF16, tag="zpad")
    nc.scalar.copy(zpad[:], zloc[:])
    # z (and zero pad) into columns 1024:1028 of rs_i: row (t p) = seg t*128+p
    nc.sync.dma_start(
        d["rs_iA"][:, 1024:1028].rearrange("(t p) c -> p t c", t=2),
        zpad[:].rearrange("p (t c) -> p t c", t=2))
    zero8 = work.tile([128, 8], BF16, tag="zero8")
    nc.vector.memset(zero8[:], 0.0)
    nc.sync.dma_start(
        d["rs_iB"][:, 1024:1028].rearrange("(t p) c -> p t c", t=2),
        zero8[:].rearrange("p (t c) -> p t c", t=2))

    psS_cm.__exit__(None, None, None)
    psT_cm.__exit__(None, None, None)
    psA_cm.__exit__(None, None, None)

    # ---------------- FC1 (contraction-sharded, out [256 segs, 1024] partial)
    # lhsT = pt seg-halves [125, 128]; rhs = fw1 it-chunk [125, 512]-halves.
    psF_cm = tc.tile_pool(name="psF", bufs=1, space="PSUM")
    ptp_cm = tc.tile_pool(name="ptp", bufs=6)
    psF = psF_cm.__enter__()
    ptp = ptp_cm.__enter__()
    r1ps = [psF.tile([128, 512], F32, name=f"r1ps_{m}", tag=f"r1_{m}")
            for m in range(4)]
    NIT = C * 2
    HIT = NIT // 2

    def fc1_evict(gen, rs_i):
        _dmaeng = [nc.sync, nc.scalar, nc.gpsimd, nc.sync]
        for m in range(4):
            s, f = m // 2, m % 2
            r1sb = big.tile([128, 512], BF16, tag=f"r1sb{gen}",
                            name=f"r1sb{gen}_{m}", bufs=2)
            if m % 2 == 0:
                nc.scalar.copy(r1sb[:], r1ps[m][:])
            else:
                nc.vector.tensor_copy(r1sb[:], r1ps[m][:])
            _dmaeng[m].dma_start(
                rs_i[s * 128 : (s + 1) * 128, f * 512 : (f + 1) * 512],
                r1sb[:])

    for ch in range(C):
        for h in range(2):
            it = ch * 2 + h
            gi = 0
            while not (fwtiles[gi][1] <= it < fwtiles[gi][1] + fwtiles[gi][2]):
                gi += 1
            fw = fwtiles[gi][0][:, it - fwtiles[gi][1], :]
            pt = ptp.tile([PH, 256], BF16, tag="pt", name=f"pt_{it}")
            nc.vector.tensor_mul(pt[:], xbv[:, ch, h, :], expT[:, h, :])
            for s in range(2):
                lhsT = pt[:, s * 128 : (s + 1) * 128]
                for f in range(2):
                    nc.tensor.matmul(
                        r1ps[s * 2 + f][:, :], lhsT,
                        fw[:, f * 512 : (f + 1) * 512],
                        start=(it in (0, HIT)), stop=(it in (HIT - 1, NIT - 1)))
            if it == HIT - 1:
                # first-half partials ship out mid-FC1 so RS#1 overlaps the
                # second half of the contraction.
                fc1_evict("A", d["rs_iA"])
                nc.gpsimd.collective_compute(
                    "ReduceScatter", mybir.AluOpType.add, replica_groups=RG,
                    ins=[d["rs_iA"][:]], outs=[d["rs_oA"][:]])
    fc1_evict("B", d["rs_iB"])
    nc.gpsimd.collective_compute(
        "ReduceScatter", mybir.AluOpType.add, replica_groups=RG,
        ins=[d["rs_iB"][:]], outs=[d["rs_oB"][:]])

    ptp_cm.__exit__(None, None, None)
    psF_cm.__exit__(None, None, None)
    fw1p_cm.__exit__(None, None, None)

    # ---------------- tail: this core owns segments [32c, 32c+32), complete.
    ps2_cm = tc.tile_pool(name="ps2", bufs=2, space="PSUM")
    ps2 = ps2_cm.__enter__()

    r1A = big.tile([SEGC, RSW], BF16, tag="r1A")
    nc.sync.dma_start(r1A[:], d["rs_oA"][:])
    r1B = big.tile([SEGC, RSW], BF16, tag="r1B")
    nc.sync.dma_start(r1B[:], d["rs_oB"][:])
    r1 = big.tile([SEGC, RSW], BF16, tag="r1")
    nc.vector.tensor_add(r1[:], r1A[:], r1B[:])
    zinv = work.tile([SEGC, 1], F32, tag="zinv")
    nc.scalar.mul(zinv[:], r1[:, 1024:1025], FW_SCALE)
    nc.vector.reciprocal(zinv[:], zinv[:])
    r1z = big.tile([SEGC, 1024], BF16, tag="r1z")
    nc.scalar.activation(r1z[:], r1[:, 0:1024], AF.Copy, scale=zinv[:])
    # transpose to 8 feature tiles [128, 32]; BN1 stats over local 32 segs
    r1T = big.tile([128, 8, SEGC], BF16, tag="r1T")
    stat8 = work.tile([128, 8, 6], F32, tag="stat8")
    for t in range(8):
        pt_ps = ps2.tile([128, SEGC], BF16, tag="tp", name=f"tp_{t}")
        nc.tensor.transpose(pt_ps[:], r1z[:, t * 128 : (t + 1) * 128],
                            identh[:])
        nc.scalar.copy(r1T[:, t, :], pt_ps[:])
        nc.vector.bn_stats(stat8[:, t, :], pt_ps[:])
    mv8 = work.tile([128, 8, 2], F32, tag="mv8")
    for t in range(8):
        nc.vector.bn_aggr(mv8[:, t, :], stat8[:, t : t + 1, :])
    ss8 = work.tile([128, 8, 2], F32, tag="ss8")
    nc.vector.tensor_mul(ss8[:, :, 1:2], mv8[:, :, 0:1], mv8[:, :, 0:1])
    nc.vector.tensor_add(ss8[:, :, 1:2], ss8[:, :, 1:2], mv8[:, :, 1:2])
    nc.scalar.mul(ss8[:, :, 0:1], mv8[:, :, 0:1], float(SEGC))
    nc.scalar.mul(ss8[:, :, 1:2], ss8[:, :, 1:2], float(SEGC))
    nc.gpsimd.dma_start(d["ag4_i"][:], ss8[:].rearrange("p t u -> p (t u)"))
    nc.gpsimd.collective_compute(
        "AllGather", mybir.AluOpType.bypass, replica_groups=RG,
        ins=[d["ag4_i"][:]], outs=[d["ag4_o"][:]])
    agg4 = work.tile([128, 8, 16], F32, tag="agg4")
    _age = [nc.sync, nc.scalar, nc.gpsimd]
    for c in range(8):
        _age[c % 3].dma_start(agg4[:, c, :],
                              d["ag4_o"][c * 128 : (c + 1) * 128, :])
    g8 = work.tile([128, 8, 2], F32, tag="g8")
    nc.vector.reduce_sum(g8[:], agg4[:].rearrange("p c w -> p w c"),
                         axis=mybir.AxisListType.X)
    # mean/rstd per feature ([128, 8] per-partition per-tile)
    epsf = work.tile([128, 1], F32, tag="epsf")
    nc.vector.memset(epsf[:], EPS_BN)
    nc.scalar.mul(g8[:, :, 0:1], g8[:, :, 0:1], 1.0 / B)
    nc.scalar.mul(g8[:, :, 1:2], g8[:, :, 1:2], 1.0 / B)
    m2t = work.tile([128, 8], F32, tag="m2t")
    nc.vector.tensor_mul(m2t[:], g8[:, :, 0:1], g8[:, :, 0:1])
    nc.vector.tensor_sub(g8[:, :, 1:2], g8[:, :, 1:2], m2t[:])
    nc.scalar.activation(g8[:, :, 1:2], g8[:, :, 1:2], AF.Sqrt, bias=epsf[:])
    nc.vector.reciprocal(g8[:, :, 1:2], g8[:, :, 1:2])
    fg1t = load("fg1t", [128, 8], pool=work)
    fbe1t = load("fbe1t", [128, 8], pool=work)
    sc1 = work.tile([128, 8], F32, tag="sc1")
    bi1 = work.tile([128, 8], F32, tag="bi1")
    nc.vector.tensor_mul(sc1[:], fg1t[:], g8[:, :, 1:2])
    nc.vector.tensor_mul(bi1[:], sc1[:], g8[:, :, 0:1])
    nc.vector.tensor_sub(bi1[:], fbe1t[:], bi1[:])
    r1Tr = big.tile([128, 8, SEGC], BF16, tag="r1Tr")
    for t in range(8):
        nc.scalar.activation(r1Tr[:, t, :], r1T[:, t, :], AF.Relu,
                             bias=bi1[:, t : t + 1], scale=sc1[:, t : t + 1])
    # FC2: out [32 segs, 256] complete (contraction over 1024 feats, local)
    ps_r2 = ps2.tile([SEGC, 256], F32, tag="r2", bufs=1)
    for t in range(8):
        nc.tensor.matmul(ps_r2[:], r1Tr[:, t, :], fw2sv[:, t, :],
                         start=(t == 0), stop=(t == 7))
    r2st = big.tile([SEGC, 512], BF16, tag="r2st")
    nc.scalar.copy(r2st[:, 0:256], ps_r2[:])
    nc.scalar.activation(r2st[:, 256:512], ps_r2[:], AF.Square)
    ps_s5 = ps2.tile([1, 512], F32, tag="s5", bufs=1)
    nc.tensor.matmul(ps_s5[:], ones32h[:], r2st[:], start=True, stop=True)
    s5 = work.tile([1, 512], F32, tag="s5sb")
    nc.scalar.copy(s5[:], ps_s5[:])
    nc.gpsimd.dma_start(d["ag5_i"][:], s5[:])
    nc.gpsimd.collective_compute(
        "AllGather", mybir.AluOpType.bypass, replica_groups=RG,
        ins=[d["ag5_i"][:]], outs=[d["ag5_o"][:]])
    agg5 = work.tile([8, 512], F32, tag="agg5")
    nc.sync.dma_start(agg5[:], d["ag5_o"][:])
    agg5h = work.tile([8, 512], BF16, tag="agg5h")
    nc.vector.tensor_copy(agg5h[:], agg5[:])
    ones8h = sing.tile([8, 1], BF16)
    nc.vector.memset(ones8h[:], 1.0)
    ps_g5 = ps2.tile([1, 512], F32, tag="g5", bufs=1)
    nc.tensor.matmul(ps_g5[:], ones8h[:], agg5h[:], start=True, stop=True)
    g5 = work.tile([1, 512], F32, tag="g5sb")
    nc.scalar.copy(g5[:], ps_g5[:])
    # scale/bias rows [1, 256] -> packed scb5 [1, 512] fp16 for PE broadcast
    eps1 = work.tile([1, 1], F32, tag="eps1")
    nc.vector.memset(eps1[:], EPS_BN)
    nc.scalar.mul(g5[:, 0:256], g5[:, 0:256], 1.0 / B)
    nc.scalar.mul(g5[:, 256:512], g5[:, 256:512], 1.0 / B)
    m2r = work.tile([1, 256], F32, tag="m2r")
    nc.vector.tensor_mul(m2r[:], g5[:, 0:256], g5[:, 0:256])
    nc.vector.tensor_sub(g5[:, 256:512], g5[:, 256:512], m2r[:])
    nc.scalar.activation(g5[:, 256:512], g5[:, 256:512], AF.Sqrt, bias=eps1[:])
    nc.vector.reciprocal(g5[:, 256:512], g5[:, 256:512])
    fg2r = load("fg2r", [1, 256], pool=work)
    fbe2r = load("fbe2r", [1, 256], pool=work)
    scb5 = work.tile([1, 512], BF16, tag="scb5")
    sc2f = work.tile([1, 256], F32, tag="sc2f")
    nc.vector.tensor_mul(sc2f[:], fg2r[:], g5[:, 256:512])
    nc.scalar.copy(scb5[:, 0:256], sc2f[:])
    bi2f = work.tile([1, 256], F32, tag="bi2f")
    nc.vector.tensor_mul(bi2f[:], sc2f[:], g5[:, 0:256])
    nc.vector.tensor_sub(bi2f[:], fbe2r[:], bi2f[:])
    nc.scalar.copy(scb5[:, 256:512], bi2f[:])
    ps_bc = ps2.tile([SEGC, 512], F32, tag="bc", bufs=1)
    nc.tensor.matmul(ps_bc[:], ones1x32h[:], scb5[:], start=True, stop=True)
    # apply BN2 + relu (per-column scale/bias via broadcast tiles)
    r2n = big.tile([SEGC, 256], BF16, tag="r2n")
    nc.vector.tensor_mul(r2n[:], r2st[:, 0:256], ps_bc[:, 0:256])
    nc.vector.tensor_add(r2n[:], r2n[:], ps_bc[:, 256:512])
    nc.vector.tensor_scalar_max(r2n[:], r2n[:], 0.0)
    # L2 normalize rows, write this core's [32, 256] slab
    nsq = work.tile([SEGC, 256], F32, tag="nsq")
    nc.scalar.activation(nsq[:], r2n[:], AF.Square)
    nrm = work.tile([SEGC, 1], F32, tag="nrm")
    nc.vector.reduce_sum(nrm[:], nsq[:], axis=mybir.AxisListType.X)
    nc.scalar.activation(nrm[:], nrm[:], AF.Sqrt)
    nc.vector.tensor_scalar_max(nrm[:], nrm[:], 1e-12)
    nc.vector.reciprocal(nrm[:], nrm[:])
    outf = big.tile([SEGC, 256], F32, tag="outf")
    nc.scalar.activation(outf[:], r2n[:], AF.Copy, scale=nrm[:])
    nc.sync.dma_start(d["out_final"][:], outf[:])

    ps2_cm.__exit__(None, None, None)
    work_cm.__exit__(None, None, None)
    big_cm.__exit__(None, None, None)
    sing_cm.__exit__(None, None, None)


# ------------------------------------------------------------------ host side
def _prep_core(x3, fw1, c):
    import ml_dtypes
    xs = x3[:, PL * c : PL * (c + 1), :]                       # [256,250,32]
    arr = np.ascontiguousarray(xs.transpose(2, 0, 1))          # [32,256,250]
    xA4 = arr.reshape(C, 4, QF).transpose(1, 0, 2).reshape(128, QF)
    xb = xs.reshape(B, 2, PH, C).transpose(2, 3, 1, 0)         # [125,32,2,256]
    xB = np.ascontiguousarray(xb).reshape(PH, C * 2 * B)
    fw = fw1.reshape(1024, P, C)[:, PL * c : PL * (c + 1), :]
    fw = fw.reshape(1024, 2, PH, C).transpose(2, 3, 1, 0)      # [125,32,2,1024]
    fw1t = np.ascontiguousarray(fw).reshape(PH, C * 2, 1024)
    bf = np.float16
    f8 = ml_dtypes.float8_e3m4
    return (np.ascontiguousarray(xA4).astype(bf), xB.astype(bf),
            (fw1t * 64.0).astype(f8))


def _qrep(v, rows):
    out = np.zeros((128, 1), np.float32)
    for a in range(4):
        out[32 * a : 32 * a + rows, 0] = v
    return out


def _wdiag(w):
    """w [out,in] -> block-diagonal lhsT [128, 128]: block a (32x32) holds
    w.T in its top-left corner."""
    t = np.zeros((128, 128), np.float32)
    wt = w.T  # [in, out]
    for a in range(4):
        t[32 * a : 32 * a + wt.shape[0], 32 * a : 32 * a + wt.shape[1]] = wt
    return t


def _w2k(w2):
    """Four column-shifted L2 lhsT variants: variant k maps quarter a's
    outputs to partition rows 32a+8k..+8, so four point-chunks stack into
    one psum tile."""
    out = np.zeros((128, 4, 128), np.float32)
    wt = w2.T  # [16, 8]
    for k in range(4):
        for a in range(4):
            out[32 * a : 32 * a + 16, k,
                32 * a + 8 * k : 32 * a + 8 * k + 8] = wt
    return out.reshape(128, 512)


def _w3k(w3):
    """L3 lhsT on the packed (a,k,c) row space: block (a,k) contracts rows
    32a+8k+c into the single score row 32a+8k."""
    t = np.zeros((128, 128), np.float32)
    wt = w3.T  # [8, 1]
    for a in range(4):
        for k in range(4):
            r = 32 * a + 8 * k
            t[r : r + 8, r : r + 1] = wt
    return t


def _rep8(v):
    """Per-partition vector on the packed (a,k,c) rows: row 32a+8k+c = v[c]."""
    out = np.zeros((128, 1), np.float32)
    for b in range(16):
        out[8 * b : 8 * b + 8, 0] = v
    return out


def kernel(**inputs):
    import ml_dtypes

    if "nc" not in _cache:
        _cache["nc"] = _build()
    nc = _cache["nc"]
    bf = np.float16

    g = {k: np.asarray(v, np.float32) for k, v in inputs.items()
         if k != "length"}
    x3 = g["x"].reshape(B, P, C)

    f1 = np.zeros((128, 16), np.float32)
    f2 = np.zeros((128, 8), np.float32)
    for a in range(4):
        f1[32 * a : 32 * a + 16, :] = np.eye(16, dtype=np.float32)
    for b in range(16):
        f2[8 * b : 8 * b + 8, :] = np.eye(8, dtype=np.float32)
    f8_16 = np.zeros((128, 16), np.float32)
    f8_8 = np.zeros((64, 8), np.float32)
    for k in range(8):
        f8_16[16 * k : 16 * k + 16, :] = np.eye(16, dtype=np.float32)
        f8_8[8 * k : 8 * k + 8, :] = np.eye(8, dtype=np.float32)

    shared = {
        "w1D": _wdiag(g["w1"]).astype(bf),
        "w2K": _w2k(g["w2"]).astype(bf),
        "w3D": _w3k(g["w3"]).astype(bf),
        "g1q": _qrep(g["g1"], 16), "be1q": _qrep(g["be1"], 16),
        "g2q": _rep8(g["g2"]), "be2q": _rep8(g["be2"]),
        "g3s": g["g3"].reshape(1, 1), "be3s": g["be3"].reshape(1, 1),
        "f1": f1, "ft1": np.ascontiguousarray(f1.T),
        "f2": f2, "ft2": np.ascontiguousarray(f2.T),
        "f8_16": f8_16, "f8_8": f8_8,
        "fw2s": np.ascontiguousarray(
            g["fw2"].reshape(256, 8, 128).transpose(2, 1, 0).reshape(
                128, 8 * 256)).astype(bf),
        "fg1t": np.ascontiguousarray(g["fg1"].reshape(8, 128).T),
        "fbe1t": np.ascontiguousarray(g["fbe1"].reshape(8, 128).T),
        "fg2r": g["fg2"].reshape(1, 256),
        "fbe2r": g["fbe2"].reshape(1, 256),
    }

    in_maps = []
    for c in range(NCORES):
        xA4, xB, fw1t = _prep_core(x3, g["fw1"], c)
        m = dict(shared)
        m["xA4"] = xA4
        m["xB"] = xB
        m["fw1t"] = fw1t
        in_maps.append(m)

    from concourse.bass_utils import run_bass_kernel_spmd

    res = run_bass_kernel_spmd(nc, in_maps, core_ids=list(range(NCORES)),
                               trace=bool(_cache.get("trace")))
    _cache["last_result"] = res
    return np.concatenate(
        [np.asarray(res.results[c]["out_final"], np.float32)
         for c in range(NCORES)], axis=0)


if __name__ == "__main__":
    nc = _build()
    print("build ok; instructions:",
          sum(len(bb.instructions) for bb in nc.main_func.blocks))
